# revision 1
# baseline (speedup 1.0000x reference)
"""Trainium2 Bass kernel for nn_CrossAtt_27711128994442.

Dual cross-attention block: two branches of channel-attention
(softmax(k @ q^T) applied to v) with a sigmoid gate + residual, concat,
3x3 conv (1024 -> 512), training-mode BatchNorm, ReLU.

Sharding: data-parallel over batch (B=8 -> 8 NeuronCores, one batch
element per core).  BatchNorm statistics are all-reduced across the 8
cores (per-channel sum / sum-of-squares, one [128,8] AllGather).

Math notes (per core / batch element, x1 = input1[b], x2 = input2[b],
both [C=512, N=4096]):
  branch1: S1 = (wk1 x1) (wq2 x2)^T = wk1 G wq2^T where G = x1 x2^T
  branch2: S2 = (wk2 x2) (wq1 x1)^T = wk2 G^T wq1^T
so one Gram matrix G serves both branches.  G and the two small [512^3]
"sandwich" matmuls run in float32r; v / attn^T v run in bf16.

The 3x3 conv runs as Winograd F(2x2,3x3): 2.25x fewer PE MACs than
direct.  Weights are transformed on the host (U = G g G^T, bf16); the
input transform (V = B^T d B over 4x4 patches, stride 2) runs on the
vector engine in two stages against the zero-padded [C,66,66] images;
the 16 per-position matmuls accumulate over input channels in PSUM;
the output inverse transform (A^T M A) runs on vector+scalar engines.
BatchNorm stats ride on the inverse-transform output; one combined
AllGather at the end; normalize+ReLU+writeout repacks the block layout
back to row-major via strided ACT ops.

The per-branch sigmoid gate is folded into the ZT copy (it is a pure
scalar per branch), so the softmax -> apply chain never waits on the
pooled-mean reduction.
"""

import os
import numpy as np
import ml_dtypes

import concourse.bass as bass
import concourse.mybir as mybir
import concourse.bacc as bacc
import concourse.tile as tile
from concourse import bass_utils

BF16 = ml_dtypes.bfloat16
F32 = mybir.dt.float32
F32R = mybir.dt.float32r
BF = mybir.dt.bfloat16

N_CORES = 8
B, C, OUT, H, W = 8, 512, 512, 64, 64
N = H * W            # 4096
CB = C // 128        # 4 channel chunks
NT = N // 512        # 8 spatial tiles of 512 (8 image rows each)
NCH = N // 128       # 32 contraction chunks for the Gram matrix
BN_EPS = 1e-5

# Winograd geometry: 32x32 grid of 2x2 output tiles; 4 sp chunks of
# 8 tile-rows (16 image rows) each.
NSP = 4
TR = 8               # tile-rows per sp chunk
TT = TR * 32         # tiles per sp chunk (256)

_CACHE = {}


def _emit(nc, tc, dr):
    """Emit the whole per-core program. dr: dict of DRAM APs."""
    AX = mybir.AxisListType

    with tc.tile_pool(name="pads", bufs=1) as pads_pool:

        # padded conv-input images, [128, 8src, 66, 66] bf16 (src 0-3 =
        # branch-1 output chunks, 4-7 = branch-2)
        pad_all = pads_pool.tile([128, 8, 66, 66], BF, tag="pad_all", name="pad_all")
        # zero only the borders; interior is fully overwritten later
        nc.vector.memset(pad_all[:, :, 0, :], 0.0)
        nc.vector.memset(pad_all[:, :, 65, :], 0.0)
        nc.vector.memset(pad_all[:, :, 1:65, 0], 0.0)
        nc.vector.memset(pad_all[:, :, 1:65, 65], 0.0)

        with tc.tile_pool(name="pwv", bufs=1) as pwv:
            # attention probability tiles (1/rowsum folded in), per branch
            P1 = [pwv.tile([128, 512], BF, tag=f"P1_{kb}", name=f"P1_{kb}") for kb in range(CB)]
            P2 = [pwv.tile([128, 512], BF, tag=f"P2_{kb}", name=f"P2_{kb}") for kb in range(CB)]
            # v-projection weights (transposed: [ci, vc]) bf16
            wv1 = [pwv.tile([128, 512], BF, tag=f"wv1_{cb}", name=f"wv1_{cb}") for cb in range(CB)]
            wv2 = [pwv.tile([128, 512], BF, tag=f"wv2_{cb}", name=f"wv2_{cb}") for cb in range(CB)]
            # per-branch gate scalars broadcast to 128 partitions
            abc1 = pwv.tile([128, 1], F32, tag="abc1", name="abc1")
            abc2 = pwv.tile([128, 1], F32, tag="abc2", name="abc2")

            with tc.tile_pool(name="xh", bufs=1) as pers:
                # natural-layout bf16 activations (for v, residual): [128, 4096] x4
                x1h = [pers.tile([128, 4096], BF, tag=f"x1h_{cb}", name=f"x1h_{cb}") for cb in range(CB)]
                x2h = [pers.tile([128, 4096], BF, tag=f"x2h_{cb}", name=f"x2h_{cb}") for cb in range(CB)]

                # ------------ Phase A1: Gram matrix, logits, softmax, gates ----
                with tc.tile_pool(name="a1sb", bufs=1) as a1sb, \
                     tc.tile_pool(name="xt", bufs=3) as xtp, \
                     tc.tile_pool(name="wkp", bufs=1) as wkp:

                    ones = a1sb.tile([128, 128], F32R, tag="ones", name="ones")
                    nc.sync.dma_start(ones[:], dr["ones"][:])
                    ident = a1sb.tile([128, 128], F32R, tag="ident", name="ident")
                    nc.sync.dma_start(ident[:], dr["ident"][:])

                    # --- G accumulation ---
                    with tc.tile_pool(name="gps", bufs=1, space="PSUM") as gps:
                        G_ps = [gps.tile([128, 512], F32, tag=f"G_{cb}", name=f"G_{cb}") for cb in range(CB)]
                        for i in range(NCH):
                            t1_ = xtp.tile([128, 512], F32R, tag="x1t", name="x1t")
                            t2_ = xtp.tile([128, 512], F32R, tag="x2t", name="x2t")
                            nc.sync.dma_start(t1_[:], dr["x1t"][i * 128:(i + 1) * 128, :])
                            nc.sync.dma_start(t2_[:], dr["x2t"][i * 128:(i + 1) * 128, :])
                            st = dict(start=(i == 0), stop=(i == NCH - 1))
                            for cb in range(CB):
                                nc.tensor.matmul(G_ps[cb][:], t1_[:, cb * 128:(cb + 1) * 128], t2_[:], **st)

                        G_sb = [a1sb.tile([128, 512], F32R, tag=f"Gsb_{cb}", name=f"Gsb_{cb}") for cb in range(CB)]
                        for cb in range(CB):
                            nc.vector.tensor_copy(G_sb[cb][:], G_ps[cb][:])

                    # all sandwich weights ride in recycled xt-pool slots; the
                    # FIFO slot rotation sequences their DMAs behind the G tail
                    # in consumption order (M2 -> M1 -> S2 -> S1)
                    wq_b2 = [xtp.tile([128, 512], F32R, tag="x1t", name=f"wqb2_{cb}") for cb in range(CB)]
                    wq_b1 = [xtp.tile([128, 512], F32R, tag="x2t", name=f"wqb1_{cb}") for cb in range(CB)]
                    wk_b2 = [wkp.tile([128, 512], F32R, tag=f"wkb2_{cb}", name=f"wkb2_{cb}") for cb in range(CB)]
                    wk_b1 = [xtp.tile([128, 512], F32R, tag="x2t", name=f"wkb1_{cb}") for cb in range(CB)]
                    for cb in range(CB):
                        cs = slice(cb * 128, (cb + 1) * 128)
                        nc.sync.dma_start(wq_b2[cb][:], dr["wq1t"][cs, :])
                        nc.sync.dma_start(wq_b1[cb][:], dr["wq2t"][cs, :])
                    for cb in range(CB):
                        cs = slice(cb * 128, (cb + 1) * 128)
                        nc.sync.dma_start(wk_b2[cb][:], dr["wk2t"][cs, :])
                        nc.sync.dma_start(wk_b1[cb][:], dr["wk1t"][cs, :])

                    # v-weights land before the bulk x-hi loads: the ZT
                    # matmuls need them right after the softmax
                    for cb in range(CB):
                        nc.sync.dma_start(wv2[cb][:], dr["wv2n"][cb * 128:(cb + 1) * 128, :])
                        nc.sync.dma_start(wv1[cb][:], dr["wv1n"][cb * 128:(cb + 1) * 128, :])
                    for cb in range(CB):
                        nc.sync.dma_start(x2h[cb][:], dr["x2h"][cb * 128:(cb + 1) * 128, :])
                        nc.sync.dma_start(x1h[cb][:], dr["x1h"][cb * 128:(cb + 1) * 128, :])

                    # --- transpose G -> GT (for branch 1) ---
                    GT_sb = [a1sb.tile([128, 512], F32R, tag=f"GTsb_{cb}", name=f"GTsb_{cb}") for cb in range(CB)]
                    with tc.tile_pool(name="trp", bufs=2, space="PSUM") as trp:
                        for c2b in range(CB):
                            for c1b in range(CB):
                                tp = trp.tile([128, 128], F32R, tag="tr", name="tr")
                                nc.tensor.transpose(tp[:], G_sb[c1b][:, c2b * 128:(c2b + 1) * 128], ident[:])
                                nc.vector.tensor_copy(GT_sb[c2b][:, c1b * 128:(c1b + 1) * 128], tp[:])

                    # pooled sums for the gates start here on DVE (overlap the
                    # sandwiches); the tiny gate matmuls are emitted AFTER the
                    # S matmuls so they never head-of-line-block the PE FIFO.
                    wlc = a1sb.tile([128, CB], F32, tag="wlc", name="wlc")
                    nc.sync.dma_start(wlc[:], dr["wlinc"][:])
                    onesb = a1sb.tile([128, 2], BF, tag="onesb", name="onesb")
                    nc.vector.tensor_copy(onesb[:], ones[:, 0:2])
                    # --- branch sandwiches + exp ---
                    # branch 1: S1 = wk1 (G wq2^T)   via lhsT=GT, then lhsT=wk1t
                    # branch 2: S2 = wk2 (G^T wq1^T) via lhsT=G,  then lhsT=wk2t
                    rs_all = {}
                    branches = [(G_sb, wq_b2, wk_b2, P2), (GT_sb, wq_b1, wk_b1, P1)]
                    M_sbs = {}
                    with tc.tile_pool(name="msps", bufs=1, space="PSUM") as msps:
                        for bi, (Gl, wq, wk, Pt) in enumerate(branches):
                            M_ps = [msps.tile([128, 512], F32, tag=f"b{bi}_{cb}", name=f"M{bi}_{cb}") for cb in range(CB)]
                            for cb in range(CB):
                                for kb in range(CB):
                                    nc.tensor.matmul(M_ps[cb][:], Gl[kb][:, cb * 128:(cb + 1) * 128],
                                                     wq[kb][:], start=(kb == 0), stop=(kb == CB - 1))
                            M_sb = [a1sb.tile([128, 512], F32R, tag=f"Msb{bi}_{cb}", name=f"Msb{bi}_{cb}") for cb in range(CB)]
                            for cb in range(CB):
                                nc.vector.tensor_copy(M_sb[cb][:], M_ps[cb][:])
                            M_sbs[bi] = M_sb
                        # S tiles reuse the same tags as the M banks they replace
                        for bi, (Gl, wq, wk, Pt) in enumerate(branches):
                            M_sb = M_sbs[bi]
                            S_ps = [msps.tile([128, 512], F32, tag=f"b{bi}_{kb}", name=f"S{bi}_{kb}") for kb in range(CB)]
                            for kb in range(CB):
                                for cb in range(CB):
                                    nc.tensor.matmul(S_ps[kb][:], wk[cb][:, kb * 128:(kb + 1) * 128],
                                                     M_sb[cb][:], start=(cb == 0), stop=(cb == CB - 1))
                            for kb in range(CB):
                                nmx = a1sb.tile([128, 1], F32, tag="nmx", name="nmx", bufs=2)
                                nc.vector.reduce_max(nmx[:], S_ps[kb][:], axis=AX.X, negate=True)
                                rs = a1sb.tile([128, 1], F32, tag=f"rs{bi}_{kb}", name=f"rs{bi}_{kb}")
                                nc.scalar.activation(Pt[kb][:], S_ps[kb][:],
                                                     mybir.ActivationFunctionType.Exp,
                                                     bias=nmx[:], accum_out=rs[:])
                                rs_all[(bi, kb)] = rs

                    # pooled sums via ACT accum_out, emitted after the exp
                    # ops so the x-DMA wait never head-of-line blocks the
                    # softmax on any engine; the dot with w_lin is tiny DVE.
                    prs_all = {}
                    for bi, xh in [(1, x2h), (0, x1h)]:
                        pph = a1sb.tile([128, 2, CB], F32, tag=f"pph{bi}", name=f"pph{bi}")
                        for cb in range(CB):
                            for h in range(2):
                                psc = a1sb.tile([128, 2048], BF, tag="poolscr", name="poolscr", bufs=1)
                                nc.scalar.activation(psc[:], xh[cb][:, h * 2048:(h + 1) * 2048],
                                                     mybir.ActivationFunctionType.Copy,
                                                     accum_out=pph[:, h, cb:cb + 1])
                        pp = a1sb.tile([128, CB], F32, tag=f"pp{bi}", name=f"pp{bi}")
                        nc.vector.tensor_add(pp[:], pph[:, 0], pph[:, 1])
                        pr_ = a1sb.tile([128, CB], F32, tag=f"pr{bi}", name=f"pr{bi}")
                        nc.vector.tensor_mul(pr_[:], pp[:], wlc[:])
                        prs = a1sb.tile([128, 1], BF, tag=f"prs{bi}", name=f"prs{bi}")
                        with nc.allow_low_precision(reason="gate dot, fp32 psum accum"):
                            nc.vector.reduce_sum(prs[:], pr_[:], axis=AX.X)
                        prs_all[bi] = prs

                    # --- gate finalization (tiny matmuls, after S) ---
                    with tc.tile_pool(name="bcp", bufs=2, space="PSUM") as bcp:
                        for bi, abc in [(1, abc2), (0, abc1)]:
                            d_ps = bcp.tile([128, 512], F32, tag="dps", name="dps")
                            nc.tensor.matmul(d_ps[0:1, 0:2], prs_all[bi][:], onesb[:], start=True, stop=True)
                            av = a1sb.tile([1, 2], F32R, tag="av", name="av")
                            nc.scalar.activation(av[:], d_ps[0:1, 0:1].to_broadcast((1, 2)),
                                                 mybir.ActivationFunctionType.Sigmoid,
                                                 scale=1.0 / float(N))
                            bc_ps = bcp.tile([128, 512], F32, tag="bc", name="bc")
                            nc.tensor.matmul(bc_ps[:, 0:2], ones[0:1, :], av[:], start=True, stop=True)
                            nc.vector.tensor_copy(abc[:], bc_ps[:, 0:1])

                    # fold 1/rowsum into P (gate folds into ZT later)
                    for gbi, Pt in enumerate([P2, P1]):
                        for kb in range(CB):
                            rs = rs_all[(gbi, kb)]
                            ri = a1sb.tile([128, 1], F32, tag="ri", name="ri", bufs=2)
                            nc.vector.reciprocal(ri[:], rs[:])
                            nc.vector.tensor_scalar_mul(Pt[kb][:], Pt[kb][:], ri[:])

                # ------------ Phase A2: out = (wv^T P)^T x + resid, pad write ---
                # re-associated: ZT[ci,c] = sum_kc wv[kc,ci] P[kc,c]  (512^3, tiny)
                # then out[c,n] = sum_ci ZT[ci,c] x[ci,n]; gate & 1/rowsum live
                # in ZT / P respectively.  P2 branch first: its softmax
                # finishes earlier, so its apply overlaps branch 1's softmax.
                with tc.tile_pool(name="zsb", bufs=1) as zsbp, \
                     tc.tile_pool(name="zps", bufs=1, space="PSUM") as zps, \
                     tc.tile_pool(name="ops", bufs=1, space="PSUM") as ops:
                    for br_i, (Pt, wv, xh, abc) in enumerate([(P2, wv2, x2h, abc2),
                                                             (P1, wv1, x1h, abc1)]):
                        pad_base = 4 if br_i == 0 else 0   # pad2 = src 4-7
                        ZT_sb = []
                        for cib in range(CB):
                            z_ps = zps.tile([128, 512], F32, tag=f"zps_{cib}", name=f"zps_{cib}")
                            for kb in range(CB):
                                nc.tensor.matmul(z_ps[:], wv[kb][:, cib * 128:(cib + 1) * 128],
                                                 Pt[kb][:], start=(kb == 0), stop=(kb == CB - 1))
                            zt = zsbp.tile([128, 512], BF, tag=f"zt_{cib}", name=f"zt_{cib}")
                            # gate folded here: zt = z_ps * a
                            nc.vector.tensor_scalar_mul(zt[:], z_ps[:], abc[:])
                            ZT_sb.append(zt)
                        for cb in range(CB):
                            for nt in range(NT):
                                ns = slice(nt * 512, (nt + 1) * 512)
                                o_ps = ops.tile([128, 512], F32, tag=f"ops_{nt % 4}", name=f"ops_{br_i}_{cb}_{nt}")
                                for cib in range(CB):
                                    nc.tensor.matmul(o_ps[:], ZT_sb[cib][:, cb * 128:(cb + 1) * 128],
                                                     xh[cib][:, ns], start=(cib == 0), stop=(cib == CB - 1))
                                nc.vector.tensor_add(
                                    pad_all[:, pad_base + cb, 1 + nt * 8:9 + nt * 8, 1:65],
                                    o_ps[:].rearrange("p (a b) -> p a b", a=8),
                                    xh[cb][:, ns].rearrange("p (a b) -> p a b", a=8))

        # ------------ Phase B: Winograd F(2x2,3x3) conv + BN ----------------
        with tc.tile_pool(name="ybp", bufs=1) as ybp, \
             tc.tile_pool(name="bsb", bufs=1) as bsb, \
             tc.tile_pool(name="dram", bufs=1, space="DRAM") as dram:

            # conv output in Winograd block layout: [128, ocb, sp, r, j, 256]
            yb_all = ybp.tile([128, CB, NSP, 2, 2, TT], BF, tag="yb_all", name="yb_all")

            stats = bsb.tile([128, 2 * CB], F32, tag="stats", name="stats")
            nc.vector.memset(stats[:], 0.0)

            with tc.tile_pool(name="rp", bufs=1) as rp, \
                 tc.tile_pool(name="vp", bufs=2) as vp, \
                 tc.tile_pool(name="up", bufs=2) as up, \
                 tc.tile_pool(name="t1p", bufs=2) as t1p, \
                 tc.tile_pool(name="map", bufs=2) as map_, \
                 tc.tile_pool(name="mps", bufs=2, space="PSUM") as mps:

              # k index = pr*8 + src (matches the host U layout)
              def emit_stage1(sp):
                """rows transform: R_all[128, 32k, TR, 66] bf16 (DVE)"""
                r0 = 16 * sp
                rt = rp.tile([128, 32, TR, 66], BF, tag="R_all", name=f"R_all_{sp}")
                a = pad_all[:, :, r0 + 0:r0 + 16:2, :]
                b = pad_all[:, :, r0 + 2:r0 + 18:2, :]
                c = pad_all[:, :, r0 + 1:r0 + 17:2, :]
                d = pad_all[:, :, r0 + 3:min(r0 + 19, 66):2, :]
                nc.vector.tensor_sub(rt[:, 0:8], a, b)
                nc.vector.tensor_add(rt[:, 8:16], c, b)
                nc.vector.tensor_sub(rt[:, 16:24], b, c)
                nc.vector.tensor_sub(rt[:, 24:32], c, d)
                return rt

              GP_K = 10  # k-slices of stage 2 offloaded to GPSIMD
              def emit_stage2(rt, sp, pc):
                """cols transform: V_all[128, 32k, TT] bf16 (DVE + GPSIMD)"""
                vt = vp.tile([128, 32, TT], BF, tag="V_all", name=f"V_all_{sp}_{pc}")
                vv = vt.rearrange("p k (a b) -> p k a b", a=TR)
                e = rt[:, :, :, 0:64:2]
                m = rt[:, :, :, 1:65:2]
                q = rt[:, :, :, 2:66:2]
                s = rt[:, :, :, 3:66:2]
                lo = slice(0, 32 - GP_K)
                hi = slice(32 - GP_K, 32)
                if pc == 0:
                    nc.vector.tensor_sub(vv[:, lo], e[:, lo], q[:, lo])
                    nc.gpsimd.tensor_sub(vv[:, hi], e[:, hi], q[:, hi])
                elif pc == 1:
                    nc.vector.tensor_add(vv[:, lo], m[:, lo], q[:, lo])
                    nc.gpsimd.tensor_add(vv[:, hi], m[:, hi], q[:, hi])
                elif pc == 2:
                    nc.vector.tensor_sub(vv[:, lo], q[:, lo], m[:, lo])
                    nc.gpsimd.tensor_sub(vv[:, hi], q[:, hi], m[:, hi])
                else:
                    nc.vector.tensor_sub(vv[:, lo], m[:, lo], s[:, lo])
                    nc.gpsimd.tensor_sub(vv[:, hi], m[:, hi], s[:, hi])
                return vt

              phases = [(sp, pc) for sp in range(NSP) for pc in range(4)]
              R = emit_stage1(0)
              V = emit_stage2(R, 0, 0)
              for idx, (sp, pc) in enumerate(phases):
                    Vcur = V
                    # ---- PE: the 16-position matmuls for this (sp, pc) ----
                    mts = []
                    for pair in range(2):
                        # M PSUM for an ocb pair: [128, 4pr, 2x256] f32
                        mt = mps.tile([128, 4, 2 * TT], F32, tag="mt", name=f"mt_{sp}_{pc}_{pair}")
                        for half in range(2):
                            ocb = pair * 2 + half
                            u = up.tile([128, 32 * 128], BF, tag="u", name=f"u_{sp}_{pc}_{ocb}")
                            nc.sync.dma_start(u[:], dr["uw"][pc * 4 + ocb])
                            hs = slice(half * TT, (half + 1) * TT)
                            for icb in range(8):
                                st = dict(start=(icb == 0), stop=(icb == 7))
                                for pr in range(4):
                                    nc.tensor.matmul(mt[:, pr, hs],
                                                     u[:, (pr * 8 + icb) * 128:(pr * 8 + icb + 1) * 128],
                                                     Vcur[:, pr * 8 + icb, :], **st)
                        mts.append(mt)

                    # ---- DVE: pre-emit NEXT phase transforms (FIFO order) ----
                    if idx + 1 < len(phases):
                        sp2, pc2 = phases[idx + 1]
                        if pc2 == 0:
                            R = emit_stage1(sp2)
                        V = emit_stage2(R, sp2, pc2)

                    # ---- inverse transforms for this phase ----
                    for pair in range(2):
                        mt = mts[pair]
                        ph = slice(pair * 2, pair * 2 + 2)
                        # rows (invA): PSUM reads all on ACT, adds on DVE (bf16 2x)
                        m0 = map_.tile([128, 2 * TT], BF, tag="m0", name=f"m0_{sp}_{pc}_{pair}")
                        m1 = map_.tile([128, 2 * TT], BF, tag="m1", name=f"m1_{sp}_{pc}_{pair}")
                        m2 = map_.tile([128, 2 * TT], BF, tag="m2", name=f"m2_{sp}_{pc}_{pair}")
                        m3 = map_.tile([128, 2 * TT], BF, tag="m3", name=f"m3_{sp}_{pc}_{pair}")
                        nc.scalar.activation(m0[:], mt[:, 0, :], mybir.ActivationFunctionType.Copy)
                        nc.scalar.activation(m1[:], mt[:, 1, :], mybir.ActivationFunctionType.Copy)
                        nc.scalar.activation(m2[:], mt[:, 2, :], mybir.ActivationFunctionType.Copy)
                        nc.scalar.activation(m3[:], mt[:, 3, :], mybir.ActivationFunctionType.Copy)
                        t1 = t1p.tile([128, 2, 2 * TT], BF, tag="t1", name=f"t1_{sp}_{pc}_{pair}")
                        nc.vector.tensor_add(t1[:, 0, :], m0[:], m1[:])
                        nc.vector.tensor_add(t1[:, 0, :], t1[:, 0, :], m2[:])
                        nc.vector.tensor_sub(t1[:, 1, :], m1[:], m2[:])
                        nc.vector.tensor_sub(t1[:, 1, :], t1[:, 1, :], m3[:])
                        # cols (invB): ops span the ocb pair (FD 512)
                        for r in range(2):
                            tr_ = t1[:, r, :].rearrange("p (o t) -> p o t", o=2)
                            y0 = yb_all[:, ph, sp, r, 0, :]
                            y1 = yb_all[:, ph, sp, r, 1, :]
                            if pc == 0:
                                nc.scalar.activation(y0, tr_, mybir.ActivationFunctionType.Copy)
                            elif pc == 1:
                                nc.vector.tensor_add(y0, y0, tr_)
                                nc.scalar.activation(y1, tr_, mybir.ActivationFunctionType.Copy)
                            elif pc == 2:
                                nc.vector.tensor_add(y0, y0, tr_)
                                nc.vector.tensor_sub(y1, y1, tr_)
                            else:
                                nc.vector.tensor_sub(y1, y1, tr_)

                    # ---- BN stats for this sp chunk (ACT accumulate passes) ----
                    if pc == 3:
                        for ob in range(CB):
                            ysl = yb_all[:, ob, sp].rearrange("p a b t -> p (a b t)")
                            ts = bsb.tile([128, 1], F32, tag="tsum", name="tsum", bufs=2)
                            sc1 = bsb.tile([128, 4 * TT], BF, tag="scr", name="scr", bufs=1)
                            nc.scalar.activation(sc1[:], ysl, mybir.ActivationFunctionType.Copy,
                                                 accum_out=ts[:])
                            tq = bsb.tile([128, 1], F32, tag="tsq", name="tsq", bufs=2)
                            sc2 = bsb.tile([128, 4 * TT], BF, tag="scr2", name="scr2", bufs=1)
                            nc.scalar.activation(sc2[:], ysl, mybir.ActivationFunctionType.Square,
                                                 accum_out=tq[:])
                            nc.vector.tensor_add(stats[:, 2 * ob:2 * ob + 1], stats[:, 2 * ob:2 * ob + 1], ts[:])
                            nc.vector.tensor_add(stats[:, 2 * ob + 1:2 * ob + 2], stats[:, 2 * ob + 1:2 * ob + 2], tq[:])

            # ---- one combined AllReduce of [128, 8] stats ----
            s_in = dram.tile([128, 2 * CB], F32, tag="arin", name="arin")
            s_out = dram.tile([128, 2 * CB], F32, tag="arout", name="arout")
            nc.sync.dma_start(s_in[:], stats[:])
            nc.gpsimd.collective_compute(
                "AllReduce", mybir.AluOpType.add,
                replica_groups=[list(range(N_CORES))],
                ins=[s_in.opt()], outs=[s_out.opt()])

            with tc.tile_pool(name="fin", bufs=1) as fin:
                sall = fin.tile([128, 2 * CB], F32, tag="sall", name="sall")
                nc.sync.dma_start(sall[:], s_out[:])

                gam = fin.tile([128, CB], F32, tag="gam", name="gam")
                bet = fin.tile([128, CB], F32, tag="bet", name="bet")
                nc.sync.dma_start(gam[:], dr["gamma"].rearrange("(c p) one -> p (c one)", p=128))
                nc.sync.dma_start(bet[:], dr["beta"].rearrange("(c p) one -> p (c one)", p=128))
                inv_n = 1.0 / float(B * N)
                eps_t = fin.tile([128, 1], F32, tag="eps", name="eps")
                nc.vector.memset(eps_t[:], BN_EPS)

                # ---- finalize per-ocb scale/shift, normalize + repack + out ----
                for ob in range(CB):
                    mean = fin.tile([128, 1], F32, tag="mean", name="mean", bufs=2)
                    nc.vector.tensor_scalar_mul(mean[:], sall[:, 2 * ob:2 * ob + 1], inv_n)
                    ex2 = fin.tile([128, 1], F32, tag="ex2", name="ex2", bufs=2)
                    nc.vector.tensor_scalar_mul(ex2[:], sall[:, 2 * ob + 1:2 * ob + 2], inv_n)
                    m2_ = fin.tile([128, 1], F32, tag="m2s", name="m2s", bufs=2)
                    nc.vector.tensor_mul(m2_[:], mean[:], mean[:])
                    var = fin.tile([128, 1], F32, tag="var", name="var", bufs=2)
                    nc.vector.tensor_sub(var[:], ex2[:], m2_[:])
                    std = fin.tile([128, 1], F32, tag="std", name="std", bufs=2)
                    nc.scalar.activation(std[:], var[:], mybir.ActivationFunctionType.Sqrt,
                                         bias=eps_t[:])
                    inv = fin.tile([128, 1], F32, tag="inv", name="inv", bufs=2)
                    nc.vector.reciprocal(inv[:], std[:])
                    sc = fin.tile([128, 1], F32, tag="sc", name="sc", bufs=2)
                    nc.vector.tensor_mul(sc[:], gam[:, ob:ob + 1], inv[:])
                    ms = fin.tile([128, 1], F32, tag="ms", name="ms", bufs=2)
                    nc.vector.tensor_mul(ms[:], mean[:], sc[:])
                    tt = fin.tile([128, 1], F32, tag="tt", name="tt", bufs=2)
                    nc.vector.tensor_sub(tt[:], bet[:, ob:ob + 1], ms[:])
                    # normalize + ReLU + repack block layout -> row-major
                    onat = fin.tile([128, 64, 64], F32, tag="onat", name="onat", bufs=2)
                    for sp in range(NSP):
                        for r in range(2):
                            src = yb_all[:, ob, sp, r, :, :]
                            dst = onat[:, 16 * sp + r:min(16 * sp + r + 16, 64):2, :]
                            dst = dst.rearrange("p a (b j) -> p j a b", j=2)
                            # dst dims (j, ti, tj) must match src scan order
                            nc.scalar.activation(dst, src.rearrange("p j (a b) -> p j a b", a=TR),
                                                 mybir.ActivationFunctionType.Relu,
                                                 bias=tt[:], scale=sc[:])
                    nc.sync.dma_start(dr["yout"][ob * 128:(ob + 1) * 128, :],
                                      onat.rearrange("p a b -> p (a b)"))



def _build():
    if "nc" in _CACHE:
        return _CACHE["nc"]
    nc = bacc.Bacc("TRN2", target_bir_lowering=False, debug=False,
                   num_devices=N_CORES)
    dr = {}
    def din(name, shape, dt):
        dr[name] = nc.dram_tensor(name, shape, dt, kind="ExternalInput").ap()
    din("x1t", [N, C], F32R)
    din("x2t", [N, C], F32R)
    din("x1h", [C, N], BF)
    din("x2h", [C, N], BF)
    for w in ["wq1t", "wq2t", "wk1t", "wk2t"]:
        din(w, [C, C], F32R)
    for w in ["wv1n", "wv2n"]:
        din(w, [C, C], BF)
    din("wlinc", [128, CB], F32)
    din("uw", [16, 128, 32 * 128], BF)
    din("gamma", [OUT, 1], F32)
    din("beta", [OUT, 1], F32)
    din("ident", [128, 128], F32R)
    din("ones", [128, 128], F32R)
    dr["yout"] = nc.dram_tensor("yout", [OUT, N], F32, kind="ExternalOutput").ap()

    with tile.TileContext(nc) as tc:
        _emit(nc, tc, dr)
    nc.compile()
    _CACHE["nc"] = nc
    return nc


def _prep_in_maps(inputs):
    f32 = np.float32
    x1 = np.ascontiguousarray(inputs["input1"], f32).reshape(B, C, N)
    x2 = np.ascontiguousarray(inputs["input2"], f32).reshape(B, C, N)
    shared = {}
    for w in ["wq1", "wq2", "wk1", "wk2"]:
        shared[w + "t"] = np.ascontiguousarray(np.asarray(inputs[w], f32).T)
    for w in ["wv1", "wv2"]:
        shared[w + "n"] = np.ascontiguousarray(np.asarray(inputs[w], f32).astype(BF16))
    shared["wlinc"] = np.ascontiguousarray(np.asarray(inputs["w_lin"], f32).reshape(CB, 128).T)
    # Winograd weight transform on host: U[pr,pc][ic,oc] = G g G^T
    g = np.asarray(inputs["w_cat"], f32)                     # [OUT, 2C, 3, 3]
    Gm = np.array([[1, 0, 0], [0.5, 0.5, 0.5], [0.5, -0.5, 0.5], [0, 0, 1]], f32)
    U = np.einsum('rj,oijk,ck->rcio', Gm, g, Gm)             # [4,4,2C,OUT]
    # layout: uw[pc*4+ocb][ic_in_chunk][pr, icb, oc] as [16, 128, 4096]
    U6 = U.reshape(4, 4, 8, 128, 4, 128)                     # [pr,pc,icb,i,ocb,o]
    uw = np.ascontiguousarray(U6.transpose(1, 4, 3, 0, 2, 5).reshape(4, 4, 128, 32 * 128))
    # uw dims now [pc, ocb, i, (pr icb o)]
    shared["uw"] = np.ascontiguousarray(uw.reshape(16, 128, 32 * 128).astype(BF16))
    shared["gamma"] = np.ascontiguousarray(np.asarray(inputs["bn_gamma"], f32).reshape(OUT, 1))
    shared["beta"] = np.ascontiguousarray(np.asarray(inputs["bn_beta"], f32).reshape(OUT, 1))
    shared["ident"] = np.eye(128, dtype=f32)
    shared["ones"] = np.ones((128, 128), f32)

    in_maps = []
    for b in range(B):
        m = dict(shared)
        m["x1t"] = np.ascontiguousarray(x1[b].T)
        m["x2t"] = np.ascontiguousarray(x2[b].T)
        m["x1h"] = np.ascontiguousarray(x1[b].astype(BF16))
        m["x2h"] = np.ascontiguousarray(x2[b].astype(BF16))
        in_maps.append(m)
    return in_maps


def run(inputs, trace=False):
    nc = _build()
    in_maps = _prep_in_maps(inputs)
    res = bass_utils.run_bass_kernel_spmd(nc, in_maps, list(range(N_CORES)),
                                          trace=trace)
    out = np.stack([res.results[b]["yout"] for b in range(B)])
    return out.reshape(B, OUT, H, W).astype(np.float32), res


def kernel(**inputs):
    out, _ = run(inputs, trace=bool(int(os.environ.get("BASS_KERNEL_TRACE", "0"))))
    return out



# revision 7
# speedup vs baseline: 1.1159x; 1.1159x over previous
"""Trainium2 Bass kernel for nn_CrossAtt_27711128994442.

Dual cross-attention block: two branches of channel-attention
(softmax(k @ q^T) applied to v) with a sigmoid gate + residual, concat,
3x3 conv (1024 -> 512), training-mode BatchNorm, ReLU.

Sharding: data-parallel over batch (B=8 -> 8 NeuronCores, one batch
element per core).  BatchNorm statistics are all-reduced across the 8
cores in two rounds (sp 0-2 early / sp 3 late) so the first collective
acts as a barrier that removes core skew from the second.

Math notes (per core / batch element, x1 = input1[b], x2 = input2[b],
both [C=512, N=4096]):
  branch1: S1 = (wk1 x1) (wq2 x2)^T = wk1 G wq2^T where G = x1 x2^T
  branch2: S2 = (wk2 x2) (wq1 x1)^T = wk2 G^T wq1^T
so one Gram matrix G serves both branches.  The pooled-mean gate sums
ride on the PE as ones-vector matmuls during the (DMA-bound) G phase.
The residual is folded into the value-projection product:
  out = (a ZT + I) x   with ZT = wv^T P,
so the attention apply writes the conv pad directly (pure copies, split
between ACT and DVE).

The 3x3 conv runs as Winograd F(2x2,3x3).  The conv pad is stored with
even/odd image columns deinterleaved ([*, 66, 2, 33]) so both input-
transform stages read/write stride-1 bf16 and hit the DVE 2x perf mode.
The 16 per-position matmuls accumulate over input channels in PSUM; the
output inverse transform (A^T M A) runs on vector+scalar engines.
BatchNorm stats ride on the inverse-transform output; the final
normalize+ReLU repacks the block layout to row-major (ACT for 3 output
chunks, DVE for 1) with per-(chunk, sp) output DMA.
"""

import os
import numpy as np
import ml_dtypes

import concourse.bass as bass
import concourse.mybir as mybir
import concourse.bacc as bacc
import concourse.tile as tile
from concourse import bass_utils

BF16 = ml_dtypes.bfloat16
F32 = mybir.dt.float32
F32R = mybir.dt.float32r
BF = mybir.dt.bfloat16

N_CORES = 8
B, C, OUT, H, W = 8, 512, 512, 64, 64
N = H * W            # 4096
CB = C // 128        # 4 channel chunks
NT = N // 512        # 8 spatial tiles of 512 (8 image rows each)
NCH = N // 128       # 32 contraction chunks for the Gram matrix
BN_EPS = 1e-5

# Winograd geometry: 32x32 grid of 2x2 output tiles; 4 sp chunks of
# 8 tile-rows (16 image rows) each.
NSP = 4
TR = 8               # tile-rows per sp chunk
TT = TR * 32         # tiles per sp chunk (256)

XH_BUFS = 24         # streaming x-hi tiles resident (3 nt of lookahead)

_CACHE = {}


def _emit(nc, tc, dr):
    """Emit the whole per-core program. dr: dict of DRAM APs."""
    AX = mybir.AxisListType
    ACTF = mybir.ActivationFunctionType
    ALU = mybir.AluOpType

    pads_pool = tc.alloc_tile_pool(name="pads", bufs=1)
    # padded conv-input images, even/odd img columns deinterleaved:
    # [128, 8src, 66row, 2parity, 33] (img col j -> (j%2, j//2));
    # src 0-3 = branch-1 output chunks, 4-7 = branch-2
    pad_all = pads_pool.tile([128, 8, 66, 2, 33], BF, tag="pad_all", name="pad_all")
    nc.vector.memset(pad_all[:, :, 0], 0.0)
    nc.vector.memset(pad_all[:, :, 65], 0.0)
    nc.vector.memset(pad_all[:, :, 1:65, 0, 0], 0.0)
    nc.vector.memset(pad_all[:, :, 1:65, 1, 32], 0.0)

    # streaming x-hi tiles (bf16 [128, 512] each), requested in apply
    # consumption order; the pool rotation sequences their DMAs
    xhp = tc.alloc_tile_pool(name="xhp", bufs=XH_BUFS, side="right")
    # ZT (value-projection, gate+identity folded) lives through the apply
    zsbp = tc.alloc_tile_pool(name="zsb", bufs=1, side="right")
    identb = zsbp.tile([128, 128], BF, tag="identb", name="identb")
    nc.sync.dma_start(identb[:], dr["identb"][:])

    pwv = tc.alloc_tile_pool(name="pwv", bufs=1, side="right")
    # attention probability tiles (1/rowsum folded in), per branch
    P1 = [pwv.tile([128, 512], BF, tag=f"P1_{kb}", name=f"P1_{kb}") for kb in range(CB)]
    P2 = [pwv.tile([128, 512], BF, tag=f"P2_{kb}", name=f"P2_{kb}") for kb in range(CB)]
    # v-projection weights (transposed: [ci, vc]) bf16
    wv1 = [pwv.tile([128, 512], BF, tag=f"wv1_{cb}", name=f"wv1_{cb}") for cb in range(CB)]
    wv2 = [pwv.tile([128, 512], BF, tag=f"wv2_{cb}", name=f"wv2_{cb}") for cb in range(CB)]
    # per-branch gate scalars broadcast to 128 partitions
    abc1 = pwv.tile([128, 1], F32, tag="abc1", name="abc1")
    abc2 = pwv.tile([128, 1], F32, tag="abc2", name="abc2")

    xht = {}

    # ------------ Phase A1: Gram matrix, pooled sums, softmax, gates ----
    with tc.tile_pool(name="a1sb", bufs=1) as a1sb, \
         tc.tile_pool(name="xt", bufs=3) as xtp, \
         tc.tile_pool(name="wkp", bufs=1) as wkp:

        ones = a1sb.tile([128, 128], F32R, tag="ones", name="ones")
        nc.sync.dma_start(ones[:], dr["ones"][:])
        ident = a1sb.tile([128, 128], F32R, tag="ident", name="ident")
        nc.sync.dma_start(ident[:], dr["ident"][:])

        # --- G accumulation + pooled sums (PE rides the DMA-bound phase) ---
        poolf1 = a1sb.tile([1, 512], F32, tag="poolf1", name="poolf1")
        poolf2 = a1sb.tile([1, 512], F32, tag="poolf2", name="poolf2")
        with tc.tile_pool(name="ppp", bufs=1, space="PSUM") as ppp:
            pp1 = ppp.tile([1, 512], F32, tag="pp1", name="pp1")
            pp2 = ppp.tile([1, 512], F32, tag="pp2", name="pp2")
            with tc.tile_pool(name="gps", bufs=1, space="PSUM") as gps:
                G_ps = [gps.tile([128, 512], F32, tag=f"G_{cb}", name=f"G_{cb}") for cb in range(CB)]
                for i in range(NCH):
                    t1_ = xtp.tile([128, 512], F32R, tag="x1t", name="x1t")
                    t2_ = xtp.tile([128, 512], F32R, tag="x2t", name="x2t")
                    nc.sync.dma_start(t1_[:], dr["x1t"][i * 128:(i + 1) * 128, :])
                    nc.sync.dma_start(t2_[:], dr["x2t"][i * 128:(i + 1) * 128, :])
                    st = dict(start=(i == 0), stop=(i == NCH - 1))
                    for cb in range(CB):
                        nc.tensor.matmul(G_ps[cb][:], t1_[:, cb * 128:(cb + 1) * 128], t2_[:], **st)
                    nc.tensor.matmul(pp1[:], ones[:, 0:1], t1_[:], **st)
                    nc.tensor.matmul(pp2[:], ones[:, 0:1], t2_[:], **st)

                G_sb = [a1sb.tile([128, 512], F32R, tag=f"Gsb_{cb}", name=f"Gsb_{cb}") for cb in range(CB)]
                for cb in range(CB):
                    nc.vector.tensor_copy(G_sb[cb][:], G_ps[cb][:])
            nc.vector.tensor_copy(poolf1[:], pp1[:])
            nc.vector.tensor_copy(poolf2[:], pp2[:])

        # all sandwich weights ride in recycled xt-pool slots; the
        # FIFO slot rotation sequences their DMAs behind the G tail
        # in consumption order (M2 -> M1 -> S2 -> S1)
        wq_b2 = [xtp.tile([128, 512], F32R, tag="x1t", name=f"wqb2_{cb}") for cb in range(CB)]
        wq_b1 = [xtp.tile([128, 512], F32R, tag="x2t", name=f"wqb1_{cb}") for cb in range(CB)]
        wk_b2 = [wkp.tile([128, 512], F32R, tag=f"wkb2_{cb}", name=f"wkb2_{cb}") for cb in range(CB)]
        wk_b1 = [xtp.tile([128, 512], F32R, tag="x2t", name=f"wkb1_{cb}") for cb in range(CB)]
        for cb in range(CB):
            cs = slice(cb * 128, (cb + 1) * 128)
            nc.sync.dma_start(wq_b2[cb][:], dr["wq1t"][cs, :])
            nc.sync.dma_start(wq_b1[cb][:], dr["wq2t"][cs, :])
        for cb in range(CB):
            cs = slice(cb * 128, (cb + 1) * 128)
            nc.sync.dma_start(wk_b2[cb][:], dr["wk2t"][cs, :])
            nc.sync.dma_start(wk_b1[cb][:], dr["wk1t"][cs, :])

        # v-weights land before the bulk x-hi loads: the ZT
        # matmuls need them right after the softmax
        for cb in range(CB):
            nc.sync.dma_start(wv2[cb][:], dr["wv2n"][cb * 128:(cb + 1) * 128, :])
            nc.sync.dma_start(wv1[cb][:], dr["wv1n"][cb * 128:(cb + 1) * 128, :])

        # x-hi streaming tiles, requested in apply consumption order
        def req_xh(inp, cib, nt):
            t = xhp.tile([128, 512], BF, tag="xht", name=f"xh_{inp}_{cib}_{nt}")
            src = dr["x1h"] if inp == "x1" else dr["x2h"]
            nc.sync.dma_start(t[:], src[cib * 128:(cib + 1) * 128,
                                        nt * 512:(nt + 1) * 512])
            xht[(inp, cib, nt)] = t
        for nt in range(3):
            for cib in range(CB):
                req_xh("x2", cib, nt)
        for nt in range(3):
            for cib in range(CB):
                req_xh("x1", cib, nt)
        for nt in range(3, NT):
            for cib in range(CB):
                req_xh("x2", cib, nt)
            for cib in range(CB):
                req_xh("x1", cib, nt)

        # --- transpose G -> GT (for branch 1) ---
        GT_sb = [a1sb.tile([128, 512], F32R, tag=f"GTsb_{cb}", name=f"GTsb_{cb}") for cb in range(CB)]
        with tc.tile_pool(name="trp", bufs=2, space="PSUM") as trp:
            for c2b in range(CB):
                for c1b in range(CB):
                    tp = trp.tile([128, 128], F32R, tag="tr", name="tr")
                    nc.tensor.transpose(tp[:], G_sb[c1b][:, c2b * 128:(c2b + 1) * 128], ident[:])
                    nc.vector.tensor_copy(GT_sb[c2b][:, c1b * 128:(c1b + 1) * 128], tp[:])

        wlf = a1sb.tile([1, 512], F32, tag="wlf", name="wlf")
        nc.sync.dma_start(wlf[:], dr["wlinf"][:])

        # --- branch sandwiches + exp ---
        # branch 1: S1 = wk1 (G wq2^T)   via lhsT=GT, then lhsT=wk1t
        # branch 2: S2 = wk2 (G^T wq1^T) via lhsT=G,  then lhsT=wk2t
        rs_all = {}
        branches = [(G_sb, wq_b2, wk_b2, P2), (GT_sb, wq_b1, wk_b1, P1)]
        M_sbs = {}
        with tc.tile_pool(name="msps", bufs=1, space="PSUM") as msps:
            for bi, (Gl, wq, wk, Pt) in enumerate(branches):
                M_ps = [msps.tile([128, 512], F32, tag=f"b{bi}_{cb}", name=f"M{bi}_{cb}") for cb in range(CB)]
                for cb in range(CB):
                    for kb in range(CB):
                        nc.tensor.matmul(M_ps[cb][:], Gl[kb][:, cb * 128:(cb + 1) * 128],
                                         wq[kb][:], start=(kb == 0), stop=(kb == CB - 1))
                M_sb = [a1sb.tile([128, 512], F32R, tag=f"Msb{bi}_{cb}", name=f"Msb{bi}_{cb}") for cb in range(CB)]
                for cb in range(CB):
                    nc.vector.tensor_copy(M_sb[cb][:], M_ps[cb][:])
                M_sbs[bi] = M_sb
            # S tiles reuse the same tags as the M banks they replace
            for bi, (Gl, wq, wk, Pt) in enumerate(branches):
                M_sb = M_sbs[bi]
                S_ps = [msps.tile([128, 512], F32, tag=f"b{bi}_{kb}", name=f"S{bi}_{kb}") for kb in range(CB)]
                for kb in range(CB):
                    for cb in range(CB):
                        nc.tensor.matmul(S_ps[kb][:], wk[cb][:, kb * 128:(kb + 1) * 128],
                                         M_sb[cb][:], start=(cb == 0), stop=(cb == CB - 1))
                for kb in range(CB):
                    nmx = a1sb.tile([128, 1], F32, tag="nmx", name="nmx", bufs=2)
                    nc.vector.reduce_max(nmx[:], S_ps[kb][:], axis=AX.X, negate=True)
                    rs = a1sb.tile([128, 1], F32, tag=f"rs{bi}_{kb}", name=f"rs{bi}_{kb}")
                    nc.scalar.activation(Pt[kb][:], S_ps[kb][:],
                                         ACTF.Exp,
                                         bias=nmx[:], accum_out=rs[:])
                    rs_all[(bi, kb)] = rs

        # --- gate finalization (tiny, after S so it never blocks PE) ---
        # a = sigmoid(sum_c pooled[c] * w_lin[c] / N), broadcast to 128
        with tc.tile_pool(name="bcp", bufs=2, space="PSUM") as bcp:
            for row, (pf, abc) in [(1, (poolf2, abc2)), (0, (poolf1, abc1))]:
                pm = a1sb.tile([1, 512], F32, tag=f"pm{row}", name=f"pm{row}")
                nc.vector.tensor_mul(pm[:], pf[:], wlf[:])
                prs = a1sb.tile([1, 1], F32, tag=f"prs{row}", name=f"prs{row}")
                nc.vector.reduce_sum(prs[:], pm[:], axis=AX.X)
                av = a1sb.tile([1, 2], F32R, tag="av", name="av", bufs=2)
                nc.scalar.activation(av[:], prs[:].to_broadcast((1, 2)),
                                     ACTF.Sigmoid, scale=1.0 / float(N))
                bc_ps = bcp.tile([128, 512], F32, tag="bc", name="bc")
                nc.tensor.matmul(bc_ps[:, 0:2], ones[0:1, :], av[:], start=True, stop=True)
                nc.vector.tensor_copy(abc[:], bc_ps[:, 0:1])
            # preload the sqrt ACT table set now (it also contains Copy/
            # Square/Relu, i.e. everything phase B + finalize uses) so the
            # BN-finalize tail pays no table switch.
            sqd = a1sb.tile([1, 1], F32, tag="sqd", name="sqd")
            nc.scalar.activation(sqd[:], abc1[0:1, 0:1], ACTF.Sqrt)

        # fold 1/rowsum into P (gate + identity fold into ZT later)
        for gbi, Pt in enumerate([P2, P1]):
            for kb in range(CB):
                rs = rs_all[(gbi, kb)]
                ri = a1sb.tile([128, 1], F32, tag="ri", name="ri", bufs=2)
                nc.vector.reciprocal(ri[:], rs[:])
                nc.vector.tensor_scalar_mul(Pt[kb][:], Pt[kb][:], ri[:])

    # ------------ ZT for both branches: ZT = a * (wv^T P) + I -----------
    # re-associated: ZT[ci,c] = a * sum_kc wv[kc,ci] P[kc,c] + I[ci,c]
    # then out[c,n] = sum_ci ZT[ci,c] x[ci,n] includes gate AND residual.
    ZT_all = []
    with tc.tile_pool(name="zps", bufs=1, space="PSUM") as zps:
        for br_i, (Pt, wv, abc) in enumerate([(P2, wv2, abc2), (P1, wv1, abc1)]):
            ZT_sb = []
            for cib in range(CB):
                z_ps = zps.tile([128, 512], F32, tag=f"zps_{cib}", name=f"zps_{br_i}_{cib}")
                for kb in range(CB):
                    nc.tensor.matmul(z_ps[:], wv[kb][:, cib * 128:(cib + 1) * 128],
                                     Pt[kb][:], start=(kb == 0), stop=(kb == CB - 1))
                zt = zsbp.tile([128, 512], BF, tag=f"zt_{br_i}_{cib}", name=f"zt_{br_i}_{cib}")
                nc.vector.tensor_scalar_mul(zt[:], z_ps[:], abc[:])
                cs = slice(cib * 128, (cib + 1) * 128)
                nc.vector.tensor_add(zt[:, cs], zt[:, cs], identb[:])
                ZT_sb.append(zt)
            ZT_all.append(ZT_sb)
    pwv.release()

    # winograd transform pools open early so stage1/2 of sp 0 can overlap
    # the apply tail
    rp = tc.alloc_tile_pool(name="rp", bufs=1)
    vp = tc.alloc_tile_pool(name="vp", bufs=2)
    up = tc.alloc_tile_pool(name="up", bufs=2)

    # k index = pr*8 + src (matches the host U layout)
    def emit_stage1(sp):
        """rows transform: R_all[128, 32k, TR, 2, 33] bf16 (DVE)"""
        r0 = 16 * sp
        rt = rp.tile([128, 32, TR, 2, 33], BF, tag="R_all", name=f"R_all_{sp}")
        a = pad_all[:, :, r0 + 0:r0 + 16:2]
        b = pad_all[:, :, r0 + 2:r0 + 18:2]
        c = pad_all[:, :, r0 + 1:r0 + 17:2]
        d = pad_all[:, :, r0 + 3:min(r0 + 19, 66):2]
        nc.vector.tensor_sub(rt[:, 0:8], a, b)
        nc.vector.tensor_add(rt[:, 8:16], c, b)
        nc.vector.tensor_sub(rt[:, 16:24], b, c)
        nc.vector.tensor_sub(rt[:, 24:32], c, d)
        return rt

    def emit_stage2(rt, sp, pc, gp_k):
        """cols transform: V_all[128, 32k, TT] bf16 (DVE + GPSIMD).
        E/O pad layout makes all four operands stride-1 -> DVE 2x."""
        vt = vp.tile([128, 32, TT], BF, tag="V_all", name=f"V_all_{sp}_{pc}")
        vv = vt.rearrange("p k (a b) -> p k a b", a=TR)
        e = rt[:, :, :, 0, 0:32]
        m = rt[:, :, :, 1, 0:32]
        q = rt[:, :, :, 0, 1:33]
        s = rt[:, :, :, 1, 1:33]
        lo = slice(0, 32 - gp_k)
        hi = slice(32 - gp_k, 32)
        if pc == 0:
            nc.vector.tensor_sub(vv[:, lo], e[:, lo], q[:, lo])
            if gp_k:
                nc.gpsimd.tensor_sub(vv[:, hi], e[:, hi], q[:, hi])
        elif pc == 1:
            nc.vector.tensor_add(vv[:, lo], m[:, lo], q[:, lo])
            if gp_k:
                nc.gpsimd.tensor_add(vv[:, hi], m[:, hi], q[:, hi])
        elif pc == 2:
            nc.vector.tensor_sub(vv[:, lo], q[:, lo], m[:, lo])
            if gp_k:
                nc.gpsimd.tensor_sub(vv[:, hi], q[:, hi], m[:, hi])
        else:
            nc.vector.tensor_sub(vv[:, lo], m[:, lo], s[:, lo])
            if gp_k:
                nc.gpsimd.tensor_sub(vv[:, hi], m[:, hi], s[:, hi])
        return vt

    def gp_k_for(sp, pc):
        # keep the gpsimd queue empty near the tail so the stats
        # collective trigger is never stuck behind transform work
        return 0 if (sp == 3 and pc >= 2) else 4

    # ------------ apply (nt-major): pad[c,n] = sum_ci ZT[ci,c] x[ci,n] --
    opsp = tc.alloc_tile_pool(name="ops", bufs=1, space="PSUM")
    apply_seq = [(br, nt) for nt in range(3) for br in [0]] \
        + [(br, nt) for nt in range(3) for br in [1]]
    apply_tail = [(br, nt) for nt in range(3, NT) for br in (0, 1)]
    g_ctr = [0]

    def apply_group(br_i, nt, act_both):
        inp = "x2" if br_i == 0 else "x1"
        pad_base = 4 if br_i == 0 else 0
        rows = slice(1 + nt * 8, 9 + nt * 8)
        for cb in range(CB):
            g = g_ctr[0]
            g_ctr[0] += 1
            o_ps = opsp.tile([128, 512], F32, tag=f"ops_{g % 4}", name=f"ops_{br_i}_{cb}_{nt}")
            for cib in range(CB):
                nc.tensor.matmul(o_ps[:], ZT_all[br_i][cib][:, cb * 128:(cb + 1) * 128],
                                 xht[(inp, cib, nt)][:], start=(cib == 0), stop=(cib == CB - 1))
            src3 = o_ps.rearrange("p (a b) -> p a b", a=8)
            # src col jc (0-based, img col jc+1): even jc -> odd img col
            # -> parity 1 idx jc/2; odd jc -> even img col -> parity 0.
            dst_o = pad_all[:, pad_base + cb, rows, 1, 0:32]
            dst_e = pad_all[:, pad_base + cb, rows, 0, 1:33]
            nc.scalar.activation(dst_o, src3[:, :, 0:64:2], ACTF.Copy)
            if act_both:
                nc.scalar.activation(dst_e, src3[:, :, 1:64:2], ACTF.Copy)
            else:
                nc.vector.tensor_copy(dst_e, src3[:, :, 1:64:2])

    for br_i, nt in apply_seq:
        apply_group(br_i, nt, act_both=False)
    # stage1/2 for sp 0 run on DVE during the apply tail; the tail's pad
    # copies go ACT-only while the DVE chews the transforms
    R = emit_stage1(0)
    V = emit_stage2(R, 0, 0, gp_k_for(0, 0))
    for br_i, nt in apply_tail:
        apply_group(br_i, nt, act_both=(nt in (3, 4)))
    opsp.release()
    zsbp.release()
    xhp.release()

    # ------------ Phase B: Winograd F(2x2,3x3) conv + BN ----------------
    ybp = tc.alloc_tile_pool(name="ybp", bufs=1, side="right")
    bsb = tc.alloc_tile_pool(name="bsb", bufs=1, side="right")
    dram = tc.alloc_tile_pool(name="dram", bufs=1, space="DRAM")
    t1p = tc.alloc_tile_pool(name="t1p", bufs=2)
    map_ = tc.alloc_tile_pool(name="map", bufs=2)
    mps = tc.alloc_tile_pool(name="mps", bufs=2, space="PSUM")

    # conv output in Winograd block layout: [128, ocb, sp, r, j, 256]
    yb_all = ybp.tile([128, CB, NSP, 2, 2, TT], BF, tag="yb_all", name="yb_all")

    stats_a = bsb.tile([128, 2 * CB], F32, tag="stats_a", name="stats_a")
    stats_b = bsb.tile([128, 2 * CB], F32, tag="stats_b", name="stats_b")
    nc.vector.memset(stats_a[:], 0.0)
    nc.vector.memset(stats_b[:], 0.0)
    s_in1 = dram.tile([128, 2 * CB], F32, tag="arin1", name="arin1")
    s_out1 = dram.tile([128, 2 * CB], F32, tag="arout1", name="arout1")
    s_in2 = dram.tile([128, 2 * CB], F32, tag="arin2", name="arin2")
    s_out2 = dram.tile([128, 2 * CB], F32, tag="arout2", name="arout2")

    def emit_stats(dst_stats, ob, ysl, nel):
        ts = bsb.tile([128, 1], F32, tag="tsum", name="tsum", bufs=2)
        sc1 = bsb.tile([128, nel], BF, tag="scr", name="scr", bufs=1)
        nc.scalar.activation(sc1[:], ysl, ACTF.Copy, accum_out=ts[:])
        tq = bsb.tile([128, 1], F32, tag="tsq", name="tsq", bufs=2)
        sc2 = bsb.tile([128, nel], BF, tag="scr2", name="scr2", bufs=1)
        nc.scalar.activation(sc2[:], ysl, ACTF.Square, accum_out=tq[:])
        nc.vector.tensor_add(dst_stats[:, 2 * ob:2 * ob + 1], dst_stats[:, 2 * ob:2 * ob + 1], ts[:])
        nc.vector.tensor_add(dst_stats[:, 2 * ob + 1:2 * ob + 2], dst_stats[:, 2 * ob + 1:2 * ob + 2], tq[:])

    phases = [(sp, pc) for sp in range(NSP) for pc in range(4)]
    for idx, (sp, pc) in enumerate(phases):
        Vcur = V
        # ---- PE: the 16-position matmuls for this (sp, pc) ----
        mts = []
        for pair in range(2):
            # M PSUM for an ocb pair: [128, 4pr, 2x256] f32
            mt = mps.tile([128, 4, 2 * TT], F32, tag="mt", name=f"mt_{sp}_{pc}_{pair}")
            for half in range(2):
                ocb = pair * 2 + half
                u = up.tile([128, 32 * 128], BF, tag="u", name=f"u_{sp}_{pc}_{ocb}")
                nc.sync.dma_start(u[:], dr["uw"][pc * 4 + ocb])
                hs = slice(half * TT, (half + 1) * TT)
                for icb in range(8):
                    st = dict(start=(icb == 0), stop=(icb == 7))
                    for pr in range(4):
                        nc.tensor.matmul(mt[:, pr, hs],
                                         u[:, (pr * 8 + icb) * 128:(pr * 8 + icb + 1) * 128],
                                         Vcur[:, pr * 8 + icb, :], **st)
            mts.append(mt)

        # ---- DVE: pre-emit NEXT phase transforms (FIFO order) ----
        if idx + 1 < len(phases):
            sp2, pc2 = phases[idx + 1]
            if pc2 == 0:
                R = emit_stage1(sp2)
            V = emit_stage2(R, sp2, pc2, gp_k_for(sp2, pc2))

        # ---- inverse transforms for this phase ----
        for pair in range(2):
            mt = mts[pair]
            ph = slice(pair * 2, pair * 2 + 2)
            # rows (invA): PSUM reads all on ACT, adds on DVE (bf16 2x)
            m0 = map_.tile([128, 2 * TT], BF, tag="m0", name=f"m0_{sp}_{pc}_{pair}")
            m1 = map_.tile([128, 2 * TT], BF, tag="m1", name=f"m1_{sp}_{pc}_{pair}")
            m2 = map_.tile([128, 2 * TT], BF, tag="m2", name=f"m2_{sp}_{pc}_{pair}")
            m3 = map_.tile([128, 2 * TT], BF, tag="m3", name=f"m3_{sp}_{pc}_{pair}")
            nc.scalar.activation(m0[:], mt[:, 0, :], ACTF.Copy)
            nc.scalar.activation(m1[:], mt[:, 1, :], ACTF.Copy)
            nc.scalar.activation(m2[:], mt[:, 2, :], ACTF.Copy)
            nc.scalar.activation(m3[:], mt[:, 3, :], ACTF.Copy)
            t1 = t1p.tile([128, 2, 2 * TT], BF, tag="t1", name=f"t1_{sp}_{pc}_{pair}")
            nc.vector.tensor_add(t1[:, 0, :], m0[:], m1[:])
            nc.vector.tensor_add(t1[:, 0, :], t1[:, 0, :], m2[:])
            nc.vector.tensor_sub(t1[:, 1, :], m1[:], m2[:])
            nc.vector.tensor_sub(t1[:, 1, :], t1[:, 1, :], m3[:])
            # cols (invB): ops span the ocb pair (FD 512)
            for r in range(2):
                tr_ = t1[:, r, :].rearrange("p (o t) -> p o t", o=2)
                y0 = yb_all[:, ph, sp, r, 0, :]
                y1 = yb_all[:, ph, sp, r, 1, :]
                if pc == 0:
                    nc.scalar.activation(y0, tr_, ACTF.Copy)
                elif pc == 1:
                    nc.vector.tensor_add(y0, y0, tr_)
                    nc.scalar.activation(y1, tr_, ACTF.Copy)
                elif pc == 2:
                    nc.vector.tensor_add(y0, y0, tr_)
                    nc.vector.tensor_sub(y1, y1, tr_)
                else:
                    nc.vector.tensor_sub(y1, y1, tr_)

        # ---- BN stats; sp3 split by j so the tail only waits on j=1 ----
        if sp < 3 and pc == 3:
            for ob in range(CB):
                ysl = yb_all[:, ob, sp].rearrange("p a b t -> p (a b t)")
                emit_stats(stats_a, ob, ysl, 4 * TT)
            if sp == 2:
                # early collective: sp 0-2 stats; doubles as a barrier so
                # the final (sp3) collective sees no core skew
                nc.sync.dma_start(s_in1[:], stats_a[:])
                nc.gpsimd.collective_compute(
                    "AllReduce", mybir.AluOpType.add,
                    replica_groups=[list(range(N_CORES))],
                    ins=[s_in1.opt()], outs=[s_out1.opt()])
        elif sp == 3 and pc >= 2:
            jj = pc - 2   # j=0 final after pc2, j=1 after pc3
            for ob in range(CB):
                ysl = yb_all[:, ob, 3, :, jj, :]
                emit_stats(stats_b, ob, ysl, 2 * TT)

    nc.sync.dma_start(s_in2[:], stats_b[:])
    nc.gpsimd.collective_compute(
        "AllReduce", mybir.AluOpType.add,
        replica_groups=[list(range(N_CORES))],
        ins=[s_in2.opt()], outs=[s_out2.opt()])

    mps.release()
    map_.release()
    t1p.release()
    up.release()
    vp.release()
    rp.release()

    # ---- finalize: scale/shift for all 4 ocb at once, then repack ------
    with tc.tile_pool(name="fin", bufs=1) as fin:
        sall1 = fin.tile([128, 2 * CB], F32, tag="sall1", name="sall1")
        sall2 = fin.tile([128, 2 * CB], F32, tag="sall2", name="sall2")
        nc.sync.dma_start(sall1[:], s_out1[:])
        nc.sync.dma_start(sall2[:], s_out2[:])
        sall = fin.tile([128, 2 * CB], F32, tag="sall", name="sall")
        nc.vector.tensor_add(sall[:], sall1[:], sall2[:])

        gam = fin.tile([128, CB], F32, tag="gam", name="gam")
        bet = fin.tile([128, CB], F32, tag="bet", name="bet")
        nc.sync.dma_start(gam[:], dr["gamma"].rearrange("(c p) one -> p (c one)", p=128))
        nc.sync.dma_start(bet[:], dr["beta"].rearrange("(c p) one -> p (c one)", p=128))
        inv_n = 1.0 / float(B * N)
        eps_t = fin.tile([128, 1], F32, tag="eps", name="eps")
        nc.vector.memset(eps_t[:], BN_EPS)

        mean4 = fin.tile([128, CB], F32, tag="mean4", name="mean4")
        nc.vector.tensor_scalar_mul(mean4[:], sall[:, 0:2 * CB:2], inv_n)
        ex24 = fin.tile([128, CB], F32, tag="ex24", name="ex24")
        nc.vector.tensor_scalar_mul(ex24[:], sall[:, 1:2 * CB:2], inv_n)
        m2s = fin.tile([128, CB], F32, tag="m2s", name="m2s")
        nc.vector.tensor_mul(m2s[:], mean4[:], mean4[:])
        var4 = fin.tile([128, CB], F32, tag="var4", name="var4")
        nc.vector.tensor_sub(var4[:], ex24[:], m2s[:])
        std4 = fin.tile([128, CB], F32, tag="std4", name="std4")
        nc.scalar.activation(std4[:], var4[:], ACTF.Sqrt, bias=eps_t[:])
        inv4 = fin.tile([128, CB], F32, tag="inv4", name="inv4")
        nc.vector.reciprocal(inv4[:], std4[:])
        sc4 = fin.tile([128, CB], F32, tag="sc4", name="sc4")
        nc.vector.tensor_mul(sc4[:], gam[:], inv4[:])
        ms4 = fin.tile([128, CB], F32, tag="ms4", name="ms4")
        nc.vector.tensor_mul(ms4[:], mean4[:], sc4[:])
        tt4 = fin.tile([128, CB], F32, tag="tt4", name="tt4")
        nc.vector.tensor_sub(tt4[:], bet[:], ms4[:])

        # normalize + ReLU + repack block layout -> row-major; ACT takes
        # 3 output chunks, DVE the 4th; output DMA per (ob, sp) slab
        for ob in range(CB):
            onat = fin.tile([128, 64, 64], F32, tag="onat", name=f"onat_{ob}", bufs=2)
            for sp in range(NSP):
                for r in range(2):
                    src = yb_all[:, ob, sp, r].rearrange("p j (a b) -> p j a b", a=TR)
                    dst = onat[:, 16 * sp + r:min(16 * sp + r + 16, 64):2, :]
                    dst = dst.rearrange("p a (b j) -> p j a b", j=2)
                    if ob < 3:
                        nc.scalar.activation(dst, src, ACTF.Relu,
                                             bias=tt4[:, ob:ob + 1], scale=sc4[:, ob:ob + 1])
                    else:
                        nc.vector.tensor_scalar(dst, src, sc4[:, ob:ob + 1], tt4[:, ob:ob + 1],
                                                ALU.mult, ALU.add)
                if ob == 3:
                    pl = onat[:, 16 * sp:16 * sp + 16, :]
                    nc.vector.tensor_scalar_max(pl, pl, 0.0)
                nc.sync.dma_start(
                    dr["yout"][ob * 128:(ob + 1) * 128, sp * 1024:(sp + 1) * 1024],
                    onat[:, 16 * sp:16 * sp + 16, :].rearrange("p a b -> p (a b)"))

    bsb.release()
    ybp.release()
    dram.release()
    pads_pool.release()


def _build():
    if "nc" in _CACHE:
        return _CACHE["nc"]
    nc = bacc.Bacc("TRN2", target_bir_lowering=False, debug=False,
                   num_devices=N_CORES)
    dr = {}
    def din(name, shape, dt):
        dr[name] = nc.dram_tensor(name, shape, dt, kind="ExternalInput").ap()
    din("x1t", [N, C], F32R)
    din("x2t", [N, C], F32R)
    din("x1h", [C, N], BF)
    din("x2h", [C, N], BF)
    for w in ["wq1t", "wq2t", "wk1t", "wk2t"]:
        din(w, [C, C], F32R)
    for w in ["wv1n", "wv2n"]:
        din(w, [C, C], BF)
    din("wlinf", [1, C], F32)
    din("uw", [16, 128, 32 * 128], BF)
    din("gamma", [OUT, 1], F32)
    din("beta", [OUT, 1], F32)
    din("ident", [128, 128], F32R)
    din("identb", [128, 128], BF)
    din("ones", [128, 128], F32R)
    dr["yout"] = nc.dram_tensor("yout", [OUT, N], F32, kind="ExternalOutput").ap()

    with tile.TileContext(nc) as tc:
        _emit(nc, tc, dr)
    nc.compile()
    _CACHE["nc"] = nc
    return nc


def _prep_in_maps(inputs):
    f32 = np.float32
    x1 = np.ascontiguousarray(inputs["input1"], f32).reshape(B, C, N)
    x2 = np.ascontiguousarray(inputs["input2"], f32).reshape(B, C, N)
    shared = {}
    for w in ["wq1", "wq2", "wk1", "wk2"]:
        shared[w + "t"] = np.ascontiguousarray(np.asarray(inputs[w], f32).T)
    for w in ["wv1", "wv2"]:
        shared[w + "n"] = np.ascontiguousarray(np.asarray(inputs[w], f32).astype(BF16))
    shared["wlinf"] = np.ascontiguousarray(np.asarray(inputs["w_lin"], f32).reshape(1, C))
    # Winograd weight transform on host: U[pr,pc][ic,oc] = G g G^T
    g = np.asarray(inputs["w_cat"], f32)                     # [OUT, 2C, 3, 3]
    Gm = np.array([[1, 0, 0], [0.5, 0.5, 0.5], [0.5, -0.5, 0.5], [0, 0, 1]], f32)
    U = np.einsum('rj,oijk,ck->rcio', Gm, g, Gm)             # [4,4,2C,OUT]
    # layout: uw[pc*4+ocb][ic_in_chunk][pr, icb, oc] as [16, 128, 4096]
    U6 = U.reshape(4, 4, 8, 128, 4, 128)                     # [pr,pc,icb,i,ocb,o]
    uw = np.ascontiguousarray(U6.transpose(1, 4, 3, 0, 2, 5).reshape(4, 4, 128, 32 * 128))
    # uw dims now [pc, ocb, i, (pr icb o)]
    shared["uw"] = np.ascontiguousarray(uw.reshape(16, 128, 32 * 128).astype(BF16))
    shared["gamma"] = np.ascontiguousarray(np.asarray(inputs["bn_gamma"], f32).reshape(OUT, 1))
    shared["beta"] = np.ascontiguousarray(np.asarray(inputs["bn_beta"], f32).reshape(OUT, 1))
    shared["ident"] = np.eye(128, dtype=f32)
    shared["identb"] = np.eye(128, dtype=f32).astype(BF16)
    shared["ones"] = np.ones((128, 128), f32)

    in_maps = []
    for b in range(B):
        m = dict(shared)
        m["x1t"] = np.ascontiguousarray(x1[b].T)
        m["x2t"] = np.ascontiguousarray(x2[b].T)
        m["x1h"] = np.ascontiguousarray(x1[b].astype(BF16))
        m["x2h"] = np.ascontiguousarray(x2[b].astype(BF16))
        in_maps.append(m)
    return in_maps


def run(inputs, trace=False):
    nc = _build()
    in_maps = _prep_in_maps(inputs)
    res = bass_utils.run_bass_kernel_spmd(nc, in_maps, list(range(N_CORES)),
                                          trace=trace)
    out = np.stack([res.results[b]["yout"] for b in range(B)])
    return out.reshape(B, OUT, H, W).astype(np.float32), res


def kernel(**inputs):
    out, _ = run(inputs, trace=bool(int(os.environ.get("BASS_KERNEL_TRACE", "0"))))
    return out


# revision 9
# speedup vs baseline: 1.2463x; 1.1169x over previous
"""Trainium2 Bass kernel for nn_CrossAtt_27711128994442.

Dual cross-attention block: two branches of channel-attention
(softmax(k @ q^T) applied to v) with a sigmoid gate + residual, concat,
3x3 conv (1024 -> 512), training-mode BatchNorm, ReLU.

Sharding: data-parallel over batch (B=8 -> 8 NeuronCores, one batch
element per core).  BatchNorm statistics are all-reduced across the 8
cores in two rounds (sp 0-2 early / sp 3 late) so the first collective
acts as a barrier that removes core skew from the second.

Math notes (per core / batch element, x1 = input1[b], x2 = input2[b],
both [C=512, N=4096]):
  branch1: S1 = (wk1 x1) (wq2 x2)^T = wk1 G wq2^T where G = x1 x2^T
  branch2: S2 = (wk2 x2) (wq1 x1)^T = wk2 G^T wq1^T
so one Gram matrix G serves both branches.  The pooled-mean gate sums
ride on the PE as ones-vector matmuls during the (DMA-bound) G phase.
The residual is folded into the value-projection product:
  out = (a ZT + I) x   with ZT = wv^T P,
so the attention apply writes the conv pad directly (pure copies, split
between ACT and DVE).

The 3x3 conv runs as Winograd F(2x2,3x3).  The conv pad is stored with
even/odd image columns deinterleaved ([*, 66, 2, 33]) so both input-
transform stages read/write stride-1 bf16 and hit the DVE 2x perf mode.
The 16 per-position matmuls accumulate over input channels in PSUM; the
output inverse transform (A^T M A) runs on vector+scalar engines.
BatchNorm stats ride on the inverse-transform output; the final
normalize+ReLU repacks the block layout to row-major (ACT for 3 output
chunks, DVE for 1) with per-(chunk, sp) output DMA.
"""

import os
import numpy as np
import ml_dtypes

import concourse.bass as bass
import concourse.mybir as mybir
import concourse.bacc as bacc
import concourse.tile as tile
from concourse import bass_utils

BF16 = ml_dtypes.bfloat16
F32 = mybir.dt.float32
F32R = mybir.dt.float32r
BF = mybir.dt.bfloat16

N_CORES = 8
B, C, OUT, H, W = 8, 512, 512, 64, 64
N = H * W            # 4096
CB = C // 128        # 4 channel chunks
NT = N // 512        # 8 spatial tiles of 512 (8 image rows each)
NCH = N // 128       # 32 contraction chunks for the Gram matrix
BN_EPS = 1e-5

# Winograd geometry: 32x32 grid of 2x2 output tiles; 4 sp chunks of
# 8 tile-rows (16 image rows) each.
NSP = 4
TR = 8               # tile-rows per sp chunk
TT = TR * 32         # tiles per sp chunk (256)

XH_BUFS = 24         # streaming x-hi tiles resident (3 nt of lookahead)

_CACHE = {}


def _emit(nc, tc, dr):
    """Emit the whole per-core program. dr: dict of DRAM APs."""
    AX = mybir.AxisListType
    ACTF = mybir.ActivationFunctionType
    ALU = mybir.AluOpType

    pads_pool = tc.alloc_tile_pool(name="pads", bufs=1)
    # padded conv-input images, even/odd img columns deinterleaved:
    # [128, 8src, 66row, 2parity, 33] (img col j -> (j%2, j//2));
    # src 0-3 = branch-1 output chunks, 4-7 = branch-2
    pad_all = pads_pool.tile([128, 8, 66, 2, 33], BF, tag="pad_all", name="pad_all")
    nc.vector.memset(pad_all[:, :, 0], 0.0)
    nc.vector.memset(pad_all[:, :, 65], 0.0)
    nc.vector.memset(pad_all[:, :, 1:65, 0, 0], 0.0)
    nc.vector.memset(pad_all[:, :, 1:65, 1, 32], 0.0)

    # streaming x-hi tiles (bf16 [128, 512] each), requested in apply
    # consumption order; the pool rotation sequences their DMAs
    xhp = tc.alloc_tile_pool(name="xhp", bufs=XH_BUFS, side="right")
    # ZT (value-projection, gate+identity folded) lives through the apply
    zsbp = tc.alloc_tile_pool(name="zsb", bufs=1, side="right")
    identb = zsbp.tile([128, 128], BF, tag="identb", name="identb")
    nc.sync.dma_start(identb[:], dr["identb"][:])

    pwv = tc.alloc_tile_pool(name="pwv", bufs=1, side="right")
    # attention probability tiles (1/rowsum folded in), per branch
    P1 = [pwv.tile([128, 512], BF, tag=f"P1_{kb}", name=f"P1_{kb}") for kb in range(CB)]
    P2 = [pwv.tile([128, 512], BF, tag=f"P2_{kb}", name=f"P2_{kb}") for kb in range(CB)]
    # v-projection weights (transposed: [ci, vc]) bf16
    wv1 = [pwv.tile([128, 512], BF, tag=f"wv1_{cb}", name=f"wv1_{cb}") for cb in range(CB)]
    wv2 = [pwv.tile([128, 512], BF, tag=f"wv2_{cb}", name=f"wv2_{cb}") for cb in range(CB)]
    # per-branch gate scalars broadcast to 128 partitions
    abc1 = pwv.tile([128, 1], F32, tag="abc1", name="abc1")
    abc2 = pwv.tile([128, 1], F32, tag="abc2", name="abc2")

    xht = {}

    # ------------ Phase A1: Gram matrix, pooled sums, softmax, gates ----
    with tc.tile_pool(name="a1sb", bufs=1) as a1sb, \
         tc.tile_pool(name="xt", bufs=3) as xtp, \
         tc.tile_pool(name="wkp", bufs=1) as wkp:

        ones = a1sb.tile([128, 128], F32R, tag="ones", name="ones")
        nc.sync.dma_start(ones[:], dr["ones"][:])
        ident = a1sb.tile([128, 128], F32R, tag="ident", name="ident")
        nc.sync.dma_start(ident[:], dr["ident"][:])
        onescol = a1sb.tile([128, 1], BF, tag="onescol", name="onescol")
        nc.sync.dma_start(onescol[:], dr["onesbf"][:])

        # --- G accumulation + pooled sums (PE rides the DMA-bound phase) ---
        poolf1 = a1sb.tile([1, 512], F32, tag="poolf1", name="poolf1")
        poolf2 = a1sb.tile([1, 512], F32, tag="poolf2", name="poolf2")
        with tc.tile_pool(name="ppp", bufs=1, space="PSUM") as ppp:
            pp1 = ppp.tile([1, 512], F32, tag="pp1", name="pp1")
            pp2 = ppp.tile([1, 512], F32, tag="pp2", name="pp2")
            with tc.tile_pool(name="gps", bufs=1, space="PSUM") as gps:
                G_ps = [gps.tile([128, 512], F32, tag=f"G_{cb}", name=f"G_{cb}") for cb in range(CB)]
                for i in range(NCH):
                    t1_ = xtp.tile([128, 512], BF, tag="x1t", name="x1t")
                    t2_ = xtp.tile([128, 512], BF, tag="x2t", name="x2t")
                    nc.sync.dma_start(t1_[:], dr["x1t"][i * 128:(i + 1) * 128, :])
                    nc.sync.dma_start(t2_[:], dr["x2t"][i * 128:(i + 1) * 128, :])
                    st = dict(start=(i == 0), stop=(i == NCH - 1))
                    for cb in range(CB):
                        nc.tensor.matmul(G_ps[cb][:], t1_[:, cb * 128:(cb + 1) * 128], t2_[:], **st)
                    nc.tensor.matmul(pp1[:], onescol[:], t1_[:], **st)
                    nc.tensor.matmul(pp2[:], onescol[:], t2_[:], **st)

                G_sb = [a1sb.tile([128, 512], F32R, tag=f"Gsb_{cb}", name=f"Gsb_{cb}") for cb in range(CB)]
                for cb in range(CB):
                    nc.vector.tensor_copy(G_sb[cb][:], G_ps[cb][:])
            nc.vector.tensor_copy(poolf1[:], pp1[:])
            nc.vector.tensor_copy(poolf2[:], pp2[:])

        # all sandwich weights ride in recycled xt-pool slots; the
        # FIFO slot rotation sequences their DMAs behind the G tail
        # in consumption order (M2 -> M1 -> S2 -> S1)
        wq_b2 = [xtp.tile([128, 512], F32R, tag="wqt", name=f"wqb2_{cb}") for cb in range(CB)]
        wq_b1 = [xtp.tile([128, 512], F32R, tag="wqt", name=f"wqb1_{cb}") for cb in range(CB)]
        wk_b2 = [wkp.tile([128, 512], F32R, tag=f"wkb2_{cb}", name=f"wkb2_{cb}") for cb in range(CB)]
        wk_b1 = [xtp.tile([128, 512], F32R, tag="wqt", name=f"wkb1_{cb}") for cb in range(CB)]
        for cb in range(CB):
            cs = slice(cb * 128, (cb + 1) * 128)
            nc.sync.dma_start(wq_b2[cb][:], dr["wq1t"][cs, :])
            nc.sync.dma_start(wq_b1[cb][:], dr["wq2t"][cs, :])
        for cb in range(CB):
            cs = slice(cb * 128, (cb + 1) * 128)
            nc.sync.dma_start(wk_b2[cb][:], dr["wk2t"][cs, :])
            nc.sync.dma_start(wk_b1[cb][:], dr["wk1t"][cs, :])

        # v-weights land before the bulk x-hi loads: the ZT
        # matmuls need them right after the softmax
        for cb in range(CB):
            nc.sync.dma_start(wv2[cb][:], dr["wv2n"][cb * 128:(cb + 1) * 128, :])
            nc.sync.dma_start(wv1[cb][:], dr["wv1n"][cb * 128:(cb + 1) * 128, :])

        # x-hi streaming tiles, requested in apply consumption order
        def req_xh(inp, cib, nt):
            t = xhp.tile([128, 512], BF, tag="xht", name=f"xh_{inp}_{cib}_{nt}")
            src = dr["x1h"] if inp == "x1" else dr["x2h"]
            nc.sync.dma_start(t[:], src[cib * 128:(cib + 1) * 128,
                                        nt * 512:(nt + 1) * 512])
            xht[(inp, cib, nt)] = t
        for nt in range(3):
            for cib in range(CB):
                req_xh("x2", cib, nt)
        for nt in range(3):
            for cib in range(CB):
                req_xh("x1", cib, nt)
        for nt in range(3, NT):
            for cib in range(CB):
                req_xh("x2", cib, nt)
            for cib in range(CB):
                req_xh("x1", cib, nt)

        # --- transpose G -> GT (for branch 1) ---
        GT_sb = [a1sb.tile([128, 512], F32R, tag=f"GTsb_{cb}", name=f"GTsb_{cb}") for cb in range(CB)]
        with tc.tile_pool(name="trp", bufs=2, space="PSUM") as trp:
            for c2b in range(CB):
                for c1b in range(CB):
                    tp = trp.tile([128, 128], F32R, tag="tr", name="tr")
                    nc.tensor.transpose(tp[:], G_sb[c1b][:, c2b * 128:(c2b + 1) * 128], ident[:])
                    nc.vector.tensor_copy(GT_sb[c2b][:, c1b * 128:(c1b + 1) * 128], tp[:])

        wlf = a1sb.tile([1, 512], F32, tag="wlf", name="wlf")
        nc.sync.dma_start(wlf[:], dr["wlinf"][:])

        # --- branch sandwiches + exp ---
        # branch 1: S1 = wk1 (G wq2^T)   via lhsT=GT, then lhsT=wk1t
        # branch 2: S2 = wk2 (G^T wq1^T) via lhsT=G,  then lhsT=wk2t
        rs_all = {}
        branches = [(G_sb, wq_b2, wk_b2, P2), (GT_sb, wq_b1, wk_b1, P1)]
        M_sbs = {}
        with tc.tile_pool(name="msps", bufs=1, space="PSUM") as msps:
            for bi, (Gl, wq, wk, Pt) in enumerate(branches):
                M_ps = [msps.tile([128, 512], F32, tag=f"b{bi}_{cb}", name=f"M{bi}_{cb}") for cb in range(CB)]
                for cb in range(CB):
                    for kb in range(CB):
                        nc.tensor.matmul(M_ps[cb][:], Gl[kb][:, cb * 128:(cb + 1) * 128],
                                         wq[kb][:], start=(kb == 0), stop=(kb == CB - 1))
                M_sb = [a1sb.tile([128, 512], F32R, tag=f"Msb{bi}_{cb}", name=f"Msb{bi}_{cb}") for cb in range(CB)]
                for cb in range(CB):
                    nc.vector.tensor_copy(M_sb[cb][:], M_ps[cb][:])
                M_sbs[bi] = M_sb
            # S tiles reuse the same tags as the M banks they replace
            for bi, (Gl, wq, wk, Pt) in enumerate(branches):
                M_sb = M_sbs[bi]
                S_ps = [msps.tile([128, 512], F32, tag=f"b{bi}_{kb}", name=f"S{bi}_{kb}") for kb in range(CB)]
                for kb in range(CB):
                    for cb in range(CB):
                        nc.tensor.matmul(S_ps[kb][:], wk[cb][:, kb * 128:(kb + 1) * 128],
                                         M_sb[cb][:], start=(cb == 0), stop=(cb == CB - 1))
                for kb in range(CB):
                    nmx = a1sb.tile([128, 1], F32, tag="nmx", name="nmx", bufs=2)
                    nc.vector.reduce_max(nmx[:], S_ps[kb][:], axis=AX.X, negate=True)
                    rs = a1sb.tile([128, 1], F32, tag=f"rs{bi}_{kb}", name=f"rs{bi}_{kb}")
                    nc.scalar.activation(Pt[kb][:], S_ps[kb][:],
                                         ACTF.Exp,
                                         bias=nmx[:], accum_out=rs[:])
                    rs_all[(bi, kb)] = rs

        # --- gate finalization (tiny, after S so it never blocks PE) ---
        # a = sigmoid(sum_c pooled[c] * w_lin[c] / N), broadcast to 128
        with tc.tile_pool(name="bcp", bufs=2, space="PSUM") as bcp:
            for row, (pf, abc) in [(1, (poolf2, abc2)), (0, (poolf1, abc1))]:
                pm = a1sb.tile([1, 512], F32, tag=f"pm{row}", name=f"pm{row}")
                nc.vector.tensor_mul(pm[:], pf[:], wlf[:])
                prs = a1sb.tile([1, 1], F32, tag=f"prs{row}", name=f"prs{row}")
                nc.vector.reduce_sum(prs[:], pm[:], axis=AX.X)
                av = a1sb.tile([1, 2], F32R, tag="av", name="av", bufs=2)
                nc.scalar.activation(av[:], prs[:].to_broadcast((1, 2)),
                                     ACTF.Sigmoid, scale=1.0 / float(N))
                bc_ps = bcp.tile([128, 512], F32, tag="bc", name="bc")
                nc.tensor.matmul(bc_ps[:, 0:2], ones[0:1, :], av[:], start=True, stop=True)
                nc.vector.tensor_copy(abc[:], bc_ps[:, 0:1])
            # preload the sqrt ACT table set now (it also contains Copy/
            # Square/Relu, i.e. everything phase B + finalize uses) so the
            # BN-finalize tail pays no table switch.
            sqd = a1sb.tile([1, 1], F32, tag="sqd", name="sqd")
            nc.scalar.activation(sqd[:], abc1[0:1, 0:1], ACTF.Sqrt)

        # fold 1/rowsum into P (gate + identity fold into ZT later)
        for gbi, Pt in enumerate([P2, P1]):
            for kb in range(CB):
                rs = rs_all[(gbi, kb)]
                ri = a1sb.tile([128, 1], F32, tag="ri", name="ri", bufs=2)
                nc.vector.reciprocal(ri[:], rs[:])
                nc.vector.tensor_scalar_mul(Pt[kb][:], Pt[kb][:], ri[:])

    # ------------ ZT for both branches: ZT = a * (wv^T P) + I -----------
    # re-associated: ZT[ci,c] = a * sum_kc wv[kc,ci] P[kc,c] + I[ci,c]
    # then out[c,n] = sum_ci ZT[ci,c] x[ci,n] includes gate AND residual.
    ZT_all = []
    with tc.tile_pool(name="zps", bufs=1, space="PSUM") as zps:
        for br_i, (Pt, wv, abc) in enumerate([(P2, wv2, abc2), (P1, wv1, abc1)]):
            ZT_sb = []
            for cib in range(CB):
                z_ps = zps.tile([128, 512], F32, tag=f"zps_{cib}", name=f"zps_{br_i}_{cib}")
                for kb in range(CB):
                    nc.tensor.matmul(z_ps[:], wv[kb][:, cib * 128:(cib + 1) * 128],
                                     Pt[kb][:], start=(kb == 0), stop=(kb == CB - 1))
                zt = zsbp.tile([128, 512], BF, tag=f"zt_{br_i}_{cib}", name=f"zt_{br_i}_{cib}")
                nc.vector.tensor_scalar_mul(zt[:], z_ps[:], abc[:])
                cs = slice(cib * 128, (cib + 1) * 128)
                nc.vector.tensor_add(zt[:, cs], zt[:, cs], identb[:])
                ZT_sb.append(zt)
            ZT_all.append(ZT_sb)
    pwv.release()

    # winograd transform pools open early so stage1/2 of sp 0 can overlap
    # the apply tail
    rp = tc.alloc_tile_pool(name="rp", bufs=1)
    vp = tc.alloc_tile_pool(name="vp", bufs=2)
    up = tc.alloc_tile_pool(name="up", bufs=3)

    # k index = pr*8 + src (matches the host U layout)
    def emit_stage1(sp):
        """rows transform: R_all[128, 32k, TR, 2, 33] bf16 (DVE)"""
        r0 = 16 * sp
        rt = rp.tile([128, 32, TR, 2, 33], BF, tag="R_all", name=f"R_all_{sp}")
        a = pad_all[:, :, r0 + 0:r0 + 16:2]
        b = pad_all[:, :, r0 + 2:r0 + 18:2]
        c = pad_all[:, :, r0 + 1:r0 + 17:2]
        d = pad_all[:, :, r0 + 3:min(r0 + 19, 66):2]
        nc.vector.tensor_sub(rt[:, 0:8], a, b)
        nc.vector.tensor_add(rt[:, 8:16], c, b)
        nc.vector.tensor_sub(rt[:, 16:24], b, c)
        nc.vector.tensor_sub(rt[:, 24:32], c, d)
        return rt

    def emit_stage2(rt, sp, pc, gp_k):
        """cols transform: V_all[128, 32k, TT] bf16 (DVE + GPSIMD).
        E/O pad layout makes all four operands stride-1 -> DVE 2x."""
        vt = vp.tile([128, 32, TT], BF, tag="V_all", name=f"V_all_{sp}_{pc}")
        vv = vt.rearrange("p k (a b) -> p k a b", a=TR)
        e = rt[:, :, :, 0, 0:32]
        m = rt[:, :, :, 1, 0:32]
        q = rt[:, :, :, 0, 1:33]
        s = rt[:, :, :, 1, 1:33]
        lo = slice(0, 32 - gp_k)
        hi = slice(32 - gp_k, 32)
        if pc == 0:
            nc.vector.tensor_sub(vv[:, lo], e[:, lo], q[:, lo])
            if gp_k:
                nc.gpsimd.tensor_sub(vv[:, hi], e[:, hi], q[:, hi])
        elif pc == 1:
            nc.vector.tensor_add(vv[:, lo], m[:, lo], q[:, lo])
            if gp_k:
                nc.gpsimd.tensor_add(vv[:, hi], m[:, hi], q[:, hi])
        elif pc == 2:
            nc.vector.tensor_sub(vv[:, lo], q[:, lo], m[:, lo])
            if gp_k:
                nc.gpsimd.tensor_sub(vv[:, hi], q[:, hi], m[:, hi])
        else:
            nc.vector.tensor_sub(vv[:, lo], m[:, lo], s[:, lo])
            if gp_k:
                nc.gpsimd.tensor_sub(vv[:, hi], m[:, hi], s[:, hi])
        return vt

    def gp_k_for(sp, pc):
        # keep the gpsimd queue empty near the tail so the stats
        # collective trigger is never stuck behind transform work
        return 0 if (sp == 3 and pc >= 2) else 4

    # ------------ apply (nt-major): pad[c,n] = sum_ci ZT[ci,c] x[ci,n] --
    opsp = tc.alloc_tile_pool(name="ops", bufs=1, space="PSUM")
    apply_seq = [(br, nt) for nt in range(3) for br in [0]] \
        + [(br, nt) for nt in range(3) for br in [1]]
    apply_tail = [(br, nt) for nt in range(3, NT) for br in (0, 1)]
    g_ctr = [0]

    def apply_group(br_i, nt, act_both):
        inp = "x2" if br_i == 0 else "x1"
        pad_base = 4 if br_i == 0 else 0
        rows = slice(1 + nt * 8, 9 + nt * 8)
        for cb in range(CB):
            g = g_ctr[0]
            g_ctr[0] += 1
            o_ps = opsp.tile([128, 512], F32, tag=f"ops_{g % 4}", name=f"ops_{br_i}_{cb}_{nt}")
            for cib in range(CB):
                nc.tensor.matmul(o_ps[:], ZT_all[br_i][cib][:, cb * 128:(cb + 1) * 128],
                                 xht[(inp, cib, nt)][:], start=(cib == 0), stop=(cib == CB - 1))
            src3 = o_ps.rearrange("p (a b) -> p a b", a=8)
            # src col jc (0-based, img col jc+1): even jc -> odd img col
            # -> parity 1 idx jc/2; odd jc -> even img col -> parity 0.
            dst_o = pad_all[:, pad_base + cb, rows, 1, 0:32]
            dst_e = pad_all[:, pad_base + cb, rows, 0, 1:33]
            nc.scalar.activation(dst_o, src3[:, :, 0:64:2], ACTF.Copy)
            if act_both:
                nc.scalar.activation(dst_e, src3[:, :, 1:64:2], ACTF.Copy)
            else:
                nc.vector.tensor_copy(dst_e, src3[:, :, 1:64:2])

    for br_i, nt in apply_seq:
        apply_group(br_i, nt, act_both=False)
    # stage1/2 for sp 0 run on DVE during the apply tail; the tail's pad
    # copies go ACT-only while the DVE chews the transforms
    R = emit_stage1(0)
    V = emit_stage2(R, 0, 0, gp_k_for(0, 0))
    for br_i, nt in apply_tail:
        apply_group(br_i, nt, act_both=(nt in (3, 4)))
    opsp.release()
    zsbp.release()
    xhp.release()

    # ------------ Phase B: Winograd F(2x2,3x3) conv + BN ----------------
    ybp = tc.alloc_tile_pool(name="ybp", bufs=1, side="right")
    bsb = tc.alloc_tile_pool(name="bsb", bufs=1, side="right")
    dram = tc.alloc_tile_pool(name="dram", bufs=1, space="DRAM")
    t1p = tc.alloc_tile_pool(name="t1p", bufs=2)
    map_ = tc.alloc_tile_pool(name="map", bufs=2)
    mps = tc.alloc_tile_pool(name="mps", bufs=2, space="PSUM")

    # conv output in Winograd block layout: [128, ocb, sp, r, j, 256]
    yb_all = ybp.tile([128, CB, NSP, 2, 2, TT], BF, tag="yb_all", name="yb_all")

    stats_a = bsb.tile([128, 2 * CB], F32, tag="stats_a", name="stats_a")
    stats_b = bsb.tile([128, 2 * CB], F32, tag="stats_b", name="stats_b")
    nc.vector.memset(stats_a[:], 0.0)
    nc.vector.memset(stats_b[:], 0.0)
    s_in1 = dram.tile([128, 2 * CB], F32, tag="arin1", name="arin1")
    s_out1 = dram.tile([128, 2 * CB], F32, tag="arout1", name="arout1")
    s_in2 = dram.tile([128, 2 * CB], F32, tag="arin2", name="arin2")
    s_out2 = dram.tile([128, 2 * CB], F32, tag="arout2", name="arout2")

    def emit_stats(dst_stats, ob, ysl, nel):
        ts = bsb.tile([128, 1], F32, tag="tsum", name="tsum", bufs=2)
        sc1 = bsb.tile([128, nel], BF, tag="scr", name="scr", bufs=1)
        nc.scalar.activation(sc1[:], ysl, ACTF.Copy, accum_out=ts[:])
        tq = bsb.tile([128, 1], F32, tag="tsq", name="tsq", bufs=2)
        sc2 = bsb.tile([128, nel], BF, tag="scr2", name="scr2", bufs=1)
        nc.scalar.activation(sc2[:], ysl, ACTF.Square, accum_out=tq[:])
        nc.vector.tensor_add(dst_stats[:, 2 * ob:2 * ob + 1], dst_stats[:, 2 * ob:2 * ob + 1], ts[:])
        nc.vector.tensor_add(dst_stats[:, 2 * ob + 1:2 * ob + 2], dst_stats[:, 2 * ob + 1:2 * ob + 2], tq[:])

    phases = [(sp, pc) for sp in range(NSP) for pc in range(4)]
    for idx, (sp, pc) in enumerate(phases):
        Vcur = V
        # ---- PE: the 16-position matmuls for this (sp, pc) ----
        mts = []
        for pair in range(2):
            # M PSUM for an ocb pair: [128, 4pr, 2x256] f32
            mt = mps.tile([128, 4, 2 * TT], F32, tag="mt", name=f"mt_{sp}_{pc}_{pair}")
            for half in range(2):
                ocb = pair * 2 + half
                u = up.tile([128, 32 * 128], BF, tag="u", name=f"u_{sp}_{pc}_{ocb}")
                nc.sync.dma_start(u[:], dr["uw"][pc * 4 + ocb])
                hs = slice(half * TT, (half + 1) * TT)
                for icb in range(8):
                    st = dict(start=(icb == 0), stop=(icb == 7))
                    for pr in range(4):
                        nc.tensor.matmul(mt[:, pr, hs],
                                         u[:, (pr * 8 + icb) * 128:(pr * 8 + icb + 1) * 128],
                                         Vcur[:, pr * 8 + icb, :], **st)
            mts.append(mt)

        # ---- DVE: pre-emit NEXT phase transforms (FIFO order) ----
        if idx + 1 < len(phases):
            sp2, pc2 = phases[idx + 1]
            if pc2 == 0:
                R = emit_stage1(sp2)
            V = emit_stage2(R, sp2, pc2, gp_k_for(sp2, pc2))

        # ---- inverse transforms for this phase ----
        for pair in range(2):
            mt = mts[pair]
            ph = slice(pair * 2, pair * 2 + 2)
            # rows (invA): PSUM reads all on ACT, adds on DVE (bf16 2x)
            m0 = map_.tile([128, 2 * TT], BF, tag="m0", name=f"m0_{sp}_{pc}_{pair}")
            m1 = map_.tile([128, 2 * TT], BF, tag="m1", name=f"m1_{sp}_{pc}_{pair}")
            m2 = map_.tile([128, 2 * TT], BF, tag="m2", name=f"m2_{sp}_{pc}_{pair}")
            m3 = map_.tile([128, 2 * TT], BF, tag="m3", name=f"m3_{sp}_{pc}_{pair}")
            nc.scalar.activation(m0[:], mt[:, 0, :], ACTF.Copy)
            nc.scalar.activation(m1[:], mt[:, 1, :], ACTF.Copy)
            nc.scalar.activation(m2[:], mt[:, 2, :], ACTF.Copy)
            nc.scalar.activation(m3[:], mt[:, 3, :], ACTF.Copy)
            t1 = t1p.tile([128, 2, 2 * TT], BF, tag="t1", name=f"t1_{sp}_{pc}_{pair}")
            nc.vector.tensor_add(t1[:, 0, :], m0[:], m1[:])
            nc.vector.tensor_add(t1[:, 0, :], t1[:, 0, :], m2[:])
            nc.vector.tensor_sub(t1[:, 1, :], m1[:], m2[:])
            nc.vector.tensor_sub(t1[:, 1, :], t1[:, 1, :], m3[:])
            # cols (invB): ops span the ocb pair (FD 512)
            for r in range(2):
                tr_ = t1[:, r, :].rearrange("p (o t) -> p o t", o=2)
                y0 = yb_all[:, ph, sp, r, 0, :]
                y1 = yb_all[:, ph, sp, r, 1, :]
                if pc == 0:
                    nc.scalar.activation(y0, tr_, ACTF.Copy)
                elif pc == 1:
                    nc.vector.tensor_add(y0, y0, tr_)
                    nc.scalar.activation(y1, tr_, ACTF.Copy)
                elif pc == 2:
                    nc.vector.tensor_add(y0, y0, tr_)
                    nc.vector.tensor_sub(y1, y1, tr_)
                else:
                    nc.vector.tensor_sub(y1, y1, tr_)

        # ---- BN stats; sp3 split by j so the tail only waits on j=1 ----
        if sp < 3 and pc == 3:
            for ob in range(CB):
                ysl = yb_all[:, ob, sp].rearrange("p a b t -> p (a b t)")
                emit_stats(stats_a, ob, ysl, 4 * TT)
        elif sp == 3 and pc == 1:
            # early collective: sp 0-2 stats; doubles as a barrier late
            # enough that the final (sp3) collective sees little skew
            nc.sync.dma_start(s_in1[:], stats_a[:])
            nc.gpsimd.collective_compute(
                "AllReduce", mybir.AluOpType.add,
                replica_groups=[list(range(N_CORES))],
                ins=[s_in1.opt()], outs=[s_out1.opt()])
        elif sp == 3 and pc >= 2:
            jj = pc - 2   # j=0 final after pc2, j=1 after pc3
            for ob in range(CB):
                ysl = yb_all[:, ob, 3, :, jj, :]
                emit_stats(stats_b, ob, ysl, 2 * TT)

    nc.sync.dma_start(s_in2[:], stats_b[:])
    nc.gpsimd.collective_compute(
        "AllReduce", mybir.AluOpType.add,
        replica_groups=[list(range(N_CORES))],
        ins=[s_in2.opt()], outs=[s_out2.opt()])

    mps.release()
    map_.release()
    t1p.release()
    up.release()
    vp.release()
    rp.release()

    # ---- finalize: scale/shift for all 4 ocb at once, then repack ------
    with tc.tile_pool(name="fin", bufs=1) as fin:
        sall1 = fin.tile([128, 2 * CB], F32, tag="sall1", name="sall1")
        sall2 = fin.tile([128, 2 * CB], F32, tag="sall2", name="sall2")
        nc.sync.dma_start(sall1[:], s_out1[:])
        nc.sync.dma_start(sall2[:], s_out2[:])
        sall = fin.tile([128, 2 * CB], F32, tag="sall", name="sall")
        nc.vector.tensor_add(sall[:], sall1[:], sall2[:])

        gam = fin.tile([128, CB], F32, tag="gam", name="gam")
        bet = fin.tile([128, CB], F32, tag="bet", name="bet")
        nc.sync.dma_start(gam[:], dr["gamma"].rearrange("(c p) one -> p (c one)", p=128))
        nc.sync.dma_start(bet[:], dr["beta"].rearrange("(c p) one -> p (c one)", p=128))
        inv_n = 1.0 / float(B * N)
        eps_t = fin.tile([128, 1], F32, tag="eps", name="eps")
        nc.vector.memset(eps_t[:], BN_EPS)

        mean4 = fin.tile([128, CB], F32, tag="mean4", name="mean4")
        nc.vector.tensor_scalar_mul(mean4[:], sall[:, 0:2 * CB:2], inv_n)
        ex24 = fin.tile([128, CB], F32, tag="ex24", name="ex24")
        nc.vector.tensor_scalar_mul(ex24[:], sall[:, 1:2 * CB:2], inv_n)
        m2s = fin.tile([128, CB], F32, tag="m2s", name="m2s")
        nc.vector.tensor_mul(m2s[:], mean4[:], mean4[:])
        var4 = fin.tile([128, CB], F32, tag="var4", name="var4")
        nc.vector.tensor_sub(var4[:], ex24[:], m2s[:])
        std4 = fin.tile([128, CB], F32, tag="std4", name="std4")
        nc.scalar.activation(std4[:], var4[:], ACTF.Sqrt, bias=eps_t[:])
        inv4 = fin.tile([128, CB], F32, tag="inv4", name="inv4")
        nc.vector.reciprocal(inv4[:], std4[:])
        sc4 = fin.tile([128, CB], F32, tag="sc4", name="sc4")
        nc.vector.tensor_mul(sc4[:], gam[:], inv4[:])
        ms4 = fin.tile([128, CB], F32, tag="ms4", name="ms4")
        nc.vector.tensor_mul(ms4[:], mean4[:], sc4[:])
        tt4 = fin.tile([128, CB], F32, tag="tt4", name="tt4")
        nc.vector.tensor_sub(tt4[:], bet[:], ms4[:])

        # normalize + ReLU + repack block layout -> row-major; ACT takes
        # 3 output chunks, DVE the 4th; output DMA per (ob, sp) slab
        for ob in [3, 0, 1, 2]:
            onat = fin.tile([128, 64, 64], F32, tag="onat", name=f"onat_{ob}", bufs=4)
            for sp in range(NSP):
                for r in range(2):
                    src = yb_all[:, ob, sp, r].rearrange("p j (a b) -> p j a b", a=TR)
                    dst = onat[:, 16 * sp + r:min(16 * sp + r + 16, 64):2, :]
                    dst = dst.rearrange("p a (b j) -> p j a b", j=2)
                    if ob < 3:
                        nc.scalar.activation(dst, src, ACTF.Relu,
                                             bias=tt4[:, ob:ob + 1], scale=sc4[:, ob:ob + 1])
                    else:
                        nc.vector.tensor_scalar(dst, src, sc4[:, ob:ob + 1], tt4[:, ob:ob + 1],
                                                ALU.mult, ALU.add)
                if ob == 3:
                    pl = onat[:, 16 * sp:16 * sp + 16, :]
                    nc.vector.tensor_scalar_max(pl, pl, 0.0)
                nc.sync.dma_start(
                    dr["yout"][ob * 128:(ob + 1) * 128, sp * 1024:(sp + 1) * 1024],
                    onat[:, 16 * sp:16 * sp + 16, :].rearrange("p a b -> p (a b)"))

    bsb.release()
    ybp.release()
    dram.release()
    pads_pool.release()


def _build():
    if "nc" in _CACHE:
        return _CACHE["nc"]
    nc = bacc.Bacc("TRN2", target_bir_lowering=False, debug=False,
                   num_devices=N_CORES)
    dr = {}
    def din(name, shape, dt):
        dr[name] = nc.dram_tensor(name, shape, dt, kind="ExternalInput").ap()
    din("x1t", [N, C], BF)
    din("x2t", [N, C], BF)
    din("x1h", [C, N], BF)
    din("x2h", [C, N], BF)
    for w in ["wq1t", "wq2t", "wk1t", "wk2t"]:
        din(w, [C, C], F32R)
    for w in ["wv1n", "wv2n"]:
        din(w, [C, C], BF)
    din("wlinf", [1, C], F32)
    din("uw", [16, 128, 32 * 128], BF)
    din("gamma", [OUT, 1], F32)
    din("beta", [OUT, 1], F32)
    din("ident", [128, 128], F32R)
    din("identb", [128, 128], BF)
    din("ones", [128, 128], F32R)
    din("onesbf", [128, 1], BF)
    dr["yout"] = nc.dram_tensor("yout", [OUT, N], F32, kind="ExternalOutput").ap()

    with tile.TileContext(nc) as tc:
        _emit(nc, tc, dr)
    nc.compile()
    _CACHE["nc"] = nc
    return nc


def _prep_in_maps(inputs):
    f32 = np.float32
    x1 = np.ascontiguousarray(inputs["input1"], f32).reshape(B, C, N)
    x2 = np.ascontiguousarray(inputs["input2"], f32).reshape(B, C, N)
    shared = {}
    for w in ["wq1", "wq2", "wk1", "wk2"]:
        shared[w + "t"] = np.ascontiguousarray(np.asarray(inputs[w], f32).T)
    for w in ["wv1", "wv2"]:
        shared[w + "n"] = np.ascontiguousarray(np.asarray(inputs[w], f32).astype(BF16))
    shared["wlinf"] = np.ascontiguousarray(np.asarray(inputs["w_lin"], f32).reshape(1, C))
    # Winograd weight transform on host: U[pr,pc][ic,oc] = G g G^T
    g = np.asarray(inputs["w_cat"], f32)                     # [OUT, 2C, 3, 3]
    Gm = np.array([[1, 0, 0], [0.5, 0.5, 0.5], [0.5, -0.5, 0.5], [0, 0, 1]], f32)
    U = np.einsum('rj,oijk,ck->rcio', Gm, g, Gm)             # [4,4,2C,OUT]
    # layout: uw[pc*4+ocb][ic_in_chunk][pr, icb, oc] as [16, 128, 4096]
    U6 = U.reshape(4, 4, 8, 128, 4, 128)                     # [pr,pc,icb,i,ocb,o]
    uw = np.ascontiguousarray(U6.transpose(1, 4, 3, 0, 2, 5).reshape(4, 4, 128, 32 * 128))
    # uw dims now [pc, ocb, i, (pr icb o)]
    shared["uw"] = np.ascontiguousarray(uw.reshape(16, 128, 32 * 128).astype(BF16))
    shared["gamma"] = np.ascontiguousarray(np.asarray(inputs["bn_gamma"], f32).reshape(OUT, 1))
    shared["beta"] = np.ascontiguousarray(np.asarray(inputs["bn_beta"], f32).reshape(OUT, 1))
    shared["ident"] = np.eye(128, dtype=f32)
    shared["identb"] = np.eye(128, dtype=f32).astype(BF16)
    shared["ones"] = np.ones((128, 128), f32)
    shared["onesbf"] = np.ones((128, 1), f32).astype(BF16)

    in_maps = []
    for b in range(B):
        m = dict(shared)
        m["x1t"] = np.ascontiguousarray(x1[b].T.astype(BF16))
        m["x2t"] = np.ascontiguousarray(x2[b].T.astype(BF16))
        m["x1h"] = np.ascontiguousarray(x1[b].astype(BF16))
        m["x2h"] = np.ascontiguousarray(x2[b].astype(BF16))
        in_maps.append(m)
    return in_maps


def run(inputs, trace=False):
    nc = _build()
    in_maps = _prep_in_maps(inputs)
    res = bass_utils.run_bass_kernel_spmd(nc, in_maps, list(range(N_CORES)),
                                          trace=trace)
    out = np.stack([res.results[b]["yout"] for b in range(B)])
    return out.reshape(B, OUT, H, W).astype(np.float32), res


def kernel(**inputs):
    out, _ = run(inputs, trace=bool(int(os.environ.get("BASS_KERNEL_TRACE", "0"))))
    return out


# revision 12
# speedup vs baseline: 1.2552x; 1.0071x over previous
"""Trainium2 Bass kernel for nn_CrossAtt_27711128994442.

Dual cross-attention block: two branches of channel-attention
(softmax(k @ q^T) applied to v) with a sigmoid gate + residual, concat,
3x3 conv (1024 -> 512), training-mode BatchNorm, ReLU.

Sharding: data-parallel over batch (B=8 -> 8 NeuronCores, one batch
element per core).  BatchNorm statistics are all-reduced across the 8
cores in two rounds (sp 0-2 early / sp 3 late) so the first collective
acts as a barrier that removes core skew from the second.

Math notes (per core / batch element, x1 = input1[b], x2 = input2[b],
both [C=512, N=4096]):
  branch1: S1 = (wk1 x1) (wq2 x2)^T = wk1 G wq2^T where G = x1 x2^T
  branch2: S2 = (wk2 x2) (wq1 x1)^T = wk2 G^T wq1^T
so one Gram matrix G serves both branches.  The pooled-mean gate sums
ride on the PE as ones-vector matmuls during the (DMA-bound) G phase.
The residual is folded into the value-projection product:
  out = (a ZT + I) x   with ZT = wv^T P,
so the attention apply writes the conv pad directly (pure copies, split
between ACT and DVE).

The 3x3 conv runs as Winograd F(2x2,3x3).  The conv pad is stored with
even/odd image columns deinterleaved ([*, 66, 2, 33]) so both input-
transform stages read/write stride-1 bf16 and hit the DVE 2x perf mode.
The 16 per-position matmuls accumulate over input channels in PSUM; the
output inverse transform (A^T M A) runs on vector+scalar engines.
BatchNorm stats ride on the inverse-transform output; the final
normalize+ReLU repacks the block layout to row-major (ACT for 3 output
chunks, DVE for 1) with per-(chunk, sp) output DMA.
"""

import os
import numpy as np
import ml_dtypes

import concourse.bass as bass
import concourse.mybir as mybir
import concourse.bacc as bacc
import concourse.tile as tile
from concourse import bass_utils

BF16 = ml_dtypes.bfloat16
F32 = mybir.dt.float32
F32R = mybir.dt.float32r
BF = mybir.dt.bfloat16

N_CORES = 8
B, C, OUT, H, W = 8, 512, 512, 64, 64
N = H * W            # 4096
CB = C // 128        # 4 channel chunks
NT = N // 512        # 8 spatial tiles of 512 (8 image rows each)
NCH = N // 128       # 32 contraction chunks for the Gram matrix
BN_EPS = 1e-5

# Winograd geometry: 32x32 grid of 2x2 output tiles; 4 sp chunks of
# 8 tile-rows (16 image rows) each.
NSP = 4
TR = 8               # tile-rows per sp chunk
TT = TR * 32         # tiles per sp chunk (256)

XH_BUFS = 24         # streaming x-hi tiles resident (3 nt of lookahead)

_CACHE = {}


def _emit(nc, tc, dr):
    """Emit the whole per-core program. dr: dict of DRAM APs."""
    AX = mybir.AxisListType
    ACTF = mybir.ActivationFunctionType
    ALU = mybir.AluOpType

    pads_pool = tc.alloc_tile_pool(name="pads", bufs=1)
    # padded conv-input images, even/odd img columns deinterleaved:
    # [128, 8src, 66row, 2parity, 33] (img col j -> (j%2, j//2));
    # src 0-3 = branch-1 output chunks, 4-7 = branch-2
    pad_all = pads_pool.tile([128, 8, 66, 2, 33], BF, tag="pad_all", name="pad_all")
    nc.vector.memset(pad_all[:, :, 0], 0.0)
    nc.vector.memset(pad_all[:, :, 65], 0.0)
    nc.vector.memset(pad_all[:, :, 1:65, 0, 0], 0.0)
    nc.vector.memset(pad_all[:, :, 1:65, 1, 32], 0.0)

    # streaming x-hi tiles (bf16 [128, 512] each), requested in apply
    # consumption order; the pool rotation sequences their DMAs
    xhp = tc.alloc_tile_pool(name="xhp", bufs=XH_BUFS, side="right")
    # ZT (value-projection, gate+identity folded) lives through the apply
    zsbp = tc.alloc_tile_pool(name="zsb", bufs=1, side="right")
    identb = zsbp.tile([128, 128], BF, tag="identb", name="identb")
    nc.sync.dma_start(identb[:], dr["identb"][:])

    pwv = tc.alloc_tile_pool(name="pwv", bufs=1, side="right")
    # attention probability tiles (1/rowsum folded in), per branch
    P1 = [pwv.tile([128, 512], BF, tag=f"P1_{kb}", name=f"P1_{kb}") for kb in range(CB)]
    P2 = [pwv.tile([128, 512], BF, tag=f"P2_{kb}", name=f"P2_{kb}") for kb in range(CB)]
    # v-projection weights (transposed: [ci, vc]) bf16
    wv1 = [pwv.tile([128, 512], BF, tag=f"wv1_{cb}", name=f"wv1_{cb}") for cb in range(CB)]
    wv2 = [pwv.tile([128, 512], BF, tag=f"wv2_{cb}", name=f"wv2_{cb}") for cb in range(CB)]
    # per-branch gate scalars broadcast to 128 partitions
    abc1 = pwv.tile([128, 1], F32, tag="abc1", name="abc1")
    abc2 = pwv.tile([128, 1], F32, tag="abc2", name="abc2")
    onesg = pwv.tile([1, 128], F32R, tag="onesg", name="onesg")
    av_all = {}

    xht = {}

    # ------------ Phase A1: Gram matrix, pooled sums, softmax, gates ----
    with tc.tile_pool(name="a1sb", bufs=1) as a1sb, \
         tc.tile_pool(name="xt", bufs=5) as xtp, \
         tc.tile_pool(name="wkp", bufs=1) as wkp:

        ones = a1sb.tile([128, 128], F32R, tag="ones", name="ones")
        nc.sync.dma_start(ones[:], dr["ones"][:])
        ident = a1sb.tile([128, 128], F32R, tag="ident", name="ident")
        nc.sync.dma_start(ident[:], dr["ident"][:])
        onescol = a1sb.tile([128, 1], BF, tag="onescol", name="onescol")
        nc.sync.dma_start(onescol[:], dr["onesbf"][:])
        nc.sync.dma_start(onesg[:], dr["ones"][0:1, :])

        # --- G accumulation + pooled sums (PE rides the DMA-bound phase) ---
        poolf1 = a1sb.tile([1, 512], F32, tag="poolf1", name="poolf1")
        poolf2 = a1sb.tile([1, 512], F32, tag="poolf2", name="poolf2")
        with tc.tile_pool(name="ppp", bufs=1, space="PSUM") as ppp:
            pp1 = ppp.tile([1, 512], F32, tag="pp1", name="pp1")
            pp2 = ppp.tile([1, 512], F32, tag="pp2", name="pp2")
            with tc.tile_pool(name="gps", bufs=1, space="PSUM") as gps:
                G_ps = [gps.tile([128, 512], F32, tag=f"G_{cb}", name=f"G_{cb}") for cb in range(CB)]
                for i in range(NCH):
                    t1_ = xtp.tile([128, 512], BF, tag="x1t", name="x1t")
                    t2_ = xtp.tile([128, 512], BF, tag="x2t", name="x2t")
                    nc.sync.dma_start(t1_[:], dr["x1t"][i * 128:(i + 1) * 128, :])
                    nc.sync.dma_start(t2_[:], dr["x2t"][i * 128:(i + 1) * 128, :])
                    st = dict(start=(i == 0), stop=(i == NCH - 1))
                    for cb in range(CB):
                        nc.tensor.matmul(G_ps[cb][:], t1_[:, cb * 128:(cb + 1) * 128], t2_[:], **st)
                    nc.tensor.matmul(pp1[:], onescol[:], t1_[:], **st)
                    nc.tensor.matmul(pp2[:], onescol[:], t2_[:], **st)

                G_sb = [a1sb.tile([128, 512], F32R, tag=f"Gsb_{cb}", name=f"Gsb_{cb}") for cb in range(CB)]
                for cb in range(CB):
                    nc.vector.tensor_copy(G_sb[cb][:], G_ps[cb][:])
            nc.vector.tensor_copy(poolf1[:], pp1[:])
            nc.vector.tensor_copy(poolf2[:], pp2[:])

        # all sandwich weights ride in recycled xt-pool slots; the
        # FIFO slot rotation sequences their DMAs behind the G tail
        # in consumption order (M2 -> M1 -> S2 -> S1)
        wq_b2 = [xtp.tile([128, 512], F32R, tag="wqt", name=f"wqb2_{cb}") for cb in range(CB)]
        wq_b1 = [xtp.tile([128, 512], F32R, tag="wqt", name=f"wqb1_{cb}") for cb in range(CB)]
        wk_b2 = [wkp.tile([128, 512], F32R, tag=f"wkb2_{cb}", name=f"wkb2_{cb}") for cb in range(CB)]
        wk_b1 = [xtp.tile([128, 512], F32R, tag="wqt", name=f"wkb1_{cb}") for cb in range(CB)]
        for cb in range(CB):
            cs = slice(cb * 128, (cb + 1) * 128)
            nc.sync.dma_start(wq_b2[cb][:], dr["wq1t"][cs, :])
            nc.sync.dma_start(wq_b1[cb][:], dr["wq2t"][cs, :])
        for cb in range(CB):
            cs = slice(cb * 128, (cb + 1) * 128)
            nc.sync.dma_start(wk_b2[cb][:], dr["wk2t"][cs, :])
            nc.sync.dma_start(wk_b1[cb][:], dr["wk1t"][cs, :])

        # v-weights land before the bulk x-hi loads: the ZT
        # matmuls need them right after the softmax
        for cb in range(CB):
            nc.sync.dma_start(wv2[cb][:], dr["wv2n"][cb * 128:(cb + 1) * 128, :])
            nc.sync.dma_start(wv1[cb][:], dr["wv1n"][cb * 128:(cb + 1) * 128, :])

        # x-hi streaming tiles, requested in apply consumption order
        def req_xh(inp, cib, nt):
            t = xhp.tile([128, 512], BF, tag="xht", name=f"xh_{inp}_{cib}_{nt}")
            src = dr["x1h"] if inp == "x1" else dr["x2h"]
            nc.sync.dma_start(t[:], src[cib * 128:(cib + 1) * 128,
                                        nt * 512:(nt + 1) * 512])
            xht[(inp, cib, nt)] = t
        for nt in range(3):
            for cib in range(CB):
                req_xh("x2", cib, nt)
        for nt in range(3):
            for cib in range(CB):
                req_xh("x1", cib, nt)
        for nt in range(3, NT):
            for cib in range(CB):
                req_xh("x2", cib, nt)
            for cib in range(CB):
                req_xh("x1", cib, nt)

        # --- transpose G -> GT (for branch 1) ---
        GT_sb = [a1sb.tile([128, 512], F32R, tag=f"GTsb_{cb}", name=f"GTsb_{cb}") for cb in range(CB)]
        with tc.tile_pool(name="trp", bufs=2, space="PSUM") as trp:
            for c2b in range(CB):
                for c1b in range(CB):
                    tp = trp.tile([128, 128], F32R, tag="tr", name="tr")
                    nc.tensor.transpose(tp[:], G_sb[c1b][:, c2b * 128:(c2b + 1) * 128], ident[:])
                    nc.vector.tensor_copy(GT_sb[c2b][:, c1b * 128:(c1b + 1) * 128], tp[:])

        wlf = a1sb.tile([1, 512], F32, tag="wlf", name="wlf")
        nc.sync.dma_start(wlf[:], dr["wlinf"][:])

        # --- branch sandwiches + exp ---
        # branch 1: S1 = wk1 (G wq2^T)   via lhsT=GT, then lhsT=wk1t
        # branch 2: S2 = wk2 (G^T wq1^T) via lhsT=G,  then lhsT=wk2t
        rs_all = {}
        branches = [(G_sb, wq_b2, wk_b2, P2), (GT_sb, wq_b1, wk_b1, P1)]
        M_sbs = {}
        with tc.tile_pool(name="msps", bufs=1, space="PSUM") as msps:
            for bi, (Gl, wq, wk, Pt) in enumerate(branches):
                M_ps = [msps.tile([128, 512], F32, tag=f"b{bi}_{cb}", name=f"M{bi}_{cb}") for cb in range(CB)]
                for cb in range(CB):
                    for kb in range(CB):
                        nc.tensor.matmul(M_ps[cb][:], Gl[kb][:, cb * 128:(cb + 1) * 128],
                                         wq[kb][:], start=(kb == 0), stop=(kb == CB - 1))
                M_sb = [a1sb.tile([128, 512], F32R, tag=f"Msb{bi}_{cb}", name=f"Msb{bi}_{cb}") for cb in range(CB)]
                for cb in range(CB):
                    nc.vector.tensor_copy(M_sb[cb][:], M_ps[cb][:])
                M_sbs[bi] = M_sb
            # S tiles reuse the same tags as the M banks they replace
            for bi, (Gl, wq, wk, Pt) in enumerate(branches):
                M_sb = M_sbs[bi]
                S_ps = [msps.tile([128, 512], F32, tag=f"b{bi}_{kb}", name=f"S{bi}_{kb}") for kb in range(CB)]
                for kb in range(CB):
                    for cb in range(CB):
                        nc.tensor.matmul(S_ps[kb][:], wk[cb][:, kb * 128:(kb + 1) * 128],
                                         M_sb[cb][:], start=(cb == 0), stop=(cb == CB - 1))
                for kb in range(CB):
                    nmx = a1sb.tile([128, 1], F32, tag="nmx", name="nmx", bufs=2)
                    nc.vector.reduce_max(nmx[:], S_ps[kb][:], axis=AX.X, negate=True)
                    rs = a1sb.tile([128, 1], F32, tag=f"rs{bi}_{kb}", name=f"rs{bi}_{kb}")
                    nc.scalar.activation(Pt[kb][:], S_ps[kb][:],
                                         ACTF.Exp,
                                         bias=nmx[:], accum_out=rs[:])
                    rs_all[(bi, kb)] = rs
                # gate sigmoid for this branch (tiny DVE+ACT): queues
                # right behind this branch's exps; the broadcast matmul
                # happens in the ZT loop so it never head-blocks the PE
                pf = poolf2 if bi == 0 else poolf1
                pm = a1sb.tile([1, 512], F32, tag=f"pm{bi}", name=f"pm{bi}")
                nc.vector.tensor_mul(pm[:], pf[:], wlf[:])
                prs = a1sb.tile([1, 1], F32, tag=f"prs{bi}", name=f"prs{bi}")
                nc.vector.reduce_sum(prs[:], pm[:], axis=AX.X)
                av = pwv.tile([1, 2], F32R, tag=f"av{bi}", name=f"av{bi}")
                nc.scalar.activation(av[:], prs[:].to_broadcast((1, 2)),
                                     ACTF.Sigmoid, scale=1.0 / float(N))
                av_all[bi] = av

        # preload the sqrt ACT table set now (it also contains Copy/
        # Square/Relu, i.e. everything phase B + finalize uses) so the
        # BN-finalize tail pays no table switch.
        sqd = a1sb.tile([1, 1], F32, tag="sqd", name="sqd")
        nc.scalar.activation(sqd[:], poolf1[0:1, 0:1], ACTF.Sqrt)

        # fold 1/rowsum into P (gate + identity fold into ZT later)
        for gbi, Pt in enumerate([P2, P1]):
            for kb in range(CB):
                rs = rs_all[(gbi, kb)]
                ri = a1sb.tile([128, 1], F32, tag="ri", name="ri", bufs=2)
                nc.vector.reciprocal(ri[:], rs[:])
                nc.vector.tensor_scalar_mul(Pt[kb][:], Pt[kb][:], ri[:])

    # ------------ ZT for both branches: ZT = a * (wv^T P) + I -----------
    # re-associated: ZT[ci,c] = a * sum_kc wv[kc,ci] P[kc,c] + I[ci,c]
    # then out[c,n] = sum_ci ZT[ci,c] x[ci,n] includes gate AND residual.
    ZT_all = []
    with tc.tile_pool(name="zps", bufs=1, space="PSUM") as zps:
        for br_i, (Pt, wv, abc) in enumerate([(P2, wv2, abc2), (P1, wv1, abc1)]):
            bc_ps = zps.tile([128, 512], F32, tag="bcg", name=f"bc{br_i}", bufs=2)
            nc.tensor.matmul(bc_ps[:, 0:2], onesg[:], av_all[br_i][:], start=True, stop=True)
            nc.vector.tensor_copy(abc[:], bc_ps[:, 0:1])
            ZT_sb = []
            for cib in range(CB):
                z_ps = zps.tile([128, 512], F32, tag=f"zps_{cib}", name=f"zps_{br_i}_{cib}")
                for kb in range(CB):
                    nc.tensor.matmul(z_ps[:], wv[kb][:, cib * 128:(cib + 1) * 128],
                                     Pt[kb][:], start=(kb == 0), stop=(kb == CB - 1))
                zt = zsbp.tile([128, 512], BF, tag=f"zt_{br_i}_{cib}", name=f"zt_{br_i}_{cib}")
                nc.vector.tensor_scalar_mul(zt[:], z_ps[:], abc[:])
                cs = slice(cib * 128, (cib + 1) * 128)
                nc.vector.tensor_add(zt[:, cs], zt[:, cs], identb[:])
                ZT_sb.append(zt)
            ZT_all.append(ZT_sb)
    pwv.release()

    # winograd transform pools open early so stage1/2 of sp 0 can overlap
    # the apply tail
    rp = tc.alloc_tile_pool(name="rp", bufs=1)
    vp = tc.alloc_tile_pool(name="vp", bufs=2)
    up = tc.alloc_tile_pool(name="up", bufs=3)

    # k index = pr*8 + src (matches the host U layout)
    def emit_stage1(sp):
        """rows transform: R_all[128, 32k, TR, 2, 33] bf16 (DVE)"""
        r0 = 16 * sp
        rt = rp.tile([128, 32, TR, 2, 33], BF, tag="R_all", name=f"R_all_{sp}")
        a = pad_all[:, :, r0 + 0:r0 + 16:2]
        b = pad_all[:, :, r0 + 2:r0 + 18:2]
        c = pad_all[:, :, r0 + 1:r0 + 17:2]
        d = pad_all[:, :, r0 + 3:min(r0 + 19, 66):2]
        nc.vector.tensor_sub(rt[:, 0:8], a, b)
        nc.vector.tensor_add(rt[:, 8:16], c, b)
        nc.vector.tensor_sub(rt[:, 16:24], b, c)
        nc.vector.tensor_sub(rt[:, 24:32], c, d)
        return rt

    def emit_stage2(rt, sp, pc, gp_k):
        """cols transform: V_all[128, 32k, TT] bf16 (DVE + GPSIMD).
        E/O pad layout makes all four operands stride-1 -> DVE 2x."""
        vt = vp.tile([128, 32, TT], BF, tag="V_all", name=f"V_all_{sp}_{pc}")
        vv = vt.rearrange("p k (a b) -> p k a b", a=TR)
        e = rt[:, :, :, 0, 0:32]
        m = rt[:, :, :, 1, 0:32]
        q = rt[:, :, :, 0, 1:33]
        s = rt[:, :, :, 1, 1:33]
        lo = slice(0, 32 - gp_k)
        hi = slice(32 - gp_k, 32)
        if pc == 0:
            nc.vector.tensor_sub(vv[:, lo], e[:, lo], q[:, lo])
            if gp_k:
                nc.gpsimd.tensor_sub(vv[:, hi], e[:, hi], q[:, hi])
        elif pc == 1:
            nc.vector.tensor_add(vv[:, lo], m[:, lo], q[:, lo])
            if gp_k:
                nc.gpsimd.tensor_add(vv[:, hi], m[:, hi], q[:, hi])
        elif pc == 2:
            nc.vector.tensor_sub(vv[:, lo], q[:, lo], m[:, lo])
            if gp_k:
                nc.gpsimd.tensor_sub(vv[:, hi], q[:, hi], m[:, hi])
        else:
            nc.vector.tensor_sub(vv[:, lo], m[:, lo], s[:, lo])
            if gp_k:
                nc.gpsimd.tensor_sub(vv[:, hi], m[:, hi], s[:, hi])
        return vt

    def gp_k_for(sp, pc):
        # keep the gpsimd queue empty near the tail so the stats
        # collective trigger is never stuck behind transform work
        return 0 if (sp == 3 and pc >= 2) else 4

    # ------------ apply (nt-major): pad[c,n] = sum_ci ZT[ci,c] x[ci,n] --
    opsp = tc.alloc_tile_pool(name="ops", bufs=1, space="PSUM")
    apply_seq = [(br, nt) for nt in range(3) for br in [0]] \
        + [(br, nt) for nt in range(3) for br in [1]]
    apply_tail = [(br, nt) for nt in range(3, NT) for br in (0, 1)]
    g_ctr = [0]

    def apply_group(br_i, nt, act_both):
        inp = "x2" if br_i == 0 else "x1"
        pad_base = 4 if br_i == 0 else 0
        rows = slice(1 + nt * 8, 9 + nt * 8)
        for cb in range(CB):
            g = g_ctr[0]
            g_ctr[0] += 1
            o_ps = opsp.tile([128, 512], F32, tag=f"ops_{g % 4}", name=f"ops_{br_i}_{cb}_{nt}")
            for cib in range(CB):
                nc.tensor.matmul(o_ps[:], ZT_all[br_i][cib][:, cb * 128:(cb + 1) * 128],
                                 xht[(inp, cib, nt)][:], start=(cib == 0), stop=(cib == CB - 1))
            src3 = o_ps.rearrange("p (a b) -> p a b", a=8)
            # src col jc (0-based, img col jc+1): even jc -> odd img col
            # -> parity 1 idx jc/2; odd jc -> even img col -> parity 0.
            dst_o = pad_all[:, pad_base + cb, rows, 1, 0:32]
            dst_e = pad_all[:, pad_base + cb, rows, 0, 1:33]
            nc.scalar.activation(dst_o, src3[:, :, 0:64:2], ACTF.Copy)
            if act_both:
                nc.scalar.activation(dst_e, src3[:, :, 1:64:2], ACTF.Copy)
            else:
                nc.vector.tensor_copy(dst_e, src3[:, :, 1:64:2])

    for br_i, nt in apply_seq:
        apply_group(br_i, nt, act_both=False)
    # stage1/2 for sp 0 run on DVE during the apply tail; the tail's pad
    # copies go ACT-only while the DVE chews the transforms
    R = emit_stage1(0)
    V = emit_stage2(R, 0, 0, gp_k_for(0, 0))
    for br_i, nt in apply_tail:
        apply_group(br_i, nt, act_both=(nt in (3, 4)))
    opsp.release()
    zsbp.release()
    xhp.release()

    # ------------ Phase B: Winograd F(2x2,3x3) conv + BN ----------------
    ybp = tc.alloc_tile_pool(name="ybp", bufs=1, side="right")
    bsb = tc.alloc_tile_pool(name="bsb", bufs=1, side="right")
    dram = tc.alloc_tile_pool(name="dram", bufs=1, space="DRAM")
    t1p = tc.alloc_tile_pool(name="t1p", bufs=2)
    map_ = tc.alloc_tile_pool(name="map", bufs=2)
    mps = tc.alloc_tile_pool(name="mps", bufs=2, space="PSUM")

    # conv output in Winograd block layout: [128, ocb, sp, r, j, 256]
    yb_all = ybp.tile([128, CB, NSP, 2, 2, TT], BF, tag="yb_all", name="yb_all")

    stats_a = bsb.tile([128, 2 * CB], F32, tag="stats_a", name="stats_a")
    stats_b = bsb.tile([128, 2 * CB], F32, tag="stats_b", name="stats_b")
    nc.vector.memset(stats_a[:], 0.0)
    nc.vector.memset(stats_b[:], 0.0)
    s_in1 = dram.tile([128, 2 * CB], F32, tag="arin1", name="arin1")
    s_out1 = dram.tile([128, 2 * CB], F32, tag="arout1", name="arout1")
    s_in2 = dram.tile([128, 2 * CB], F32, tag="arin2", name="arin2")
    s_out2 = dram.tile([128, 2 * CB], F32, tag="arout2", name="arout2")

    def emit_stats(dst_stats, ob, ysl, nel):
        ts = bsb.tile([128, 1], F32, tag="tsum", name="tsum", bufs=2)
        sc1 = bsb.tile([128, nel], BF, tag="scr", name="scr", bufs=1)
        nc.scalar.activation(sc1[:], ysl, ACTF.Copy, accum_out=ts[:])
        tq = bsb.tile([128, 1], F32, tag="tsq", name="tsq", bufs=2)
        sc2 = bsb.tile([128, nel], BF, tag="scr2", name="scr2", bufs=1)
        nc.scalar.activation(sc2[:], ysl, ACTF.Square, accum_out=tq[:])
        nc.vector.tensor_add(dst_stats[:, 2 * ob:2 * ob + 1], dst_stats[:, 2 * ob:2 * ob + 1], ts[:])
        nc.vector.tensor_add(dst_stats[:, 2 * ob + 1:2 * ob + 2], dst_stats[:, 2 * ob + 1:2 * ob + 2], tq[:])

    phases = [(sp, pc) for sp in range(NSP) for pc in range(4)]
    for idx, (sp, pc) in enumerate(phases):
        Vcur = V
        # ---- PE: the 16-position matmuls for this (sp, pc) ----
        mts = []
        for pair in range(2):
            # M PSUM for an ocb pair: [128, 4pr, 2x256] f32
            mt = mps.tile([128, 4, 2 * TT], F32, tag="mt", name=f"mt_{sp}_{pc}_{pair}")
            for half in range(2):
                ocb = pair * 2 + half
                u = up.tile([128, 32 * 128], BF, tag="u", name=f"u_{sp}_{pc}_{ocb}")
                nc.sync.dma_start(u[:], dr["uw"][pc * 4 + ocb])
                hs = slice(half * TT, (half + 1) * TT)
                for icb in range(8):
                    st = dict(start=(icb == 0), stop=(icb == 7))
                    for pr in range(4):
                        nc.tensor.matmul(mt[:, pr, hs],
                                         u[:, (pr * 8 + icb) * 128:(pr * 8 + icb + 1) * 128],
                                         Vcur[:, pr * 8 + icb, :], **st)
            mts.append(mt)

        # ---- DVE: pre-emit NEXT phase transforms (FIFO order) ----
        if idx + 1 < len(phases):
            sp2, pc2 = phases[idx + 1]
            if pc2 == 0:
                R = emit_stage1(sp2)
            V = emit_stage2(R, sp2, pc2, gp_k_for(sp2, pc2))

        # ---- inverse transforms for this phase ----
        for pair in range(2):
            mt = mts[pair]
            ph = slice(pair * 2, pair * 2 + 2)
            # rows (invA): PSUM reads all on ACT, adds on DVE (bf16 2x)
            m0 = map_.tile([128, 2 * TT], BF, tag="m0", name=f"m0_{sp}_{pc}_{pair}")
            m1 = map_.tile([128, 2 * TT], BF, tag="m1", name=f"m1_{sp}_{pc}_{pair}")
            m2 = map_.tile([128, 2 * TT], BF, tag="m2", name=f"m2_{sp}_{pc}_{pair}")
            m3 = map_.tile([128, 2 * TT], BF, tag="m3", name=f"m3_{sp}_{pc}_{pair}")
            nc.scalar.activation(m0[:], mt[:, 0, :], ACTF.Copy)
            nc.scalar.activation(m1[:], mt[:, 1, :], ACTF.Copy)
            nc.scalar.activation(m2[:], mt[:, 2, :], ACTF.Copy)
            nc.scalar.activation(m3[:], mt[:, 3, :], ACTF.Copy)
            t1 = t1p.tile([128, 2, 2 * TT], BF, tag="t1", name=f"t1_{sp}_{pc}_{pair}")
            nc.vector.tensor_add(t1[:, 0, :], m0[:], m1[:])
            nc.vector.tensor_add(t1[:, 0, :], t1[:, 0, :], m2[:])
            nc.vector.tensor_sub(t1[:, 1, :], m1[:], m2[:])
            nc.vector.tensor_sub(t1[:, 1, :], t1[:, 1, :], m3[:])
            # cols (invB): ops span the ocb pair (FD 512)
            for r in range(2):
                tr_ = t1[:, r, :].rearrange("p (o t) -> p o t", o=2)
                y0 = yb_all[:, ph, sp, r, 0, :]
                y1 = yb_all[:, ph, sp, r, 1, :]
                if pc == 0:
                    nc.scalar.activation(y0, tr_, ACTF.Copy)
                elif pc == 1:
                    nc.vector.tensor_add(y0, y0, tr_)
                    nc.scalar.activation(y1, tr_, ACTF.Copy)
                elif pc == 2:
                    nc.vector.tensor_add(y0, y0, tr_)
                    nc.vector.tensor_sub(y1, y1, tr_)
                else:
                    nc.vector.tensor_sub(y1, y1, tr_)
            if sp == 3 and pc >= 2:
                jj = pc - 2   # j=0 final after pc2, j=1 after pc3
                for ob in (pair * 2, pair * 2 + 1):
                    ysl = yb_all[:, ob, 3, :, jj, :]
                    emit_stats(stats_b, ob, ysl, 2 * TT)

        # ---- BN stats; sp3 split by j so the tail only waits on j=1 ----
        if sp < 3 and pc == 3:
            for ob in range(CB):
                ysl = yb_all[:, ob, sp].rearrange("p a b t -> p (a b t)")
                emit_stats(stats_a, ob, ysl, 4 * TT)
        elif sp == 3 and pc == 1:
            # early collective: sp 0-2 stats; doubles as a barrier late
            # enough that the final (sp3) collective sees little skew
            nc.sync.dma_start(s_in1[:], stats_a[:])
            nc.gpsimd.collective_compute(
                "AllReduce", mybir.AluOpType.add,
                replica_groups=[list(range(N_CORES))],
                ins=[s_in1.opt()], outs=[s_out1.opt()])


    nc.sync.dma_start(s_in2[:], stats_b[:])
    nc.gpsimd.collective_compute(
        "AllReduce", mybir.AluOpType.add,
        replica_groups=[list(range(N_CORES))],
        ins=[s_in2.opt()], outs=[s_out2.opt()])

    mps.release()
    map_.release()
    t1p.release()
    up.release()
    vp.release()
    rp.release()

    # ---- finalize: scale/shift for all 4 ocb at once, then repack ------
    with tc.tile_pool(name="fin", bufs=1) as fin:
        sall1 = fin.tile([128, 2 * CB], F32, tag="sall1", name="sall1")
        sall2 = fin.tile([128, 2 * CB], F32, tag="sall2", name="sall2")
        nc.sync.dma_start(sall1[:], s_out1[:])
        nc.sync.dma_start(sall2[:], s_out2[:])
        sall = fin.tile([128, 2 * CB], F32, tag="sall", name="sall")
        nc.vector.tensor_add(sall[:], sall1[:], sall2[:])

        gam = fin.tile([128, CB], F32, tag="gam", name="gam")
        bet = fin.tile([128, CB], F32, tag="bet", name="bet")
        nc.sync.dma_start(gam[:], dr["gamma"].rearrange("(c p) one -> p (c one)", p=128))
        nc.sync.dma_start(bet[:], dr["beta"].rearrange("(c p) one -> p (c one)", p=128))
        inv_n = 1.0 / float(B * N)
        eps_t = fin.tile([128, 1], F32, tag="eps", name="eps")
        nc.vector.memset(eps_t[:], BN_EPS)

        mean4 = fin.tile([128, CB], F32, tag="mean4", name="mean4")
        nc.vector.tensor_scalar_mul(mean4[:], sall[:, 0:2 * CB:2], inv_n)
        ex24 = fin.tile([128, CB], F32, tag="ex24", name="ex24")
        nc.vector.tensor_scalar_mul(ex24[:], sall[:, 1:2 * CB:2], inv_n)
        m2s = fin.tile([128, CB], F32, tag="m2s", name="m2s")
        nc.vector.tensor_mul(m2s[:], mean4[:], mean4[:])
        var4 = fin.tile([128, CB], F32, tag="var4", name="var4")
        nc.vector.tensor_sub(var4[:], ex24[:], m2s[:])
        std4 = fin.tile([128, CB], F32, tag="std4", name="std4")
        nc.scalar.activation(std4[:], var4[:], ACTF.Sqrt, bias=eps_t[:])
        inv4 = fin.tile([128, CB], F32, tag="inv4", name="inv4")
        nc.vector.reciprocal(inv4[:], std4[:])
        sc4 = fin.tile([128, CB], F32, tag="sc4", name="sc4")
        nc.vector.tensor_mul(sc4[:], gam[:], inv4[:])
        ms4 = fin.tile([128, CB], F32, tag="ms4", name="ms4")
        nc.vector.tensor_mul(ms4[:], mean4[:], sc4[:])
        tt4 = fin.tile([128, CB], F32, tag="tt4", name="tt4")
        nc.vector.tensor_sub(tt4[:], bet[:], ms4[:])

        # normalize + ReLU + repack block layout -> row-major; ACT takes
        # 3 output chunks, DVE the 4th; output DMA per (ob, sp) slab
        for ob in [3, 2, 0, 1]:
            onat = fin.tile([128, 64, 64], BF, tag="onat", name=f"onat_{ob}", bufs=4)
            for sp in range(NSP):
                for r in range(2):
                    src = yb_all[:, ob, sp, r].rearrange("p j (a b) -> p j a b", a=TR)
                    dst = onat[:, 16 * sp + r:min(16 * sp + r + 16, 64):2, :]
                    dst = dst.rearrange("p a (b j) -> p j a b", j=2)
                    if ob < 2:
                        nc.scalar.activation(dst, src, ACTF.Relu,
                                             bias=tt4[:, ob:ob + 1], scale=sc4[:, ob:ob + 1])
                    else:
                        nc.vector.tensor_scalar(dst, src, sc4[:, ob:ob + 1], tt4[:, ob:ob + 1],
                                                ALU.mult, ALU.add)
                if ob >= 2:
                    pl = onat[:, 16 * sp:16 * sp + 16, :]
                    nc.vector.tensor_scalar_max(pl, pl, 0.0)
                nc.sync.dma_start(
                    dr["yout"][ob * 128:(ob + 1) * 128, sp * 1024:(sp + 1) * 1024],
                    onat[:, 16 * sp:16 * sp + 16, :].rearrange("p a b -> p (a b)"))

    bsb.release()
    ybp.release()
    dram.release()
    pads_pool.release()


def _build():
    if "nc" in _CACHE:
        return _CACHE["nc"]
    nc = bacc.Bacc("TRN2", target_bir_lowering=False, debug=False,
                   num_devices=N_CORES)
    dr = {}
    def din(name, shape, dt):
        dr[name] = nc.dram_tensor(name, shape, dt, kind="ExternalInput").ap()
    din("x1t", [N, C], BF)
    din("x2t", [N, C], BF)
    din("x1h", [C, N], BF)
    din("x2h", [C, N], BF)
    for w in ["wq1t", "wq2t", "wk1t", "wk2t"]:
        din(w, [C, C], F32R)
    for w in ["wv1n", "wv2n"]:
        din(w, [C, C], BF)
    din("wlinf", [1, C], F32)
    din("uw", [16, 128, 32 * 128], BF)
    din("gamma", [OUT, 1], F32)
    din("beta", [OUT, 1], F32)
    din("ident", [128, 128], F32R)
    din("identb", [128, 128], BF)
    din("ones", [128, 128], F32R)
    din("onesbf", [128, 1], BF)
    dr["yout"] = nc.dram_tensor("yout", [OUT, N], BF, kind="ExternalOutput").ap()

    with tile.TileContext(nc) as tc:
        _emit(nc, tc, dr)
    nc.compile()
    _CACHE["nc"] = nc
    return nc


def _prep_in_maps(inputs):
    f32 = np.float32
    x1 = np.ascontiguousarray(inputs["input1"], f32).reshape(B, C, N)
    x2 = np.ascontiguousarray(inputs["input2"], f32).reshape(B, C, N)
    shared = {}
    for w in ["wq1", "wq2", "wk1", "wk2"]:
        shared[w + "t"] = np.ascontiguousarray(np.asarray(inputs[w], f32).T)
    for w in ["wv1", "wv2"]:
        shared[w + "n"] = np.ascontiguousarray(np.asarray(inputs[w], f32).astype(BF16))
    shared["wlinf"] = np.ascontiguousarray(np.asarray(inputs["w_lin"], f32).reshape(1, C))
    # Winograd weight transform on host: U[pr,pc][ic,oc] = G g G^T
    g = np.asarray(inputs["w_cat"], f32)                     # [OUT, 2C, 3, 3]
    Gm = np.array([[1, 0, 0], [0.5, 0.5, 0.5], [0.5, -0.5, 0.5], [0, 0, 1]], f32)
    U = np.einsum('rj,oijk,ck->rcio', Gm, g, Gm)             # [4,4,2C,OUT]
    # layout: uw[pc*4+ocb][ic_in_chunk][pr, icb, oc] as [16, 128, 4096]
    U6 = U.reshape(4, 4, 8, 128, 4, 128)                     # [pr,pc,icb,i,ocb,o]
    uw = np.ascontiguousarray(U6.transpose(1, 4, 3, 0, 2, 5).reshape(4, 4, 128, 32 * 128))
    # uw dims now [pc, ocb, i, (pr icb o)]
    shared["uw"] = np.ascontiguousarray(uw.reshape(16, 128, 32 * 128).astype(BF16))
    shared["gamma"] = np.ascontiguousarray(np.asarray(inputs["bn_gamma"], f32).reshape(OUT, 1))
    shared["beta"] = np.ascontiguousarray(np.asarray(inputs["bn_beta"], f32).reshape(OUT, 1))
    shared["ident"] = np.eye(128, dtype=f32)
    shared["identb"] = np.eye(128, dtype=f32).astype(BF16)
    shared["ones"] = np.ones((128, 128), f32)
    shared["onesbf"] = np.ones((128, 1), f32).astype(BF16)

    in_maps = []
    for b in range(B):
        m = dict(shared)
        m["x1t"] = np.ascontiguousarray(x1[b].T.astype(BF16))
        m["x2t"] = np.ascontiguousarray(x2[b].T.astype(BF16))
        m["x1h"] = np.ascontiguousarray(x1[b].astype(BF16))
        m["x2h"] = np.ascontiguousarray(x2[b].astype(BF16))
        in_maps.append(m)
    return in_maps


def run(inputs, trace=False):
    nc = _build()
    in_maps = _prep_in_maps(inputs)
    res = bass_utils.run_bass_kernel_spmd(nc, in_maps, list(range(N_CORES)),
                                          trace=trace)
    out = np.stack([np.asarray(res.results[b]["yout"], dtype=np.float32) for b in range(B)])
    return out.reshape(B, OUT, H, W), res


def kernel(**inputs):
    out, _ = run(inputs, trace=bool(int(os.environ.get("BASS_KERNEL_TRACE", "0"))))
    return out


# revision 13
# speedup vs baseline: 1.2867x; 1.0251x over previous
"""Trainium2 Bass kernel for nn_CrossAtt_27711128994442.

Dual cross-attention block: two branches of channel-attention
(softmax(k @ q^T) applied to v) with a sigmoid gate + residual, concat,
3x3 conv (1024 -> 512), training-mode BatchNorm, ReLU.

Sharding: data-parallel over batch (B=8 -> 8 NeuronCores, one batch
element per core).  BatchNorm statistics are all-reduced across the 8
cores in two rounds (sp 0-2 early / sp 3 late) so the first collective
acts as a barrier that removes core skew from the second.

Math notes (per core / batch element, x1 = input1[b], x2 = input2[b],
both [C=512, N=4096]):
  branch1: S1 = (wk1 x1) (wq2 x2)^T = wk1 G wq2^T where G = x1 x2^T
  branch2: S2 = (wk2 x2) (wq1 x1)^T = wk2 G^T wq1^T
so one Gram matrix G serves both branches.  The pooled-mean gate sums
ride on the PE as ones-vector matmuls during the (DMA-bound) G phase.
The residual is folded into the value-projection product:
  out = (a ZT + I) x   with ZT = wv^T P,
so the attention apply writes the conv pad directly (pure copies, split
between ACT and DVE).

The 3x3 conv runs as Winograd F(2x2,3x3).  The conv pad is stored with
even/odd image columns deinterleaved ([*, 66, 2, 33]) so both input-
transform stages read/write stride-1 bf16 and hit the DVE 2x perf mode.
The 16 per-position matmuls accumulate over input channels in PSUM; the
output inverse transform (A^T M A) runs on vector+scalar engines.
BatchNorm stats ride on the inverse-transform output; the final
normalize+ReLU repacks the block layout to row-major (ACT for 3 output
chunks, DVE for 1) with per-(chunk, sp) output DMA.
"""

import os
import numpy as np
import ml_dtypes

import concourse.bass as bass
import concourse.mybir as mybir
import concourse.bacc as bacc
import concourse.tile as tile
from concourse import bass_utils

BF16 = ml_dtypes.bfloat16
F32 = mybir.dt.float32
F32R = mybir.dt.float32r
BF = mybir.dt.bfloat16

N_CORES = 8
B, C, OUT, H, W = 8, 512, 512, 64, 64
N = H * W            # 4096
CB = C // 128        # 4 channel chunks
NT = N // 512        # 8 spatial tiles of 512 (8 image rows each)
NCH = N // 128       # 32 contraction chunks for the Gram matrix
BN_EPS = 1e-5

# Winograd geometry: 32x32 grid of 2x2 output tiles; 4 sp chunks of
# 8 tile-rows (16 image rows) each.
NSP = 4
TR = 8               # tile-rows per sp chunk
TT = TR * 32         # tiles per sp chunk (256)

XH_BUFS = 24         # streaming x-hi tiles resident (3 nt of lookahead)

_CACHE = {}


def _emit(nc, tc, dr):
    """Emit the whole per-core program. dr: dict of DRAM APs."""
    AX = mybir.AxisListType
    ACTF = mybir.ActivationFunctionType
    ALU = mybir.AluOpType

    pads_pool = tc.alloc_tile_pool(name="pads", bufs=1)
    # padded conv-input images, even/odd img columns deinterleaved:
    # [128, 8src, 66row, 2parity, 33] (img col j -> (j%2, j//2));
    # src 0-3 = branch-1 output chunks, 4-7 = branch-2
    pad_all = pads_pool.tile([128, 8, 66, 2, 33], BF, tag="pad_all", name="pad_all")
    nc.vector.memset(pad_all[:, :, 0], 0.0)
    nc.vector.memset(pad_all[:, :, 65], 0.0)
    nc.vector.memset(pad_all[:, :, 1:65, 0, 0], 0.0)
    nc.vector.memset(pad_all[:, :, 1:65, 1, 32], 0.0)

    # streaming x-hi tiles (bf16 [128, 512] each), requested in apply
    # consumption order; the pool rotation sequences their DMAs
    xhp = tc.alloc_tile_pool(name="xhp", bufs=XH_BUFS, side="right")
    # ZT (value-projection, gate+identity folded) lives through the apply
    zsbp = tc.alloc_tile_pool(name="zsb", bufs=1, side="right")
    identb = zsbp.tile([128, 128], BF, tag="identb", name="identb")
    nc.sync.dma_start(identb[:], dr["identb"][:])

    pwv = tc.alloc_tile_pool(name="pwv", bufs=1, side="right")
    # attention probability tiles (1/rowsum folded in), per branch
    P1 = [pwv.tile([128, 512], BF, tag=f"P1_{kb}", name=f"P1_{kb}") for kb in range(CB)]
    P2 = [pwv.tile([128, 512], BF, tag=f"P2_{kb}", name=f"P2_{kb}") for kb in range(CB)]
    # v-projection weights (transposed: [ci, vc]) bf16
    wv1 = [pwv.tile([128, 512], BF, tag=f"wv1_{cb}", name=f"wv1_{cb}") for cb in range(CB)]
    wv2 = [pwv.tile([128, 512], BF, tag=f"wv2_{cb}", name=f"wv2_{cb}") for cb in range(CB)]
    # per-branch gate scalars broadcast to 128 partitions
    abc1 = pwv.tile([128, 1], F32, tag="abc1", name="abc1")
    abc2 = pwv.tile([128, 1], F32, tag="abc2", name="abc2")
    onesg = pwv.tile([1, 128], F32R, tag="onesg", name="onesg")
    av_all = {}

    xht = {}

    # ------------ Phase A1: Gram matrix, pooled sums, softmax, gates ----
    with tc.tile_pool(name="a1sb", bufs=1) as a1sb, \
         tc.tile_pool(name="xt", bufs=5) as xtp, \
         tc.tile_pool(name="wkp", bufs=1) as wkp:

        onescol = a1sb.tile([128, 1], BF, tag="onescol", name="onescol")
        nc.sync.dma_start(onescol[:], dr["onesbf"][:])
        nc.sync.dma_start(onesg[:], dr["ones"][0:1, :])
        ident = a1sb.tile([128, 128], F32R, tag="ident", name="ident")
        nc.sync.dma_start(ident[:], dr["ident"][:])

        # --- G accumulation + pooled sums (PE rides the DMA-bound phase) ---
        poolf1 = a1sb.tile([1, 512], F32, tag="poolf1", name="poolf1")
        poolf2 = a1sb.tile([1, 512], F32, tag="poolf2", name="poolf2")
        with tc.tile_pool(name="ppp", bufs=1, space="PSUM") as ppp:
            pp1 = ppp.tile([1, 512], F32, tag="pp1", name="pp1")
            pp2 = ppp.tile([1, 512], F32, tag="pp2", name="pp2")
            with tc.tile_pool(name="gps", bufs=1, space="PSUM") as gps:
                G_ps = [gps.tile([128, 512], F32, tag=f"G_{cb}", name=f"G_{cb}") for cb in range(CB)]
                for i in range(NCH):
                    t1_ = xtp.tile([128, 512], BF, tag="x1t", name="x1t")
                    t2_ = xtp.tile([128, 512], BF, tag="x2t", name="x2t")
                    nc.sync.dma_start(t1_[:], dr["x1t"][i * 128:(i + 1) * 128, :])
                    nc.sync.dma_start(t2_[:], dr["x2t"][i * 128:(i + 1) * 128, :])
                    st = dict(start=(i == 0), stop=(i == NCH - 1))
                    for cb in range(CB):
                        nc.tensor.matmul(G_ps[cb][:], t1_[:, cb * 128:(cb + 1) * 128], t2_[:], **st)
                    nc.tensor.matmul(pp1[:], onescol[:], t1_[:], **st)
                    nc.tensor.matmul(pp2[:], onescol[:], t2_[:], **st)

                G_sb = [a1sb.tile([128, 512], F32R, tag=f"Gsb_{cb}", name=f"Gsb_{cb}") for cb in range(CB)]
                for cb in range(CB):
                    nc.vector.tensor_copy(G_sb[cb][:], G_ps[cb][:])
            nc.vector.tensor_copy(poolf1[:], pp1[:])
            nc.vector.tensor_copy(poolf2[:], pp2[:])

        # all sandwich weights ride in recycled xt-pool slots; the
        # FIFO slot rotation sequences their DMAs behind the G tail
        # in consumption order (M2 -> M1 -> S2 -> S1)
        wq_b2 = [xtp.tile([128, 512], F32R, tag="wqt", name=f"wqb2_{cb}") for cb in range(CB)]
        wq_b1 = [xtp.tile([128, 512], F32R, tag="wqt", name=f"wqb1_{cb}") for cb in range(CB)]
        wk_b2 = [wkp.tile([128, 512], F32R, tag=f"wkb2_{cb}", name=f"wkb2_{cb}") for cb in range(CB)]
        wk_b1 = [xtp.tile([128, 512], F32R, tag="wqt", name=f"wkb1_{cb}") for cb in range(CB)]
        for cb in range(CB):
            cs = slice(cb * 128, (cb + 1) * 128)
            nc.sync.dma_start(wq_b2[cb][:], dr["wq1t"][cs, :])
            nc.sync.dma_start(wq_b1[cb][:], dr["wq2t"][cs, :])
        for cb in range(CB):
            cs = slice(cb * 128, (cb + 1) * 128)
            nc.sync.dma_start(wk_b2[cb][:], dr["wk2t"][cs, :])
            nc.sync.dma_start(wk_b1[cb][:], dr["wk1t"][cs, :])

        # v-weights land before the bulk x-hi loads: the ZT
        # matmuls need them right after the softmax
        for cb in range(CB):
            nc.sync.dma_start(wv2[cb][:], dr["wv2n"][cb * 128:(cb + 1) * 128, :])
            nc.sync.dma_start(wv1[cb][:], dr["wv1n"][cb * 128:(cb + 1) * 128, :])

        # x-hi streaming tiles, requested in apply consumption order
        def req_xh(inp, cib, nt):
            t = xhp.tile([128, 512], BF, tag="xht", name=f"xh_{inp}_{cib}_{nt}")
            src = dr["x1h"] if inp == "x1" else dr["x2h"]
            nc.sync.dma_start(t[:], src[cib * 128:(cib + 1) * 128,
                                        nt * 512:(nt + 1) * 512])
            xht[(inp, cib, nt)] = t
        for nt in range(3):
            for cib in range(CB):
                req_xh("x2", cib, nt)
        for nt in range(3):
            for cib in range(CB):
                req_xh("x1", cib, nt)
        for nt in range(3, NT):
            for cib in range(CB):
                req_xh("x2", cib, nt)
            for cib in range(CB):
                req_xh("x1", cib, nt)

        # --- transpose G -> GT (for branch 1) ---
        GT_sb = [a1sb.tile([128, 512], F32R, tag=f"GTsb_{cb}", name=f"GTsb_{cb}") for cb in range(CB)]
        with tc.tile_pool(name="trp", bufs=2, space="PSUM") as trp:
            for c2b in range(CB):
                for c1b in range(CB):
                    tp = trp.tile([128, 128], F32R, tag="tr", name="tr")
                    nc.tensor.transpose(tp[:], G_sb[c1b][:, c2b * 128:(c2b + 1) * 128], ident[:])
                    nc.vector.tensor_copy(GT_sb[c2b][:, c1b * 128:(c1b + 1) * 128], tp[:])

        wlf = a1sb.tile([1, 512], F32, tag="wlf", name="wlf")
        nc.sync.dma_start(wlf[:], dr["wlinf"][:])

        # --- branch sandwiches + exp ---
        # branch 1: S1 = wk1 (G wq2^T)   via lhsT=GT, then lhsT=wk1t
        # branch 2: S2 = wk2 (G^T wq1^T) via lhsT=G,  then lhsT=wk2t
        rs_all = {}
        branches = [(G_sb, wq_b2, wk_b2, P2), (GT_sb, wq_b1, wk_b1, P1)]
        M_sbs = {}
        with tc.tile_pool(name="msps", bufs=1, space="PSUM") as msps:
            for bi, (Gl, wq, wk, Pt) in enumerate(branches):
                M_ps = [msps.tile([128, 512], F32, tag=f"b{bi}_{cb}", name=f"M{bi}_{cb}") for cb in range(CB)]
                for cb in range(CB):
                    for kb in range(CB):
                        nc.tensor.matmul(M_ps[cb][:], Gl[kb][:, cb * 128:(cb + 1) * 128],
                                         wq[kb][:], start=(kb == 0), stop=(kb == CB - 1))
                M_sb = [a1sb.tile([128, 512], F32R, tag=f"Msb{bi}_{cb}", name=f"Msb{bi}_{cb}") for cb in range(CB)]
                for cb in range(CB):
                    nc.vector.tensor_copy(M_sb[cb][:], M_ps[cb][:])
                M_sbs[bi] = M_sb
            # S tiles reuse the same tags as the M banks they replace
            for bi, (Gl, wq, wk, Pt) in enumerate(branches):
                M_sb = M_sbs[bi]
                S_ps = [msps.tile([128, 512], F32, tag=f"b{bi}_{kb}", name=f"S{bi}_{kb}") for kb in range(CB)]
                for kb in range(CB):
                    for cb in range(CB):
                        nc.tensor.matmul(S_ps[kb][:], wk[cb][:, kb * 128:(kb + 1) * 128],
                                         M_sb[cb][:], start=(cb == 0), stop=(cb == CB - 1))
                for kb in range(CB):
                    nmx = a1sb.tile([128, 1], F32, tag="nmx", name="nmx", bufs=2)
                    nc.vector.reduce_max(nmx[:], S_ps[kb][:], axis=AX.X, negate=True)
                    rs = a1sb.tile([128, 1], F32, tag=f"rs{bi}_{kb}", name=f"rs{bi}_{kb}")
                    nc.scalar.activation(Pt[kb][:], S_ps[kb][:],
                                         ACTF.Exp,
                                         bias=nmx[:], accum_out=rs[:])
                    rs_all[(bi, kb)] = rs
                # gate sigmoid for this branch (tiny DVE+ACT): queues
                # right behind this branch's exps; the broadcast matmul
                # happens in the ZT loop so it never head-blocks the PE
                pf = poolf2 if bi == 0 else poolf1
                pm = a1sb.tile([1, 512], F32, tag=f"pm{bi}", name=f"pm{bi}")
                nc.vector.tensor_mul(pm[:], pf[:], wlf[:])
                prs = a1sb.tile([1, 1], F32, tag=f"prs{bi}", name=f"prs{bi}")
                nc.vector.reduce_sum(prs[:], pm[:], axis=AX.X)
                av = pwv.tile([1, 2], F32R, tag=f"av{bi}", name=f"av{bi}")
                nc.scalar.activation(av[:], prs[:].to_broadcast((1, 2)),
                                     ACTF.Sigmoid, scale=1.0 / float(N))
                av_all[bi] = av

        # preload the sqrt ACT table set now (it also contains Copy/
        # Square/Relu, i.e. everything phase B + finalize uses) so the
        # BN-finalize tail pays no table switch.
        sqd = a1sb.tile([1, 1], F32, tag="sqd", name="sqd")
        nc.scalar.activation(sqd[:], poolf1[0:1, 0:1], ACTF.Sqrt)

        # fold 1/rowsum into P (gate + identity fold into ZT later)
        for gbi, Pt in enumerate([P2, P1]):
            for kb in range(CB):
                rs = rs_all[(gbi, kb)]
                ri = a1sb.tile([128, 1], F32, tag="ri", name="ri", bufs=2)
                nc.vector.reciprocal(ri[:], rs[:])
                nc.vector.tensor_scalar_mul(Pt[kb][:], Pt[kb][:], ri[:])

    # ------------ ZT for both branches: ZT = a * (wv^T P) + I -----------
    # re-associated: ZT[ci,c] = a * sum_kc wv[kc,ci] P[kc,c] + I[ci,c]
    # then out[c,n] = sum_ci ZT[ci,c] x[ci,n] includes gate AND residual.
    ZT_all = []
    with tc.tile_pool(name="zps", bufs=1, space="PSUM") as zps:
        for br_i, (Pt, wv, abc) in enumerate([(P2, wv2, abc2), (P1, wv1, abc1)]):
            bc_ps = zps.tile([128, 512], F32, tag="bcg", name=f"bc{br_i}", bufs=2)
            nc.tensor.matmul(bc_ps[:, 0:2], onesg[:], av_all[br_i][:], start=True, stop=True)
            nc.vector.tensor_copy(abc[:], bc_ps[:, 0:1])
            ZT_sb = []
            for cib in range(CB):
                z_ps = zps.tile([128, 512], F32, tag=f"zps_{cib}", name=f"zps_{br_i}_{cib}")
                for kb in range(CB):
                    nc.tensor.matmul(z_ps[:], wv[kb][:, cib * 128:(cib + 1) * 128],
                                     Pt[kb][:], start=(kb == 0), stop=(kb == CB - 1))
                zt = zsbp.tile([128, 512], BF, tag=f"zt_{br_i}_{cib}", name=f"zt_{br_i}_{cib}")
                nc.vector.tensor_scalar_mul(zt[:], z_ps[:], abc[:])
                cs = slice(cib * 128, (cib + 1) * 128)
                nc.vector.tensor_add(zt[:, cs], zt[:, cs], identb[:])
                ZT_sb.append(zt)
            ZT_all.append(ZT_sb)
    pwv.release()

    # winograd transform pools open early so stage1/2 of sp 0 can overlap
    # the apply tail
    rp = tc.alloc_tile_pool(name="rp", bufs=1)
    vp = tc.alloc_tile_pool(name="vp", bufs=2)
    up = tc.alloc_tile_pool(name="up", bufs=3)

    # k index = pr*8 + src (matches the host U layout)
    def emit_stage1(sp):
        """rows transform: R_all[128, 32k, TR, 2, 33] bf16 (DVE)"""
        r0 = 16 * sp
        rt = rp.tile([128, 32, TR, 2, 33], BF, tag="R_all", name=f"R_all_{sp}")
        a = pad_all[:, :, r0 + 0:r0 + 16:2]
        b = pad_all[:, :, r0 + 2:r0 + 18:2]
        c = pad_all[:, :, r0 + 1:r0 + 17:2]
        d = pad_all[:, :, r0 + 3:min(r0 + 19, 66):2]
        nc.vector.tensor_sub(rt[:, 0:8], a, b)
        nc.vector.tensor_add(rt[:, 8:16], c, b)
        nc.vector.tensor_sub(rt[:, 16:24], b, c)
        nc.vector.tensor_sub(rt[:, 24:32], c, d)
        return rt

    def emit_stage2(rt, sp, pc, gp_k):
        """cols transform: V_all[128, 32k, TT] bf16 (DVE + GPSIMD).
        E/O pad layout makes all four operands stride-1 -> DVE 2x."""
        vt = vp.tile([128, 32, TT], BF, tag="V_all", name=f"V_all_{sp}_{pc}")
        vv = vt.rearrange("p k (a b) -> p k a b", a=TR)
        e = rt[:, :, :, 0, 0:32]
        m = rt[:, :, :, 1, 0:32]
        q = rt[:, :, :, 0, 1:33]
        s = rt[:, :, :, 1, 1:33]
        lo = slice(0, 32 - gp_k)
        hi = slice(32 - gp_k, 32)
        if pc == 0:
            nc.vector.tensor_sub(vv[:, lo], e[:, lo], q[:, lo])
            if gp_k:
                nc.gpsimd.tensor_sub(vv[:, hi], e[:, hi], q[:, hi])
        elif pc == 1:
            nc.vector.tensor_add(vv[:, lo], m[:, lo], q[:, lo])
            if gp_k:
                nc.gpsimd.tensor_add(vv[:, hi], m[:, hi], q[:, hi])
        elif pc == 2:
            nc.vector.tensor_sub(vv[:, lo], q[:, lo], m[:, lo])
            if gp_k:
                nc.gpsimd.tensor_sub(vv[:, hi], q[:, hi], m[:, hi])
        else:
            nc.vector.tensor_sub(vv[:, lo], m[:, lo], s[:, lo])
            if gp_k:
                nc.gpsimd.tensor_sub(vv[:, hi], m[:, hi], s[:, hi])
        return vt

    def gp_k_for(sp, pc):
        # keep the gpsimd queue empty near the tail so the stats
        # collective trigger is never stuck behind transform work
        return 0 if (sp == 3 and pc >= 2) else 4

    # ------------ apply (nt-major): pad[c,n] = sum_ci ZT[ci,c] x[ci,n] --
    opsp = tc.alloc_tile_pool(name="ops", bufs=1, space="PSUM")
    apply_seq = [(br, nt) for nt in range(3) for br in [0]] \
        + [(br, nt) for nt in range(3) for br in [1]]
    apply_tail = [(br, nt) for nt in range(3, NT) for br in (0, 1)]
    g_ctr = [0]

    def apply_group(br_i, nt, act_both):
        inp = "x2" if br_i == 0 else "x1"
        pad_base = 4 if br_i == 0 else 0
        rows = slice(1 + nt * 8, 9 + nt * 8)
        for cb in range(CB):
            g = g_ctr[0]
            g_ctr[0] += 1
            o_ps = opsp.tile([128, 512], F32, tag=f"ops_{g % 4}", name=f"ops_{br_i}_{cb}_{nt}")
            for cib in range(CB):
                nc.tensor.matmul(o_ps[:], ZT_all[br_i][cib][:, cb * 128:(cb + 1) * 128],
                                 xht[(inp, cib, nt)][:], start=(cib == 0), stop=(cib == CB - 1))
            src3 = o_ps.rearrange("p (a b) -> p a b", a=8)
            # src col jc (0-based, img col jc+1): even jc -> odd img col
            # -> parity 1 idx jc/2; odd jc -> even img col -> parity 0.
            dst_o = pad_all[:, pad_base + cb, rows, 1, 0:32]
            dst_e = pad_all[:, pad_base + cb, rows, 0, 1:33]
            nc.scalar.activation(dst_o, src3[:, :, 0:64:2], ACTF.Copy)
            if act_both:
                nc.scalar.activation(dst_e, src3[:, :, 1:64:2], ACTF.Copy)
            else:
                nc.vector.tensor_copy(dst_e, src3[:, :, 1:64:2])

    for br_i, nt in apply_seq:
        apply_group(br_i, nt, act_both=False)
    # stage1/2 for sp 0 run on DVE during the apply tail; the tail's pad
    # copies go ACT-only while the DVE chews the transforms
    R = emit_stage1(0)
    V = emit_stage2(R, 0, 0, gp_k_for(0, 0))
    for br_i, nt in apply_tail:
        apply_group(br_i, nt, act_both=(nt in (3, 4)))
    opsp.release()
    zsbp.release()
    xhp.release()

    # ------------ Phase B: Winograd F(2x2,3x3) conv + BN ----------------
    ybp = tc.alloc_tile_pool(name="ybp", bufs=1, side="right")
    bsb = tc.alloc_tile_pool(name="bsb", bufs=1, side="right")
    dram = tc.alloc_tile_pool(name="dram", bufs=1, space="DRAM")
    t1p = tc.alloc_tile_pool(name="t1p", bufs=2)
    map_ = tc.alloc_tile_pool(name="map", bufs=2)
    mps = tc.alloc_tile_pool(name="mps", bufs=2, space="PSUM")

    # conv output in Winograd block layout: [128, ocb, sp, r, j, 256]
    yb_all = ybp.tile([128, CB, NSP, 2, 2, TT], BF, tag="yb_all", name="yb_all")

    stats_a = bsb.tile([128, 2 * CB], F32, tag="stats_a", name="stats_a")
    stats_b = bsb.tile([128, 2 * CB], F32, tag="stats_b", name="stats_b")
    nc.vector.memset(stats_a[:], 0.0)
    nc.vector.memset(stats_b[:], 0.0)
    s_in1 = dram.tile([128, 2 * CB], F32, tag="arin1", name="arin1")
    s_out1 = dram.tile([128, 2 * CB], F32, tag="arout1", name="arout1")
    s_in2 = dram.tile([128, 2 * CB], F32, tag="arin2", name="arin2")
    s_out2 = dram.tile([128, 2 * CB], F32, tag="arout2", name="arout2")

    def emit_stats(dst_stats, ob, ysl, nel):
        ts = bsb.tile([128, 1], F32, tag="tsum", name="tsum", bufs=2)
        sc1 = bsb.tile([128, nel], BF, tag="scr", name="scr", bufs=1)
        nc.scalar.activation(sc1[:], ysl, ACTF.Copy, accum_out=ts[:])
        tq = bsb.tile([128, 1], F32, tag="tsq", name="tsq", bufs=2)
        sc2 = bsb.tile([128, nel], BF, tag="scr2", name="scr2", bufs=1)
        nc.scalar.activation(sc2[:], ysl, ACTF.Square, accum_out=tq[:])
        nc.vector.tensor_add(dst_stats[:, 2 * ob:2 * ob + 1], dst_stats[:, 2 * ob:2 * ob + 1], ts[:])
        nc.vector.tensor_add(dst_stats[:, 2 * ob + 1:2 * ob + 2], dst_stats[:, 2 * ob + 1:2 * ob + 2], tq[:])

    phases = [(sp, pc) for sp in range(NSP) for pc in range(4)]
    for idx, (sp, pc) in enumerate(phases):
        Vcur = V
        # ---- PE: the 16-position matmuls for this (sp, pc) ----
        mts = []
        for pair in range(2):
            # M PSUM for an ocb pair: [128, 4pr, 2x256] f32
            mt = mps.tile([128, 4, 2 * TT], F32, tag="mt", name=f"mt_{sp}_{pc}_{pair}")
            for half in range(2):
                ocb = pair * 2 + half
                u = up.tile([128, 32 * 128], BF, tag="u", name=f"u_{sp}_{pc}_{ocb}")
                nc.sync.dma_start(u[:], dr["uw"][pc * 4 + ocb])
                hs = slice(half * TT, (half + 1) * TT)
                for icb in range(8):
                    st = dict(start=(icb == 0), stop=(icb == 7))
                    for pr in range(4):
                        nc.tensor.matmul(mt[:, pr, hs],
                                         u[:, (pr * 8 + icb) * 128:(pr * 8 + icb + 1) * 128],
                                         Vcur[:, pr * 8 + icb, :], **st)
            mts.append(mt)

        # ---- DVE: pre-emit NEXT phase transforms (FIFO order) ----
        if idx + 1 < len(phases):
            sp2, pc2 = phases[idx + 1]
            if pc2 == 0:
                R = emit_stage1(sp2)
            V = emit_stage2(R, sp2, pc2, gp_k_for(sp2, pc2))

        # ---- inverse transforms for this phase ----
        for pair in range(2):
            mt = mts[pair]
            ph = slice(pair * 2, pair * 2 + 2)
            # rows (invA): PSUM reads all on ACT, adds on DVE (bf16 2x)
            m0 = map_.tile([128, 2 * TT], BF, tag="m0", name=f"m0_{sp}_{pc}_{pair}")
            m1 = map_.tile([128, 2 * TT], BF, tag="m1", name=f"m1_{sp}_{pc}_{pair}")
            m2 = map_.tile([128, 2 * TT], BF, tag="m2", name=f"m2_{sp}_{pc}_{pair}")
            m3 = map_.tile([128, 2 * TT], BF, tag="m3", name=f"m3_{sp}_{pc}_{pair}")
            nc.scalar.activation(m0[:], mt[:, 0, :], ACTF.Copy)
            nc.scalar.activation(m1[:], mt[:, 1, :], ACTF.Copy)
            nc.scalar.activation(m2[:], mt[:, 2, :], ACTF.Copy)
            nc.scalar.activation(m3[:], mt[:, 3, :], ACTF.Copy)
            t1 = t1p.tile([128, 2, 2 * TT], BF, tag="t1", name=f"t1_{sp}_{pc}_{pair}")
            nc.vector.tensor_add(t1[:, 0, :], m0[:], m1[:])
            nc.vector.tensor_add(t1[:, 0, :], t1[:, 0, :], m2[:])
            nc.vector.tensor_sub(t1[:, 1, :], m1[:], m2[:])
            nc.vector.tensor_sub(t1[:, 1, :], t1[:, 1, :], m3[:])
            # cols (invB): ops span the ocb pair (FD 512)
            for r in range(2):
                tr_ = t1[:, r, :].rearrange("p (o t) -> p o t", o=2)
                y0 = yb_all[:, ph, sp, r, 0, :]
                y1 = yb_all[:, ph, sp, r, 1, :]
                if pc == 0:
                    nc.scalar.activation(y0, tr_, ACTF.Copy)
                elif pc == 1:
                    nc.vector.tensor_add(y0, y0, tr_)
                    nc.scalar.activation(y1, tr_, ACTF.Copy)
                elif pc == 2:
                    nc.vector.tensor_add(y0, y0, tr_)
                    nc.vector.tensor_sub(y1, y1, tr_)
                else:
                    nc.vector.tensor_sub(y1, y1, tr_)
            if sp == 3 and pc >= 2:
                jj = pc - 2   # j=0 final after pc2, j=1 after pc3
                for ob in (pair * 2, pair * 2 + 1):
                    ysl = yb_all[:, ob, 3, :, jj, :]
                    emit_stats(stats_b, ob, ysl, 2 * TT)

        # ---- BN stats; sp3 split by j so the tail only waits on j=1 ----
        if sp < 3 and pc == 3:
            for ob in range(CB):
                ysl = yb_all[:, ob, sp].rearrange("p a b t -> p (a b t)")
                emit_stats(stats_a, ob, ysl, 4 * TT)
        elif sp == 3 and pc == 1:
            # early collective: sp 0-2 stats; doubles as a barrier late
            # enough that the final (sp3) collective sees little skew
            nc.sync.dma_start(s_in1[:], stats_a[:])
            nc.gpsimd.collective_compute(
                "AllReduce", mybir.AluOpType.add,
                replica_groups=[list(range(N_CORES))],
                ins=[s_in1.opt()], outs=[s_out1.opt()])


    nc.sync.dma_start(s_in2[:], stats_b[:])
    nc.gpsimd.collective_compute(
        "AllReduce", mybir.AluOpType.add,
        replica_groups=[list(range(N_CORES))],
        ins=[s_in2.opt()], outs=[s_out2.opt()])

    mps.release()
    map_.release()
    t1p.release()
    up.release()
    vp.release()
    rp.release()

    # ---- finalize: scale/shift for all 4 ocb at once, then repack ------
    with tc.tile_pool(name="fin", bufs=1) as fin:
        sall1 = fin.tile([128, 2 * CB], F32, tag="sall1", name="sall1")
        sall2 = fin.tile([128, 2 * CB], F32, tag="sall2", name="sall2")
        nc.sync.dma_start(sall1[:], s_out1[:])
        nc.sync.dma_start(sall2[:], s_out2[:])
        sall = fin.tile([128, 2 * CB], F32, tag="sall", name="sall")
        nc.vector.tensor_add(sall[:], sall1[:], sall2[:])

        gam = fin.tile([128, CB], F32, tag="gam", name="gam")
        bet = fin.tile([128, CB], F32, tag="bet", name="bet")
        nc.sync.dma_start(gam[:], dr["gamma"].rearrange("(c p) one -> p (c one)", p=128))
        nc.sync.dma_start(bet[:], dr["beta"].rearrange("(c p) one -> p (c one)", p=128))
        inv_n = 1.0 / float(B * N)
        eps_t = fin.tile([128, 1], F32, tag="eps", name="eps")
        nc.vector.memset(eps_t[:], BN_EPS)

        mean4 = fin.tile([128, CB], F32, tag="mean4", name="mean4")
        nc.vector.tensor_scalar_mul(mean4[:], sall[:, 0:2 * CB:2], inv_n)
        ex24 = fin.tile([128, CB], F32, tag="ex24", name="ex24")
        nc.vector.tensor_scalar_mul(ex24[:], sall[:, 1:2 * CB:2], inv_n)
        m2s = fin.tile([128, CB], F32, tag="m2s", name="m2s")
        nc.vector.tensor_mul(m2s[:], mean4[:], mean4[:])
        var4 = fin.tile([128, CB], F32, tag="var4", name="var4")
        nc.vector.tensor_sub(var4[:], ex24[:], m2s[:])
        std4 = fin.tile([128, CB], F32, tag="std4", name="std4")
        nc.scalar.activation(std4[:], var4[:], ACTF.Sqrt, bias=eps_t[:])
        inv4 = fin.tile([128, CB], F32, tag="inv4", name="inv4")
        nc.vector.reciprocal(inv4[:], std4[:])
        sc4 = fin.tile([128, CB], F32, tag="sc4", name="sc4")
        nc.vector.tensor_mul(sc4[:], gam[:], inv4[:])
        ms4 = fin.tile([128, CB], F32, tag="ms4", name="ms4")
        nc.vector.tensor_mul(ms4[:], mean4[:], sc4[:])
        tt4 = fin.tile([128, CB], F32, tag="tt4", name="tt4")
        nc.vector.tensor_sub(tt4[:], bet[:], ms4[:])

        # normalize + ReLU + repack block layout -> row-major; ACT takes
        # 3 output chunks, DVE the 4th; output DMA per (ob, sp) slab
        for ob in [3, 0, 1, 2]:
            onat = fin.tile([128, 64, 64], BF, tag="onat", name=f"onat_{ob}", bufs=4)
            for sp in range(NSP):
                for r in range(2):
                    src = yb_all[:, ob, sp, r].rearrange("p j (a b) -> p j a b", a=TR)
                    dst = onat[:, 16 * sp + r:min(16 * sp + r + 16, 64):2, :]
                    dst = dst.rearrange("p a (b j) -> p j a b", j=2)
                    if ob < 3:
                        nc.scalar.activation(dst, src, ACTF.Relu,
                                             bias=tt4[:, ob:ob + 1], scale=sc4[:, ob:ob + 1])
                    else:
                        nc.vector.tensor_scalar(dst, src, sc4[:, ob:ob + 1], tt4[:, ob:ob + 1],
                                                ALU.mult, ALU.add)
                if ob >= 3:
                    pl = onat[:, 16 * sp:16 * sp + 16, :]
                    nc.vector.tensor_scalar_max(pl, pl, 0.0)
                nc.sync.dma_start(
                    dr["yout"][ob * 128:(ob + 1) * 128, sp * 1024:(sp + 1) * 1024],
                    onat[:, 16 * sp:16 * sp + 16, :].rearrange("p a b -> p (a b)"))

    bsb.release()
    ybp.release()
    dram.release()
    pads_pool.release()


def _build():
    if "nc" in _CACHE:
        return _CACHE["nc"]
    nc = bacc.Bacc("TRN2", target_bir_lowering=False, debug=False,
                   num_devices=N_CORES)
    dr = {}
    def din(name, shape, dt):
        dr[name] = nc.dram_tensor(name, shape, dt, kind="ExternalInput").ap()
    din("x1t", [N, C], BF)
    din("x2t", [N, C], BF)
    din("x1h", [C, N], BF)
    din("x2h", [C, N], BF)
    for w in ["wq1t", "wq2t", "wk1t", "wk2t"]:
        din(w, [C, C], F32R)
    for w in ["wv1n", "wv2n"]:
        din(w, [C, C], BF)
    din("wlinf", [1, C], F32)
    din("uw", [16, 128, 32 * 128], BF)
    din("gamma", [OUT, 1], F32)
    din("beta", [OUT, 1], F32)
    din("ident", [128, 128], F32R)
    din("identb", [128, 128], BF)
    din("ones", [128, 128], F32R)
    din("onesbf", [128, 1], BF)
    dr["yout"] = nc.dram_tensor("yout", [OUT, N], BF, kind="ExternalOutput").ap()

    with tile.TileContext(nc) as tc:
        _emit(nc, tc, dr)
    nc.compile()
    _CACHE["nc"] = nc
    return nc


def _prep_in_maps(inputs):
    f32 = np.float32
    x1 = np.ascontiguousarray(inputs["input1"], f32).reshape(B, C, N)
    x2 = np.ascontiguousarray(inputs["input2"], f32).reshape(B, C, N)
    shared = {}
    for w in ["wq1", "wq2", "wk1", "wk2"]:
        shared[w + "t"] = np.ascontiguousarray(np.asarray(inputs[w], f32).T)
    for w in ["wv1", "wv2"]:
        shared[w + "n"] = np.ascontiguousarray(np.asarray(inputs[w], f32).astype(BF16))
    shared["wlinf"] = np.ascontiguousarray(np.asarray(inputs["w_lin"], f32).reshape(1, C))
    # Winograd weight transform on host: U[pr,pc][ic,oc] = G g G^T
    g = np.asarray(inputs["w_cat"], f32)                     # [OUT, 2C, 3, 3]
    Gm = np.array([[1, 0, 0], [0.5, 0.5, 0.5], [0.5, -0.5, 0.5], [0, 0, 1]], f32)
    U = np.einsum('rj,oijk,ck->rcio', Gm, g, Gm)             # [4,4,2C,OUT]
    # layout: uw[pc*4+ocb][ic_in_chunk][pr, icb, oc] as [16, 128, 4096]
    U6 = U.reshape(4, 4, 8, 128, 4, 128)                     # [pr,pc,icb,i,ocb,o]
    uw = np.ascontiguousarray(U6.transpose(1, 4, 3, 0, 2, 5).reshape(4, 4, 128, 32 * 128))
    # uw dims now [pc, ocb, i, (pr icb o)]
    shared["uw"] = np.ascontiguousarray(uw.reshape(16, 128, 32 * 128).astype(BF16))
    shared["gamma"] = np.ascontiguousarray(np.asarray(inputs["bn_gamma"], f32).reshape(OUT, 1))
    shared["beta"] = np.ascontiguousarray(np.asarray(inputs["bn_beta"], f32).reshape(OUT, 1))
    shared["ident"] = np.eye(128, dtype=f32)
    shared["identb"] = np.eye(128, dtype=f32).astype(BF16)
    shared["ones"] = np.ones((128, 128), f32)
    shared["onesbf"] = np.ones((128, 1), f32).astype(BF16)

    in_maps = []
    for b in range(B):
        m = dict(shared)
        m["x1t"] = np.ascontiguousarray(x1[b].T.astype(BF16))
        m["x2t"] = np.ascontiguousarray(x2[b].T.astype(BF16))
        m["x1h"] = np.ascontiguousarray(x1[b].astype(BF16))
        m["x2h"] = np.ascontiguousarray(x2[b].astype(BF16))
        in_maps.append(m)
    return in_maps


def run(inputs, trace=False):
    nc = _build()
    in_maps = _prep_in_maps(inputs)
    res = bass_utils.run_bass_kernel_spmd(nc, in_maps, list(range(N_CORES)),
                                          trace=trace)
    out = np.stack([np.asarray(res.results[b]["yout"], dtype=np.float32) for b in range(B)])
    return out.reshape(B, OUT, H, W), res


def kernel(**inputs):
    out, _ = run(inputs, trace=bool(int(os.environ.get("BASS_KERNEL_TRACE", "0"))))
    return out


# revision 14
# speedup vs baseline: 1.3020x; 1.0119x over previous
"""Trainium2 Bass kernel for nn_CrossAtt_27711128994442.

Dual cross-attention block: two branches of channel-attention
(softmax(k @ q^T) applied to v) with a sigmoid gate + residual, concat,
3x3 conv (1024 -> 512), training-mode BatchNorm, ReLU.

Sharding: data-parallel over batch (B=8 -> 8 NeuronCores, one batch
element per core).  BatchNorm statistics are all-reduced across the 8
cores in two rounds (sp 0-2 early / sp 3 late) so the first collective
acts as a barrier that removes core skew from the second.

Math notes (per core / batch element, x1 = input1[b], x2 = input2[b],
both [C=512, N=4096]):
  branch1: S1 = (wk1 x1) (wq2 x2)^T = wk1 G wq2^T where G = x1 x2^T
  branch2: S2 = (wk2 x2) (wq1 x1)^T = wk2 G^T wq1^T
so one Gram matrix G serves both branches.  The pooled-mean gate sums
ride on the PE as ones-vector matmuls during the (DMA-bound) G phase.
The residual is folded into the value-projection product:
  out = (a ZT + I) x   with ZT = wv^T P,
so the attention apply writes the conv pad directly (pure copies, split
between ACT and DVE).

The 3x3 conv runs as Winograd F(2x2,3x3).  The conv pad is stored with
even/odd image columns deinterleaved ([*, 66, 2, 33]) so both input-
transform stages read/write stride-1 bf16 and hit the DVE 2x perf mode.
The 16 per-position matmuls accumulate over input channels in PSUM; the
output inverse transform (A^T M A) runs on vector+scalar engines.
BatchNorm stats ride on the inverse-transform output; the final
normalize+ReLU repacks the block layout to row-major (ACT for 3 output
chunks, DVE for 1) with per-(chunk, sp) output DMA.
"""

import os
import numpy as np
import ml_dtypes

import concourse.bass as bass
import concourse.mybir as mybir
import concourse.bacc as bacc
import concourse.tile as tile
from concourse import bass_utils

BF16 = ml_dtypes.bfloat16
F32 = mybir.dt.float32
F32R = mybir.dt.float32r
BF = mybir.dt.bfloat16

N_CORES = 8
B, C, OUT, H, W = 8, 512, 512, 64, 64
N = H * W            # 4096
CB = C // 128        # 4 channel chunks
NT = N // 512        # 8 spatial tiles of 512 (8 image rows each)
NCH = N // 128       # 32 contraction chunks for the Gram matrix
BN_EPS = 1e-5

# Winograd geometry: 32x32 grid of 2x2 output tiles; 4 sp chunks of
# 8 tile-rows (16 image rows) each.
NSP = 4
TR = 8               # tile-rows per sp chunk
TT = TR * 32         # tiles per sp chunk (256)

XH_BUFS = 24         # streaming x-hi tiles resident (3 nt of lookahead)

_CACHE = {}


def _emit(nc, tc, dr):
    """Emit the whole per-core program. dr: dict of DRAM APs."""
    AX = mybir.AxisListType
    ACTF = mybir.ActivationFunctionType
    ALU = mybir.AluOpType

    pads_pool = tc.alloc_tile_pool(name="pads", bufs=1)
    # padded conv-input images, even/odd img columns deinterleaved:
    # [128, 8src, 66row, 2parity, 33] (img col j -> (j%2, j//2));
    # src 0-3 = branch-1 output chunks, 4-7 = branch-2
    pad_all = pads_pool.tile([128, 8, 66, 2, 33], BF, tag="pad_all", name="pad_all")
    nc.vector.memset(pad_all[:, :, 0], 0.0)
    nc.vector.memset(pad_all[:, :, 65], 0.0)
    nc.vector.memset(pad_all[:, :, 1:65, 0, 0], 0.0)
    nc.vector.memset(pad_all[:, :, 1:65, 1, 32], 0.0)

    # streaming x-hi tiles (bf16 [128, 512] each), requested in apply
    # consumption order; the pool rotation sequences their DMAs
    xhp = tc.alloc_tile_pool(name="xhp", bufs=XH_BUFS, side="right")
    # ZT (value-projection, gate+identity folded) lives through the apply
    zsbp = tc.alloc_tile_pool(name="zsb", bufs=1, side="right")
    identb = zsbp.tile([128, 128], BF, tag="identb", name="identb")
    nc.sync.dma_start(identb[:], dr["identb"][:])

    pwv = tc.alloc_tile_pool(name="pwv", bufs=1, side="right")
    # attention probability tiles (1/rowsum folded in), per branch
    P1 = [pwv.tile([128, 512], BF, tag=f"P1_{kb}", name=f"P1_{kb}") for kb in range(CB)]
    P2 = [pwv.tile([128, 512], BF, tag=f"P2_{kb}", name=f"P2_{kb}") for kb in range(CB)]
    # v-projection weights (transposed: [ci, vc]) bf16
    wv1 = [pwv.tile([128, 512], BF, tag=f"wv1_{cb}", name=f"wv1_{cb}") for cb in range(CB)]
    wv2 = [pwv.tile([128, 512], BF, tag=f"wv2_{cb}", name=f"wv2_{cb}") for cb in range(CB)]
    # per-branch gate scalars broadcast to 128 partitions
    abc1 = pwv.tile([128, 1], F32, tag="abc1", name="abc1")
    abc2 = pwv.tile([128, 1], F32, tag="abc2", name="abc2")
    onesg = pwv.tile([1, 128], F32R, tag="onesg", name="onesg")
    av_all = {}

    xht = {}

    # ------------ Phase A1: Gram matrix, pooled sums, softmax, gates ----
    with tc.tile_pool(name="a1sb", bufs=1) as a1sb, \
         tc.tile_pool(name="xt", bufs=5) as xtp, \
         tc.tile_pool(name="wkp", bufs=1) as wkp:

        onescol = a1sb.tile([128, 1], BF, tag="onescol", name="onescol")
        nc.sync.dma_start(onescol[:], dr["onesbf"][:])
        nc.sync.dma_start(onesg[:], dr["ones"][0:1, :])
        ident = a1sb.tile([128, 128], F32R, tag="ident", name="ident")
        nc.sync.dma_start(ident[:], dr["ident"][:])

        # --- G accumulation + pooled sums (PE rides the DMA-bound phase) ---
        poolf1 = a1sb.tile([1, 512], F32, tag="poolf1", name="poolf1")
        poolf2 = a1sb.tile([1, 512], F32, tag="poolf2", name="poolf2")
        with tc.tile_pool(name="ppp", bufs=1, space="PSUM") as ppp:
            pp1 = ppp.tile([1, 512], F32, tag="pp1", name="pp1")
            pp2 = ppp.tile([1, 512], F32, tag="pp2", name="pp2")
            with tc.tile_pool(name="gps", bufs=1, space="PSUM") as gps:
                G_ps = [gps.tile([128, 512], F32, tag=f"G_{cb}", name=f"G_{cb}") for cb in range(CB)]
                for i in range(NCH):
                    t1_ = xtp.tile([128, 512], BF, tag="x1t", name="x1t")
                    t2_ = xtp.tile([128, 512], BF, tag="x2t", name="x2t")
                    nc.sync.dma_start(t1_[:], dr["x1t"][i * 128:(i + 1) * 128, :])
                    nc.sync.dma_start(t2_[:], dr["x2t"][i * 128:(i + 1) * 128, :])
                    st = dict(start=(i == 0), stop=(i == NCH - 1))
                    for cb in range(CB):
                        nc.tensor.matmul(G_ps[cb][:], t1_[:, cb * 128:(cb + 1) * 128], t2_[:], **st)
                    nc.tensor.matmul(pp1[:], onescol[:], t1_[:], **st)
                    nc.tensor.matmul(pp2[:], onescol[:], t2_[:], **st)

                G_sb = [a1sb.tile([128, 512], F32R, tag=f"Gsb_{cb}", name=f"Gsb_{cb}") for cb in range(CB)]
                for cb in range(CB):
                    nc.vector.tensor_copy(G_sb[cb][:], G_ps[cb][:])
            nc.vector.tensor_copy(poolf1[:], pp1[:])
            nc.vector.tensor_copy(poolf2[:], pp2[:])

        # all sandwich weights ride in recycled xt-pool slots; the
        # FIFO slot rotation sequences their DMAs behind the G tail
        # in consumption order (M2 -> M1 -> S2 -> S1)
        wq_b2 = [xtp.tile([128, 512], F32R, tag="wqt", name=f"wqb2_{cb}") for cb in range(CB)]
        wq_b1 = [xtp.tile([128, 512], F32R, tag="wqt", name=f"wqb1_{cb}") for cb in range(CB)]
        wk_b2 = [wkp.tile([128, 512], F32R, tag=f"wkb2_{cb}", name=f"wkb2_{cb}") for cb in range(CB)]
        wk_b1 = [xtp.tile([128, 512], F32R, tag="wqt", name=f"wkb1_{cb}") for cb in range(CB)]
        for cb in range(CB):
            cs = slice(cb * 128, (cb + 1) * 128)
            nc.sync.dma_start(wq_b2[cb][:], dr["wq1t"][cs, :])
            nc.sync.dma_start(wq_b1[cb][:], dr["wq2t"][cs, :])
        for cb in range(CB):
            cs = slice(cb * 128, (cb + 1) * 128)
            nc.sync.dma_start(wk_b2[cb][:], dr["wk2t"][cs, :])
            nc.sync.dma_start(wk_b1[cb][:], dr["wk1t"][cs, :])

        # v-weights land before the bulk x-hi loads: the ZT
        # matmuls need them right after the softmax
        for cb in range(CB):
            nc.sync.dma_start(wv2[cb][:], dr["wv2n"][cb * 128:(cb + 1) * 128, :])
            nc.sync.dma_start(wv1[cb][:], dr["wv1n"][cb * 128:(cb + 1) * 128, :])

        # x-hi streaming tiles, requested in apply consumption order
        def req_xh(inp, cib, nt):
            t = xhp.tile([128, 512], BF, tag="xht", name=f"xh_{inp}_{cib}_{nt}")
            src = dr["x1h"] if inp == "x1" else dr["x2h"]
            nc.sync.dma_start(t[:], src[cib * 128:(cib + 1) * 128,
                                        nt * 512:(nt + 1) * 512])
            xht[(inp, cib, nt)] = t
        for nt in range(3):
            for cib in range(CB):
                req_xh("x2", cib, nt)
        for nt in range(3):
            for cib in range(CB):
                req_xh("x1", cib, nt)
        for nt in range(3, NT):
            for cib in range(CB):
                req_xh("x2", cib, nt)
            for cib in range(CB):
                req_xh("x1", cib, nt)

        # --- transpose G -> GT (for branch 1) ---
        GT_sb = [a1sb.tile([128, 512], F32R, tag=f"GTsb_{cb}", name=f"GTsb_{cb}") for cb in range(CB)]
        with tc.tile_pool(name="trp", bufs=2, space="PSUM") as trp:
            for c2b in range(CB):
                for c1b in range(CB):
                    tp = trp.tile([128, 128], F32R, tag="tr", name="tr")
                    nc.tensor.transpose(tp[:], G_sb[c1b][:, c2b * 128:(c2b + 1) * 128], ident[:])
                    nc.vector.tensor_copy(GT_sb[c2b][:, c1b * 128:(c1b + 1) * 128], tp[:])

        wlf = a1sb.tile([1, 512], F32, tag="wlf", name="wlf")
        nc.sync.dma_start(wlf[:], dr["wlinf"][:])

        # --- branch sandwiches + exp ---
        # branch 1: S1 = wk1 (G wq2^T)   via lhsT=GT, then lhsT=wk1t
        # branch 2: S2 = wk2 (G^T wq1^T) via lhsT=G,  then lhsT=wk2t
        rs_all = {}
        branches = [(G_sb, wq_b2, wk_b2, P2), (GT_sb, wq_b1, wk_b1, P1)]
        M_sbs = {}
        with tc.tile_pool(name="msps", bufs=1, space="PSUM") as msps:
            for bi, (Gl, wq, wk, Pt) in enumerate(branches):
                M_ps = [msps.tile([128, 512], F32, tag=f"b{bi}_{cb}", name=f"M{bi}_{cb}") for cb in range(CB)]
                for cb in range(CB):
                    for kb in range(CB):
                        nc.tensor.matmul(M_ps[cb][:], Gl[kb][:, cb * 128:(cb + 1) * 128],
                                         wq[kb][:], start=(kb == 0), stop=(kb == CB - 1))
                M_sb = [a1sb.tile([128, 512], F32R, tag=f"Msb{bi}_{cb}", name=f"Msb{bi}_{cb}") for cb in range(CB)]
                for cb in range(CB):
                    nc.vector.tensor_copy(M_sb[cb][:], M_ps[cb][:])
                M_sbs[bi] = M_sb
            # S tiles reuse the same tags as the M banks they replace
            for bi, (Gl, wq, wk, Pt) in enumerate(branches):
                M_sb = M_sbs[bi]
                S_ps = [msps.tile([128, 512], F32, tag=f"b{bi}_{kb}", name=f"S{bi}_{kb}") for kb in range(CB)]
                for kb in range(CB):
                    for cb in range(CB):
                        nc.tensor.matmul(S_ps[kb][:], wk[cb][:, kb * 128:(kb + 1) * 128],
                                         M_sb[cb][:], start=(cb == 0), stop=(cb == CB - 1))
                for kb in range(CB):
                    nmx = a1sb.tile([128, 1], F32, tag="nmx", name="nmx", bufs=2)
                    nc.vector.reduce_max(nmx[:], S_ps[kb][:], axis=AX.X, negate=True)
                    rs = a1sb.tile([128, 1], F32, tag=f"rs{bi}_{kb}", name=f"rs{bi}_{kb}")
                    nc.scalar.activation(Pt[kb][:], S_ps[kb][:],
                                         ACTF.Exp,
                                         bias=nmx[:], accum_out=rs[:])
                    rs_all[(bi, kb)] = rs
                # gate sigmoid for this branch (tiny DVE+ACT): queues
                # right behind this branch's exps; the broadcast matmul
                # happens in the ZT loop so it never head-blocks the PE
                pf = poolf2 if bi == 0 else poolf1
                pm = a1sb.tile([1, 512], F32, tag=f"pm{bi}", name=f"pm{bi}")
                nc.vector.tensor_mul(pm[:], pf[:], wlf[:])
                prs = a1sb.tile([1, 1], F32, tag=f"prs{bi}", name=f"prs{bi}")
                nc.vector.reduce_sum(prs[:], pm[:], axis=AX.X)
                av = pwv.tile([1, 2], F32R, tag=f"av{bi}", name=f"av{bi}")
                nc.scalar.activation(av[:], prs[:].to_broadcast((1, 2)),
                                     ACTF.Sigmoid, scale=1.0 / float(N))
                av_all[bi] = av
                # fold 1/rowsum into this branch's P right away so the ZT
                # matmuls never wait on the other branch's softmax
                for kb in range(CB):
                    ri = a1sb.tile([128, 1], F32, tag="ri", name="ri", bufs=2)
                    nc.vector.reciprocal(ri[:], rs_all[(bi, kb)][:])
                    nc.vector.tensor_scalar_mul(Pt[kb][:], Pt[kb][:], ri[:])

        # preload the sqrt ACT table set now (it also contains Copy/
        # Square/Relu, i.e. everything phase B + finalize uses) so the
        # BN-finalize tail pays no table switch.
        sqd = a1sb.tile([1, 1], F32, tag="sqd", name="sqd")
        nc.scalar.activation(sqd[:], poolf1[0:1, 0:1], ACTF.Sqrt)


    # ------------ ZT for both branches: ZT = a * (wv^T P) + I -----------
    # re-associated: ZT[ci,c] = a * sum_kc wv[kc,ci] P[kc,c] + I[ci,c]
    # then out[c,n] = sum_ci ZT[ci,c] x[ci,n] includes gate AND residual.
    ZT_all = []
    with tc.tile_pool(name="zps", bufs=1, space="PSUM") as zps:
        for br_i, (Pt, wv, abc) in enumerate([(P2, wv2, abc2), (P1, wv1, abc1)]):
            bc_ps = zps.tile([128, 512], F32, tag="bcg", name=f"bc{br_i}", bufs=2)
            nc.tensor.matmul(bc_ps[:, 0:2], onesg[:], av_all[br_i][:], start=True, stop=True)
            nc.vector.tensor_copy(abc[:], bc_ps[:, 0:1])
            ZT_sb = []
            for cib in range(CB):
                z_ps = zps.tile([128, 512], F32, tag=f"zps_{cib}", name=f"zps_{br_i}_{cib}")
                for kb in range(CB):
                    nc.tensor.matmul(z_ps[:], wv[kb][:, cib * 128:(cib + 1) * 128],
                                     Pt[kb][:], start=(kb == 0), stop=(kb == CB - 1))
                zt = zsbp.tile([128, 512], BF, tag=f"zt_{br_i}_{cib}", name=f"zt_{br_i}_{cib}")
                nc.vector.tensor_scalar_mul(zt[:], z_ps[:], abc[:])
                cs = slice(cib * 128, (cib + 1) * 128)
                nc.vector.tensor_add(zt[:, cs], zt[:, cs], identb[:])
                ZT_sb.append(zt)
            ZT_all.append(ZT_sb)
    pwv.release()

    # winograd transform pools open early so stage1/2 of sp 0 can overlap
    # the apply tail
    rp = tc.alloc_tile_pool(name="rp", bufs=1)
    vp = tc.alloc_tile_pool(name="vp", bufs=2)
    up = tc.alloc_tile_pool(name="up", bufs=3)

    # k index = pr*8 + src (matches the host U layout)
    def emit_stage1(sp):
        """rows transform: R_all[128, 32k, TR, 2, 33] bf16 (DVE)"""
        r0 = 16 * sp
        rt = rp.tile([128, 32, TR, 2, 33], BF, tag="R_all", name=f"R_all_{sp}")
        a = pad_all[:, :, r0 + 0:r0 + 16:2]
        b = pad_all[:, :, r0 + 2:r0 + 18:2]
        c = pad_all[:, :, r0 + 1:r0 + 17:2]
        d = pad_all[:, :, r0 + 3:min(r0 + 19, 66):2]
        nc.vector.tensor_sub(rt[:, 0:8], a, b)
        nc.vector.tensor_add(rt[:, 8:16], c, b)
        nc.vector.tensor_sub(rt[:, 16:24], b, c)
        nc.vector.tensor_sub(rt[:, 24:32], c, d)
        return rt

    def emit_stage2(rt, sp, pc, gp_k):
        """cols transform: V_all[128, 32k, TT] bf16 (DVE + GPSIMD).
        E/O pad layout makes all four operands stride-1 -> DVE 2x."""
        vt = vp.tile([128, 32, TT], BF, tag="V_all", name=f"V_all_{sp}_{pc}")
        vv = vt.rearrange("p k (a b) -> p k a b", a=TR)
        e = rt[:, :, :, 0, 0:32]
        m = rt[:, :, :, 1, 0:32]
        q = rt[:, :, :, 0, 1:33]
        s = rt[:, :, :, 1, 1:33]
        lo = slice(0, 32 - gp_k)
        hi = slice(32 - gp_k, 32)
        if pc == 0:
            nc.vector.tensor_sub(vv[:, lo], e[:, lo], q[:, lo])
            if gp_k:
                nc.gpsimd.tensor_sub(vv[:, hi], e[:, hi], q[:, hi])
        elif pc == 1:
            nc.vector.tensor_add(vv[:, lo], m[:, lo], q[:, lo])
            if gp_k:
                nc.gpsimd.tensor_add(vv[:, hi], m[:, hi], q[:, hi])
        elif pc == 2:
            nc.vector.tensor_sub(vv[:, lo], q[:, lo], m[:, lo])
            if gp_k:
                nc.gpsimd.tensor_sub(vv[:, hi], q[:, hi], m[:, hi])
        else:
            nc.vector.tensor_sub(vv[:, lo], m[:, lo], s[:, lo])
            if gp_k:
                nc.gpsimd.tensor_sub(vv[:, hi], m[:, hi], s[:, hi])
        return vt

    def gp_k_for(sp, pc):
        # keep the gpsimd queue empty near the tail so the stats
        # collective trigger is never stuck behind transform work
        return 0 if (sp == 3 and pc >= 2) else 4

    # ------------ apply (nt-major): pad[c,n] = sum_ci ZT[ci,c] x[ci,n] --
    opsp = tc.alloc_tile_pool(name="ops", bufs=1, space="PSUM")
    apply_seq = [(br, nt) for nt in range(3) for br in [0]] \
        + [(br, nt) for nt in range(3) for br in [1]]
    apply_tail = [(br, nt) for nt in range(3, NT) for br in (0, 1)]
    g_ctr = [0]

    def apply_group(br_i, nt, act_both):
        inp = "x2" if br_i == 0 else "x1"
        pad_base = 4 if br_i == 0 else 0
        rows = slice(1 + nt * 8, 9 + nt * 8)
        for cb in range(CB):
            g = g_ctr[0]
            g_ctr[0] += 1
            o_ps = opsp.tile([128, 512], F32, tag=f"ops_{g % 4}", name=f"ops_{br_i}_{cb}_{nt}")
            for cib in range(CB):
                nc.tensor.matmul(o_ps[:], ZT_all[br_i][cib][:, cb * 128:(cb + 1) * 128],
                                 xht[(inp, cib, nt)][:], start=(cib == 0), stop=(cib == CB - 1))
            src3 = o_ps.rearrange("p (a b) -> p a b", a=8)
            # src col jc (0-based, img col jc+1): even jc -> odd img col
            # -> parity 1 idx jc/2; odd jc -> even img col -> parity 0.
            dst_o = pad_all[:, pad_base + cb, rows, 1, 0:32]
            dst_e = pad_all[:, pad_base + cb, rows, 0, 1:33]
            nc.scalar.activation(dst_o, src3[:, :, 0:64:2], ACTF.Copy)
            if act_both:
                nc.scalar.activation(dst_e, src3[:, :, 1:64:2], ACTF.Copy)
            else:
                nc.vector.tensor_copy(dst_e, src3[:, :, 1:64:2])

    for br_i, nt in apply_seq:
        apply_group(br_i, nt, act_both=False)
    # stage1/2 for sp 0 run on DVE during the apply tail; the tail's pad
    # copies go ACT-only while the DVE chews the transforms
    R = emit_stage1(0)
    V = emit_stage2(R, 0, 0, gp_k_for(0, 0))
    for br_i, nt in apply_tail:
        apply_group(br_i, nt, act_both=(nt in (3, 4)))
    opsp.release()
    zsbp.release()
    xhp.release()

    # ------------ Phase B: Winograd F(2x2,3x3) conv + BN ----------------
    ybp = tc.alloc_tile_pool(name="ybp", bufs=1, side="right")
    bsb = tc.alloc_tile_pool(name="bsb", bufs=1, side="right")
    dram = tc.alloc_tile_pool(name="dram", bufs=1, space="DRAM")
    t1p = tc.alloc_tile_pool(name="t1p", bufs=2)
    map_ = tc.alloc_tile_pool(name="map", bufs=2)
    mps = tc.alloc_tile_pool(name="mps", bufs=2, space="PSUM")

    # conv output in Winograd block layout: [128, ocb, sp, r, j, 256]
    yb_all = ybp.tile([128, CB, NSP, 2, 2, TT], BF, tag="yb_all", name="yb_all")

    stats_a = bsb.tile([128, 2 * CB], F32, tag="stats_a", name="stats_a")
    stats_b = bsb.tile([128, 2 * CB], F32, tag="stats_b", name="stats_b")
    nc.vector.memset(stats_a[:], 0.0)
    nc.vector.memset(stats_b[:], 0.0)
    s_in1 = dram.tile([128, 2 * CB], F32, tag="arin1", name="arin1")
    s_out1 = dram.tile([128, 2 * CB], F32, tag="arout1", name="arout1")
    s_in2 = dram.tile([128, 2 * CB], F32, tag="arin2", name="arin2")
    s_out2 = dram.tile([128, 2 * CB], F32, tag="arout2", name="arout2")

    def emit_stats(dst_stats, ob, ysl, nel):
        ts = bsb.tile([128, 1], F32, tag="tsum", name="tsum", bufs=2)
        sc1 = bsb.tile([128, nel], BF, tag="scr", name="scr", bufs=1)
        nc.scalar.activation(sc1[:], ysl, ACTF.Copy, accum_out=ts[:])
        tq = bsb.tile([128, 1], F32, tag="tsq", name="tsq", bufs=2)
        sc2 = bsb.tile([128, nel], BF, tag="scr2", name="scr2", bufs=1)
        nc.scalar.activation(sc2[:], ysl, ACTF.Square, accum_out=tq[:])
        nc.vector.tensor_add(dst_stats[:, 2 * ob:2 * ob + 1], dst_stats[:, 2 * ob:2 * ob + 1], ts[:])
        nc.vector.tensor_add(dst_stats[:, 2 * ob + 1:2 * ob + 2], dst_stats[:, 2 * ob + 1:2 * ob + 2], tq[:])

    phases = [(sp, pc) for sp in range(NSP) for pc in range(4)]
    for idx, (sp, pc) in enumerate(phases):
        Vcur = V
        # ---- PE: the 16-position matmuls for this (sp, pc) ----
        mts = []
        for pair in range(2):
            # M PSUM for an ocb pair: [128, 4pr, 2x256] f32
            mt = mps.tile([128, 4, 2 * TT], F32, tag="mt", name=f"mt_{sp}_{pc}_{pair}")
            for half in range(2):
                ocb = pair * 2 + half
                u = up.tile([128, 32 * 128], BF, tag="u", name=f"u_{sp}_{pc}_{ocb}")
                nc.sync.dma_start(u[:], dr["uw"][pc * 4 + ocb])
                hs = slice(half * TT, (half + 1) * TT)
                for icb in range(8):
                    st = dict(start=(icb == 0), stop=(icb == 7))
                    for pr in range(4):
                        nc.tensor.matmul(mt[:, pr, hs],
                                         u[:, (pr * 8 + icb) * 128:(pr * 8 + icb + 1) * 128],
                                         Vcur[:, pr * 8 + icb, :], **st)
            mts.append(mt)

        # ---- DVE: pre-emit NEXT phase transforms (FIFO order) ----
        if idx + 1 < len(phases):
            sp2, pc2 = phases[idx + 1]
            if pc2 == 0:
                R = emit_stage1(sp2)
            V = emit_stage2(R, sp2, pc2, gp_k_for(sp2, pc2))

        # ---- inverse transforms for this phase ----
        for pair in range(2):
            mt = mts[pair]
            ph = slice(pair * 2, pair * 2 + 2)
            # rows (invA): PSUM reads all on ACT, adds on DVE (bf16 2x)
            m0 = map_.tile([128, 2 * TT], BF, tag="m0", name=f"m0_{sp}_{pc}_{pair}")
            m1 = map_.tile([128, 2 * TT], BF, tag="m1", name=f"m1_{sp}_{pc}_{pair}")
            m2 = map_.tile([128, 2 * TT], BF, tag="m2", name=f"m2_{sp}_{pc}_{pair}")
            m3 = map_.tile([128, 2 * TT], BF, tag="m3", name=f"m3_{sp}_{pc}_{pair}")
            nc.scalar.activation(m0[:], mt[:, 0, :], ACTF.Copy)
            nc.scalar.activation(m1[:], mt[:, 1, :], ACTF.Copy)
            nc.scalar.activation(m2[:], mt[:, 2, :], ACTF.Copy)
            nc.scalar.activation(m3[:], mt[:, 3, :], ACTF.Copy)
            t1 = t1p.tile([128, 2, 2 * TT], BF, tag="t1", name=f"t1_{sp}_{pc}_{pair}")
            nc.vector.tensor_add(t1[:, 0, :], m0[:], m1[:])
            nc.vector.tensor_add(t1[:, 0, :], t1[:, 0, :], m2[:])
            nc.vector.tensor_sub(t1[:, 1, :], m1[:], m2[:])
            nc.vector.tensor_sub(t1[:, 1, :], t1[:, 1, :], m3[:])
            # cols (invB): ops span the ocb pair (FD 512)
            for r in range(2):
                tr_ = t1[:, r, :].rearrange("p (o t) -> p o t", o=2)
                y0 = yb_all[:, ph, sp, r, 0, :]
                y1 = yb_all[:, ph, sp, r, 1, :]
                if pc == 0:
                    nc.scalar.activation(y0, tr_, ACTF.Copy)
                elif pc == 1:
                    nc.vector.tensor_add(y0, y0, tr_)
                    nc.scalar.activation(y1, tr_, ACTF.Copy)
                elif pc == 2:
                    nc.vector.tensor_add(y0, y0, tr_)
                    nc.vector.tensor_sub(y1, y1, tr_)
                else:
                    nc.vector.tensor_sub(y1, y1, tr_)
            if sp == 3 and pc >= 2:
                jj = pc - 2   # j=0 final after pc2, j=1 after pc3
                for ob in (pair * 2, pair * 2 + 1):
                    ysl = yb_all[:, ob, 3, :, jj, :]
                    emit_stats(stats_b, ob, ysl, 2 * TT)

        # ---- BN stats; sp3 split by j so the tail only waits on j=1 ----
        if sp < 3 and pc == 3:
            for ob in range(CB):
                ysl = yb_all[:, ob, sp].rearrange("p a b t -> p (a b t)")
                emit_stats(stats_a, ob, ysl, 4 * TT)
        elif sp == 3 and pc == 1:
            # early collective: sp 0-2 stats; doubles as a barrier late
            # enough that the final (sp3) collective sees little skew
            nc.sync.dma_start(s_in1[:], stats_a[:])
            nc.gpsimd.collective_compute(
                "AllReduce", mybir.AluOpType.add,
                replica_groups=[list(range(N_CORES))],
                ins=[s_in1.opt()], outs=[s_out1.opt()])


    nc.sync.dma_start(s_in2[:], stats_b[:])
    nc.gpsimd.collective_compute(
        "AllReduce", mybir.AluOpType.add,
        replica_groups=[list(range(N_CORES))],
        ins=[s_in2.opt()], outs=[s_out2.opt()])

    mps.release()
    map_.release()
    t1p.release()
    up.release()
    vp.release()
    rp.release()

    # ---- finalize: scale/shift for all 4 ocb at once, then repack ------
    with tc.tile_pool(name="fin", bufs=1) as fin:
        sall1 = fin.tile([128, 2 * CB], F32, tag="sall1", name="sall1")
        sall2 = fin.tile([128, 2 * CB], F32, tag="sall2", name="sall2")
        nc.sync.dma_start(sall1[:], s_out1[:])
        nc.sync.dma_start(sall2[:], s_out2[:])
        sall = fin.tile([128, 2 * CB], F32, tag="sall", name="sall")
        nc.vector.tensor_add(sall[:], sall1[:], sall2[:])

        gam = fin.tile([128, CB], F32, tag="gam", name="gam")
        bet = fin.tile([128, CB], F32, tag="bet", name="bet")
        nc.sync.dma_start(gam[:], dr["gamma"].rearrange("(c p) one -> p (c one)", p=128))
        nc.sync.dma_start(bet[:], dr["beta"].rearrange("(c p) one -> p (c one)", p=128))
        inv_n = 1.0 / float(B * N)
        eps_t = fin.tile([128, 1], F32, tag="eps", name="eps")
        nc.vector.memset(eps_t[:], BN_EPS)

        mean4 = fin.tile([128, CB], F32, tag="mean4", name="mean4")
        nc.vector.tensor_scalar_mul(mean4[:], sall[:, 0:2 * CB:2], inv_n)
        ex24 = fin.tile([128, CB], F32, tag="ex24", name="ex24")
        nc.vector.tensor_scalar_mul(ex24[:], sall[:, 1:2 * CB:2], inv_n)
        m2s = fin.tile([128, CB], F32, tag="m2s", name="m2s")
        nc.vector.tensor_mul(m2s[:], mean4[:], mean4[:])
        var4 = fin.tile([128, CB], F32, tag="var4", name="var4")
        nc.vector.tensor_sub(var4[:], ex24[:], m2s[:])
        std4 = fin.tile([128, CB], F32, tag="std4", name="std4")
        nc.scalar.activation(std4[:], var4[:], ACTF.Sqrt, bias=eps_t[:])
        inv4 = fin.tile([128, CB], F32, tag="inv4", name="inv4")
        nc.vector.reciprocal(inv4[:], std4[:])
        sc4 = fin.tile([128, CB], F32, tag="sc4", name="sc4")
        nc.vector.tensor_mul(sc4[:], gam[:], inv4[:])
        ms4 = fin.tile([128, CB], F32, tag="ms4", name="ms4")
        nc.vector.tensor_mul(ms4[:], mean4[:], sc4[:])
        tt4 = fin.tile([128, CB], F32, tag="tt4", name="tt4")
        nc.vector.tensor_sub(tt4[:], bet[:], ms4[:])

        # normalize + ReLU + repack block layout -> row-major; ACT takes
        # 3 output chunks, DVE the 4th; output DMA per (ob, sp) slab
        for ob in [3, 0, 1, 2]:
            onat = fin.tile([128, 64, 64], BF, tag="onat", name=f"onat_{ob}", bufs=4)
            for sp in range(NSP):
                for r in range(2):
                    src = yb_all[:, ob, sp, r].rearrange("p j (a b) -> p j a b", a=TR)
                    dst = onat[:, 16 * sp + r:min(16 * sp + r + 16, 64):2, :]
                    dst = dst.rearrange("p a (b j) -> p j a b", j=2)
                    if ob < 3:
                        nc.scalar.activation(dst, src, ACTF.Relu,
                                             bias=tt4[:, ob:ob + 1], scale=sc4[:, ob:ob + 1])
                    else:
                        nc.vector.tensor_scalar(dst, src, sc4[:, ob:ob + 1], tt4[:, ob:ob + 1],
                                                ALU.mult, ALU.add)
                if ob >= 3:
                    pl = onat[:, 16 * sp:16 * sp + 16, :]
                    nc.vector.tensor_scalar_max(pl, pl, 0.0)
                nc.sync.dma_start(
                    dr["yout"][ob * 128:(ob + 1) * 128, sp * 1024:(sp + 1) * 1024],
                    onat[:, 16 * sp:16 * sp + 16, :].rearrange("p a b -> p (a b)"))

    bsb.release()
    ybp.release()
    dram.release()
    pads_pool.release()


def _build():
    if "nc" in _CACHE:
        return _CACHE["nc"]
    nc = bacc.Bacc("TRN2", target_bir_lowering=False, debug=False,
                   num_devices=N_CORES)
    dr = {}
    def din(name, shape, dt):
        dr[name] = nc.dram_tensor(name, shape, dt, kind="ExternalInput").ap()
    din("x1t", [N, C], BF)
    din("x2t", [N, C], BF)
    din("x1h", [C, N], BF)
    din("x2h", [C, N], BF)
    for w in ["wq1t", "wq2t", "wk1t", "wk2t"]:
        din(w, [C, C], F32R)
    for w in ["wv1n", "wv2n"]:
        din(w, [C, C], BF)
    din("wlinf", [1, C], F32)
    din("uw", [16, 128, 32 * 128], BF)
    din("gamma", [OUT, 1], F32)
    din("beta", [OUT, 1], F32)
    din("ident", [128, 128], F32R)
    din("identb", [128, 128], BF)
    din("ones", [128, 128], F32R)
    din("onesbf", [128, 1], BF)
    dr["yout"] = nc.dram_tensor("yout", [OUT, N], BF, kind="ExternalOutput").ap()

    with tile.TileContext(nc) as tc:
        _emit(nc, tc, dr)
    nc.compile()
    _CACHE["nc"] = nc
    return nc


def _prep_in_maps(inputs):
    f32 = np.float32
    x1 = np.ascontiguousarray(inputs["input1"], f32).reshape(B, C, N)
    x2 = np.ascontiguousarray(inputs["input2"], f32).reshape(B, C, N)
    shared = {}
    for w in ["wq1", "wq2", "wk1", "wk2"]:
        shared[w + "t"] = np.ascontiguousarray(np.asarray(inputs[w], f32).T)
    for w in ["wv1", "wv2"]:
        shared[w + "n"] = np.ascontiguousarray(np.asarray(inputs[w], f32).astype(BF16))
    shared["wlinf"] = np.ascontiguousarray(np.asarray(inputs["w_lin"], f32).reshape(1, C))
    # Winograd weight transform on host: U[pr,pc][ic,oc] = G g G^T
    g = np.asarray(inputs["w_cat"], f32)                     # [OUT, 2C, 3, 3]
    Gm = np.array([[1, 0, 0], [0.5, 0.5, 0.5], [0.5, -0.5, 0.5], [0, 0, 1]], f32)
    U = np.einsum('rj,oijk,ck->rcio', Gm, g, Gm)             # [4,4,2C,OUT]
    # layout: uw[pc*4+ocb][ic_in_chunk][pr, icb, oc] as [16, 128, 4096]
    U6 = U.reshape(4, 4, 8, 128, 4, 128)                     # [pr,pc,icb,i,ocb,o]
    uw = np.ascontiguousarray(U6.transpose(1, 4, 3, 0, 2, 5).reshape(4, 4, 128, 32 * 128))
    # uw dims now [pc, ocb, i, (pr icb o)]
    shared["uw"] = np.ascontiguousarray(uw.reshape(16, 128, 32 * 128).astype(BF16))
    shared["gamma"] = np.ascontiguousarray(np.asarray(inputs["bn_gamma"], f32).reshape(OUT, 1))
    shared["beta"] = np.ascontiguousarray(np.asarray(inputs["bn_beta"], f32).reshape(OUT, 1))
    shared["ident"] = np.eye(128, dtype=f32)
    shared["identb"] = np.eye(128, dtype=f32).astype(BF16)
    shared["ones"] = np.ones((128, 128), f32)
    shared["onesbf"] = np.ones((128, 1), f32).astype(BF16)

    in_maps = []
    for b in range(B):
        m = dict(shared)
        m["x1t"] = np.ascontiguousarray(x1[b].T.astype(BF16))
        m["x2t"] = np.ascontiguousarray(x2[b].T.astype(BF16))
        m["x1h"] = np.ascontiguousarray(x1[b].astype(BF16))
        m["x2h"] = np.ascontiguousarray(x2[b].astype(BF16))
        in_maps.append(m)
    return in_maps


def run(inputs, trace=False):
    nc = _build()
    in_maps = _prep_in_maps(inputs)
    res = bass_utils.run_bass_kernel_spmd(nc, in_maps, list(range(N_CORES)),
                                          trace=trace)
    out = np.stack([np.asarray(res.results[b]["yout"], dtype=np.float32) for b in range(B)])
    return out.reshape(B, OUT, H, W), res


def kernel(**inputs):
    out, _ = run(inputs, trace=bool(int(os.environ.get("BASS_KERNEL_TRACE", "0"))))
    return out
